# revision 1
# baseline (speedup 1.0000x reference)
"""ConvLinformer forward on 8 Trainium2 NeuronCores (Bass/Tile).

Sharding: 8-way over sequence (512 tokens/core/batch). Weights replicated,
except the conv kernels [O,C,S] which are channel(C)-sliced per core; the
conv contraction is channel-sharded via an AllToAll of the ke/ve activations
followed by an AllReduce of the k_/v_ partials (the Linformer layers use the
same AllReduce for their sequence-projection partials).

Layout: residual stream kept feature-major in SBUF: x^T = [128, (dt:8, b:2,
tl:512)] (partition = feature-within-tile). Matmuls in fp32r (FP22 multiply,
fp32 accumulate, full PE rate at free>=256); conv + FFN-w2 paths in bf16.

Self-contained: shapes hardcoded; host shards inputs / gathers outputs.
"""

import numpy as np

import concourse.bacc as bacc
import concourse.mybir as mybir
import concourse.tile as tile
from concourse.bass_utils import run_bass_kernel_spmd
from concourse.masks import make_identity

P = 128
B, N, D, H, DH, K, S, DFF, L = 2, 4096, 1024, 8, 128, 256, 16, 4096, 2
NC = 8
NL = N // NC          # 512 local tokens per batch
T = B * NL            # 1024 local tokens, free layout (b, tl)
DT = D // P           # 8 feature tiles
DFT = DFF // P        # 32 dff tiles
KT = K // P           # 2 kv-position tiles
SCALE = float(DH) ** -0.5

F32 = mybir.dt.float32
F32R = mybir.dt.float32r
BF16 = mybir.dt.bfloat16
AX = mybir.AxisListType
OP = mybir.AluOpType
AF = mybir.ActivationFunctionType

PARAM_NAMES = [
    "ln1_g", "ln1_b", "wq", "wk", "wv", "pk", "pv", "wo", "bo",
    "ln2_g", "ln2_b", "w1", "b1", "w2", "b2",
]


def _declare_io(nc):
    d = {}
    d["x_local"] = nc.dram_tensor("x_local", [B, NL, D], F32, kind="ExternalInput").ap()
    for kind in ("lin", "conv"):
        for li in range(L):
            pre = f"{kind}{li}_"
            for v in ("ln1_g", "ln1_b", "bo", "ln2_g", "ln2_b", "b2"):
                d[pre + v] = nc.dram_tensor(pre + v, [D], F32, kind="ExternalInput").ap()
            d[pre + "b1"] = nc.dram_tensor(pre + "b1", [DFF], F32, kind="ExternalInput").ap()
            for w in ("wq", "wk", "wv", "wo"):
                d[pre + w] = nc.dram_tensor(pre + w, [D, D], F32, kind="ExternalInput").ap()
            d[pre + "w1"] = nc.dram_tensor(pre + "w1", [D, DFF], F32, kind="ExternalInput").ap()
            d[pre + "w2"] = nc.dram_tensor(pre + "w2", [DFF, D], F32, kind="ExternalInput").ap()
            if kind == "lin":
                d[pre + "pk"] = nc.dram_tensor(pre + "pk", [NL, K], F32, kind="ExternalInput").ap()
                d[pre + "pv"] = nc.dram_tensor(pre + "pv", [NL, K], F32, kind="ExternalInput").ap()
            else:
                # channel-sliced conv kernels: [O, 128(c-slice), S]
                d[pre + "pk"] = nc.dram_tensor(pre + "pk", [D, P, S], F32, kind="ExternalInput").ap()
                d[pre + "pv"] = nc.dram_tensor(pre + "pv", [D, P, S], F32, kind="ExternalInput").ap()
    d["y_local"] = nc.dram_tensor("y_local", [B, NL, D], F32, kind="ExternalOutput").ap()
    return d


class Ctx:
    def __init__(self, nc, tc, io):
        self.nc, self.tc, self.io = nc, tc, io


def _load_col(ctx, dram_vec, width, pool, name):
    """Load a [width*128] dram vector as a [128, width] column tile (f32)."""
    nc = ctx.nc
    t = pool.tile([P, width], F32, name=name)
    nc.sync.dma_start(t[:], dram_vec.rearrange("(w p) -> p w", p=P))
    return t


def _layernorm(ctx, x, g_col, b_col, xn, pfx):
    """xn = LN(x) * g + b, feature-major [128, DT*T] f32r."""
    nc, tc = ctx.nc, ctx.tc
    with (
        tc.tile_pool(name=pfx + "sb", bufs=2) as sb,
        tc.tile_pool(name=pfx + "xq", bufs=1) as xqp,
        tc.tile_pool(name=pfx + "ps", bufs=2, space="PSUM") as ps,
        tc.tile_pool(name=pfx + "bps", bufs=2, space="PSUM") as bps,
    ):
        xsq = xqp.tile([P, DT * T], F32R, name="xsq")
        nc.vector.tensor_mul(xsq[:], x[:].bitcast(F32), x[:].bitcast(F32))
        s0_row = sb.tile([1, T], F32R, name="s0row", bufs=1)
        s1_row = sb.tile([1, T], F32R, name="s1row", bufs=1)
        for c in range(2):  # token chunks of 512 (c == batch)
            st1 = ps.tile([1, 512], F32, tag="st")
            st2 = ps.tile([1, 512], F32, tag="st")
            for dt in range(DT):
                nc.tensor.matmul(st1[:], ctx.ones_col[:], x[:, dt * T + c * 512:][:, :512],
                                 start=(dt == 0), stop=(dt == DT - 1))
            for dt in range(DT):
                nc.tensor.matmul(st2[:], ctx.ones_col[:], xsq[:, dt * T + c * 512:][:, :512],
                                 start=(dt == 0), stop=(dt == DT - 1))
            m_row = sb.tile([1, 512], F32, tag="m")
            nc.vector.tensor_scalar_mul(m_row[:], st1[:], 1.0 / D)
            msq = sb.tile([1, 512], F32, tag="msq")
            nc.vector.tensor_mul(msq[:], m_row[:], m_row[:])
            var = sb.tile([1, 512], F32, tag="var")
            nc.vector.scalar_tensor_tensor(var[:], st2[:], 1.0 / D, msq[:], OP.mult, OP.subtract)
            sd = sb.tile([1, 512], F32, tag="sd")
            nc.scalar.activation(sd[:], var[:], AF.Sqrt, bias=ctx.eps_b[:], scale=1.0)
            r_row = sb.tile([1, 512], F32, tag="r")
            nc.vector.reciprocal(r_row[:], sd[:])
            nc.vector.tensor_copy(s1_row[:, c * 512:][:, :512], r_row[:])
            nc.vector.scalar_tensor_tensor(
                s0_row[:, c * 512:][:, :512], m_row[:], -1.0,
                r_row[:], OP.mult, OP.mult)
        for c in range(2):
            s0bc = bps.tile([P, 512], F32, tag="bc0")
            s1bc = bps.tile([P, 512], F32, tag="bc1")
            nc.tensor.matmul(s0bc[:], ctx.ones_row[:], s0_row[:, c * 512:][:, :512],
                             start=True, stop=True)
            nc.tensor.matmul(s1bc[:], ctx.ones_row[:], s1_row[:, c * 512:][:, :512],
                             start=True, stop=True)
            for dt in range(DT):
                sl = slice(dt * T + c * 512, dt * T + c * 512 + 512)
                p1 = sb.tile([P, 512], F32, tag="p1")
                nc.vector.scalar_tensor_tensor(p1[:], x[:, sl].bitcast(F32),
                                               g_col[:, dt:dt + 1], s1bc[:], OP.mult, OP.mult)
                p2 = sb.tile([P, 512], F32, tag="p2")
                nc.vector.tensor_scalar(p2[:], s0bc[:], g_col[:, dt:dt + 1],
                                        b_col[:, dt:dt + 1], OP.mult, OP.add)
                nc.vector.tensor_add(xn[:, sl], p1[:], p2[:])


def _proj_T(ctx, w_dram, src, out_cb, pfx):
    """Feature-major projection: psum[ot, c] = sum_dt W[dt,ot].T @ src[dt,c]."""
    nc, tc = ctx.nc, ctx.tc
    with (
        tc.tile_pool(name=pfx + "w", bufs=DT + 1) as wp,
        tc.tile_pool(name=pfx + "ps", bufs=3, space="PSUM") as ps,
    ):
        w_sb = []
        for dt in range(DT):
            wt = wp.tile([P, D], F32R, tag="w", name=f"w{dt}")
            nc.sync.dma_start(wt[:], w_dram[dt * P:(dt + 1) * P, :].bitcast(F32R))
            w_sb.append(wt)
        for c in range(2):
            for ot in range(DT):
                pp = ps.tile([P, 512], F32, tag="pj")
                for dt in range(DT):
                    nc.tensor.matmul(pp[:], w_sb[dt][:, ot * P:(ot + 1) * P],
                                     src[:, dt * T + c * 512:][:, :512],
                                     start=(dt == 0), stop=(dt == DT - 1))
                out_cb(ot, c, pp)


def _lin_kv(ctx, li, xn, cc_in, pfx):
    """Linformer: kfull/vfull token-major, then pk/pv projections -> partials."""
    nc, tc = ctx.nc, ctx.tc
    io = ctx.io
    pre = f"lin{li}_"
    with (
        tc.tile_pool(name=pfx + "w", bufs=DT) as wp,
        tc.tile_pool(name=pfx + "kv", bufs=2) as kvp,
        tc.tile_pool(name=pfx + "p", bufs=1) as pp_,
        tc.tile_pool(name=pfx + "ar", bufs=1) as arp,
        tc.tile_pool(name=pfx + "ps", bufs=2, space="PSUM") as ps,
        tc.tile_pool(name=pfx + "ps2", bufs=2, space="PSUM") as ps2,
    ):
        pk_sb = pp_.tile([P, 4 * K], F32R, name="pk_sb")
        nc.sync.dma_start(pk_sb[:].rearrange("p (nt k) -> p nt k", nt=4),
                          io[pre + "pk"].rearrange("(nt p) k -> p nt k", p=P).bitcast(F32R))
        pv_sb = pp_.tile([P, 4 * K], F32R, name="pv_sb")
        nc.sync.dma_start(pv_sb[:].rearrange("p (nt k) -> p nt k", nt=4),
                          io[pre + "pv"].rearrange("(nt p) k -> p nt k", p=P).bitcast(F32R))

        for ten in range(2):  # 0 = k, 1 = v
            wname = pre + ("wk" if ten == 0 else "wv")
            w_sb = []
            for dt in range(DT):
                wt = wp.tile([P, D], F32R, tag="w", name=f"w{dt}")
                nc.sync.dma_start(wt[:], io[wname][dt * P:(dt + 1) * P, :].bitcast(F32R))
                w_sb.append(wt)
            arh = arp.tile([P, 4096], F32, tag="arh", name="arh")
            for b in range(B):
                full = kvp.tile([P, 4 * D], F32R, tag="full", name="full")
                for nt in range(4):
                    for c2 in range(2):
                        fp = ps.tile([P, 512], F32, tag="pf")
                        for dt in range(DT):
                            lhs = xn[:, dt * T + b * 512 + nt * P:][:, :P]
                            nc.tensor.matmul(fp[:], lhs, w_sb[dt][:, c2 * 512:][:, :512],
                                             start=(dt == 0), stop=(dt == DT - 1))
                        nc.vector.tensor_copy(full[:, nt * D + c2 * 512:][:, :512], fp[:])
                if ten == 0:
                    # k_^T partials: [dt][128, K] per b
                    for dt in range(DT):
                        kp = ps2.tile([P, K], F32, tag="kp")
                        for nt in range(4):
                            nc.tensor.matmul(kp[:], full[:, nt * D + dt * P:][:, :P],
                                             pk_sb[:, nt * K:][:, :K],
                                             start=(nt == 0), stop=(nt == 3))
                        nc.vector.tensor_copy(arh[:, b * (DT * K) + dt * K:][:, :K], kp[:])
                else:
                    # v_ token-major partials: [kt][128, D] per b
                    for kt in range(KT):
                        for c2 in range(2):
                            vp = ps2.tile([P, 512], F32, tag="vp")
                            for nt in range(4):
                                nc.tensor.matmul(vp[:], pv_sb[:, nt * K + kt * P:][:, :P],
                                                 full[:, nt * D + c2 * 512:][:, :512],
                                                 start=(nt == 0), stop=(nt == 3))
                            nc.vector.tensor_copy(
                                arh[:, (b * KT + kt) * D + c2 * 512:][:, :512], vp[:])
            nc.sync.dma_start(cc_in[:, ten * 4096:(ten + 1) * 4096], arh[:])


def _conv_kv(ctx, li, a2a_out, cc_in, pfx):
    """Conv: contract local c-slice of the A2A'd ke/ve with sliced kernels."""
    nc, tc = ctx.nc, ctx.tc
    io = ctx.io
    pre = f"conv{li}_"
    with (
        tc.tile_pool(name=pfx + "cs", bufs=1) as csp,
        tc.tile_pool(name=pfx + "wn", bufs=1) as wnp,
        tc.tile_pool(name=pfx + "wt", bufs=1) as wtp,
        tc.tile_pool(name=pfx + "ar", bufs=1) as arp,
        tc.tile_pool(name=pfx + "ps", bufs=2, space="PSUM") as ps,
        tc.tile_pool(name=pfx + "cps", bufs=2, space="PSUM") as cps,
    ):
        # readback: [128c, (b, peer, tl)]
        ke_cs = csp.tile([P, B * N], BF16, tag="kecs", name="ke_cs")
        ve_cs = csp.tile([P, B * N], BF16, tag="vecs", name="ve_cs")
        for ten, dst in ((0, ke_cs), (1, ve_cs)):
            nc.sync.dma_start(
                dst[:].rearrange("c (b j t) -> c b j t", b=B, j=NC),
                a2a_out[:, ten].rearrange("j c (b t) -> c b j t", b=B))
        for ten, wname, ecs in ((0, pre + "pk", ke_cs), (1, pre + "pv", ve_cs)):
            wt_sb = wtp.tile([P, S * D], BF16, tag="wt", name="wt")
            for ot in range(DT):
                wn = wnp.tile([P, P * S], F32R, tag="wn")
                nc.sync.dma_start(
                    wn[:], io[wname][ot * P:(ot + 1) * P].rearrange("o c s -> o (c s)").bitcast(F32R))
                for s4 in range(4):
                    tp_ps = ps.tile([P, 512], F32R, tag="wtp")
                    for si in range(4):
                        s = s4 * 4 + si
                        nc.tensor.transpose(
                            tp_ps[:, si * P:(si + 1) * P],
                            wn[:].rearrange("o (c s) -> o s c", s=S)[:, s],
                            ctx.ident_r[:])
                    nc.vector.tensor_copy(
                        wt_sb[:].rearrange("c (s o) -> c s o", s=S)[:, s4 * 4:(s4 + 1) * 4, ot * P:(ot + 1) * P],
                        tp_ps[:].rearrange("c (si o) -> c si o", si=4).bitcast(F32))
            arh = arp.tile([P, 4096], F32, tag="arh", name="arh")
            if ten == 0:
                # k_^T feature-major: [ot][128, (b, K)]
                for ot in range(DT):
                    kp = cps.tile([P, 512], F32, tag="ck")
                    for s in range(S):
                        rhs = ecs[:].rearrange("c (b j t) -> c b j t", b=B, j=NC)[:, :, :, s::S]
                        nc.tensor.matmul(kp[:].rearrange("o (b k) -> o b k", b=B),
                                         wt_sb[:, s * D + ot * P:][:, :P], rhs,
                                         start=(s == 0), stop=(s == S - 1))
                    nc.vector.tensor_copy(
                        arh[:].rearrange("p (b dt k) -> p b dt k", b=B, dt=DT)[:, :, ot, :],
                        kp[:].rearrange("o (b k) -> o b k", b=B))
            else:
                # v_ token-major: [b, kt][128, D]
                for b in range(B):
                    for kt in range(KT):
                        for c2 in range(2):
                            vp = cps.tile([P, 512], F32, tag="cv")
                            for s in range(S):
                                lhs = ecs[:].rearrange(
                                    "c (b j t) -> c b j t", b=B, j=NC)[:, b, kt * 4:(kt + 1) * 4, s::S]
                                nc.tensor.matmul(vp[:], lhs,
                                                 wt_sb[:, s * D + c2 * 512:][:, :512],
                                                 start=(s == 0), stop=(s == S - 1))
                            nc.vector.tensor_copy(
                                arh[:, (b * KT + kt) * D + c2 * 512:][:, :512],
                                vp[:])
            nc.sync.dma_start(cc_in[:, ten * 4096:(ten + 1) * 4096], arh[:])


def _attention(ctx, qo_sb, kv_sb, pfx):
    """Per (b, h): scores -> softmax -> A^T -> o. o overwrites q's slice."""
    nc, tc = ctx.nc, ctx.tc
    with (
        tc.tile_pool(name=pfx + "sb", bufs=3) as sb,
        tc.tile_pool(name=pfx + "at", bufs=2) as atp,
        tc.tile_pool(name=pfx + "ps", bufs=2, space="PSUM") as ps_s,
        tc.tile_pool(name=pfx + "pt", bufs=2, space="PSUM") as ps_t,
        tc.tile_pool(name=pfx + "po", bufs=2, space="PSUM") as ps_o,
    ):
        for b in range(B):
            for h in range(H):
                at_sb = atp.tile([P, KT * 512], F32R, tag="at", name="at_sb")
                for tt in range(4):
                    sc_ps = ps_s.tile([P, K], F32, tag="sc")
                    nc.tensor.matmul(sc_ps[:],
                                     qo_sb[:, h * T + b * 512 + tt * P:][:, :P],
                                     kv_sb[:, b * (DT * K) + h * K:][:, :K],
                                     start=True, stop=True)
                    mx = sb.tile([P, 1], F32, tag="mx")
                    nc.vector.tensor_reduce(mx[:], sc_ps[:], AX.X, OP.max)
                    negs = sb.tile([P, 1], F32, tag="negs")
                    nc.vector.tensor_scalar_mul(negs[:], mx[:], -SCALE)
                    a_e = sb.tile([P, K], F32, tag="ae")
                    den = sb.tile([P, 1], F32, tag="den")
                    nc.scalar.activation(a_e[:], sc_ps[:], AF.Exp,
                                         bias=negs[:], scale=SCALE, accum_out=den[:])
                    inv = sb.tile([P, 1], F32, tag="inv")
                    nc.vector.reciprocal(inv[:], den[:])
                    a_r = sb.tile([P, K], F32R, tag="ar")
                    nc.vector.tensor_scalar_mul(a_r[:], a_e[:], inv[:])
                    tp = ps_t.tile([P, K], F32R, tag="tp")
                    for kt in range(KT):
                        nc.tensor.transpose(tp[:, kt * P:(kt + 1) * P],
                                            a_r[:, kt * P:(kt + 1) * P], ctx.ident_r[:])
                    nc.vector.tensor_copy(
                        at_sb[:].rearrange("p (kt t) -> p kt t", kt=KT)[:, :, tt * P:(tt + 1) * P],
                        tp[:].rearrange("p (kt t) -> p kt t", kt=KT).bitcast(F32))
                oo = ps_o.tile([P, 512], F32, tag="oo")
                for kt in range(KT):
                    nc.tensor.matmul(oo[:],
                                     kv_sb[:, 4096 + (b * KT + kt) * D + h * P:][:, :P],
                                     at_sb[:, kt * 512:][:, :512],
                                     start=(kt == 0), stop=(kt == KT - 1))
                nc.vector.tensor_copy(qo_sb[:, h * T + b * 512:][:, :512], oo[:])


def _ffn(ctx, pre, x, xn2, clp, pfx):
    nc, tc = ctx.nc, ctx.tc
    io = ctx.io
    with (
        tc.tile_pool(name=pfx + "w2", bufs=1) as w2p,
        tc.tile_pool(name=pfx + "st", bufs=2) as stp,
        tc.tile_pool(name=pfx + "h", bufs=1) as hp,
        tc.tile_pool(name=pfx + "w1", bufs=DT + 2) as w1p,
        tc.tile_pool(name=pfx + "ph", bufs=3, space="PSUM") as ps_h,
        tc.tile_pool(name=pfx + "pf", bufs=2, space="PSUM") as ps_f,
    ):
        b1_col = _load_col(ctx, io[pre + "b1"], DFT, clp, "b1c")
        b2_col = _load_col(ctx, io[pre + "b2"], DT, clp, "b2c")
        w2bf = w2p.tile([P, DFT * D], BF16, name="w2bf")
        for ft in range(DFT):
            st = stp.tile([P, D], F32, tag="w2st")
            nc.sync.dma_start(st[:], io[pre + "w2"][ft * P:(ft + 1) * P, :])
            nc.vector.tensor_copy(w2bf[:, ft * D:(ft + 1) * D], st[:])
        for c in range(2):
            h_sb = hp.tile([P, DFT * 512], BF16, tag="h", name="h_sb")
            for fc in range(8):
                w1_t = []
                for dt in range(DT):
                    wt = w1p.tile([P, 512], F32R, tag="w1")
                    nc.sync.dma_start(
                        wt[:], io[pre + "w1"][dt * P:(dt + 1) * P, fc * 512:(fc + 1) * 512].bitcast(F32R))
                    w1_t.append(wt)
                for fi in range(4):
                    ft = fc * 4 + fi
                    hh = ps_h.tile([P, 512], F32, tag="hh")
                    for dt in range(DT):
                        nc.tensor.matmul(hh[:], w1_t[dt][:, fi * P:(fi + 1) * P],
                                         xn2[:, dt * T + c * 512:][:, :512],
                                         start=(dt == 0), stop=(dt == DT - 1))
                    nc.scalar.activation(h_sb[:, ft * 512:(ft + 1) * 512], hh[:],
                                         AF.Gelu, bias=b1_col[:, ft:ft + 1], scale=1.0)
            for ot in range(DT):
                ff = ps_f.tile([P, 512], F32, tag="ff")
                for ft in range(DFT):
                    nc.tensor.matmul(ff[:], w2bf[:, ft * D + ot * P:][:, :P],
                                     h_sb[:, ft * 512:(ft + 1) * 512],
                                     start=(ft == 0), stop=(ft == DFT - 1))
                sl = slice(ot * T + c * 512, ot * T + c * 512 + 512)
                nc.vector.scalar_tensor_tensor(x[:, sl], ff[:],
                                               b2_col[:, ot:ot + 1], x[:, sl].bitcast(F32),
                                               OP.add, OP.add)


def _build_layer(ctx, li, kind, x):
    nc, tc = ctx.nc, ctx.tc
    io = ctx.io
    pre = f"{kind}{li}_"
    pfx = pre
    with (
        tc.tile_pool(name=pfx + "cl", bufs=1) as clp,
        tc.tile_pool(name=pfx + "wa", bufs=1) as wap,
        tc.tile_pool(name=pfx + "dram", bufs=1, space="DRAM") as dp,
    ):
        g1_col = _load_col(ctx, io[pre + "ln1_g"], DT, clp, "g1c")
        b1c_col = _load_col(ctx, io[pre + "ln1_b"], DT, clp, "b1cc")
        bo_col = _load_col(ctx, io[pre + "bo"], DT, clp, "boc")

        xn = wap.tile([P, DT * T], F32R, tag="workA", name="xn")
        _layernorm(ctx, x, g1_col, b1c_col, xn, pfx + "ln1")

        with tc.tile_pool(name=pfx + "qo", bufs=1) as qop:
            qo_sb = qop.tile([P, DT * T], F32R, tag="qo", name="qo_sb")

            def q_cb(ot, c, pp):
                nc.vector.tensor_copy(qo_sb[:, ot * T + c * 512:][:, :512], pp[:])

            _proj_T(ctx, io[pre + "wq"], xn, q_cb, pfx + "q")

            cc_in = dp.tile([P, 8192], F32, tag="cc_in", name="cc_in")
            cc_out = dp.tile([P, 8192], F32, tag="cc_out", name="cc_out", addr_space="Shared")
            if kind == "lin":
                _lin_kv(ctx, li, xn, cc_in, pfx + "kv")
            else:
                a2a_in = dp.tile([NC, 2, P, T], BF16, tag="a2a_in", name="a2a_in")
                a2a_out = dp.tile([NC, 2, P, T], BF16, tag="a2a_out", name="a2a_out")
                with tc.tile_pool(name=pfx + "kest", bufs=3) as ksp:
                    def mk_cb(ten):
                        def cb(ot, c, pp):
                            st = ksp.tile([P, 512], BF16, tag="kest", name="kest")
                            nc.vector.tensor_copy(st[:], pp[:])
                            nc.sync.dma_start(a2a_in[ot, ten, :, c * 512:(c + 1) * 512], st[:])
                        return cb
                    _proj_T(ctx, io[pre + "wk"], xn, mk_cb(0), pfx + "ke")
                    _proj_T(ctx, io[pre + "wv"], xn, mk_cb(1), pfx + "ve")
                nc.gpsimd.collective_compute(
                    "AllToAll", OP.bypass,
                    replica_groups=[list(range(NC))],
                    ins=[a2a_in[:]], outs=[a2a_out[:]])
                _conv_kv(ctx, li, a2a_out, cc_in, pfx + "ckv")

            nc.gpsimd.collective_compute(
                "AllReduce", OP.add,
                replica_groups=[list(range(NC))],
                ins=[cc_in[:]], outs=[cc_out[:]])

            with tc.tile_pool(name=pfx + "kvp", bufs=1) as kvp:
                kv_sb = kvp.tile([P, 8192], F32R, tag="kv", name="kv_sb")
                nc.sync.dma_start(kv_sb[:], cc_out[:].bitcast(F32R))

                _attention(ctx, qo_sb, kv_sb, pfx + "att")

                def wo_cb(ot, c, pp):
                    sl = slice(ot * T + c * 512, ot * T + c * 512 + 512)
                    nc.vector.scalar_tensor_tensor(x[:, sl], pp[:],
                                                   bo_col[:, ot:ot + 1], x[:, sl].bitcast(F32),
                                                   OP.add, OP.add)

                _proj_T(ctx, io[pre + "wo"], qo_sb, wo_cb, pfx + "wo")

        g2_col = _load_col(ctx, io[pre + "ln2_g"], DT, clp, "g2c")
        b2c_col = _load_col(ctx, io[pre + "ln2_b"], DT, clp, "b2cc")
        xn2 = wap.tile([P, DT * T], F32R, tag="workA", name="xn2")
        _layernorm(ctx, x, g2_col, b2c_col, xn2, pfx + "ln2")
        _ffn(ctx, pre, x, xn2, clp, pfx + "ffn")


def build_program():
    nc = bacc.Bacc("TRN2", target_bir_lowering=False, debug=False, num_devices=NC)
    io = _declare_io(nc)
    with tile.TileContext(nc) as tc:
        with (
            tc.tile_pool(name="cst", bufs=1) as cst,
            tc.tile_pool(name="xp", bufs=1) as xp,
        ):
            ctx = Ctx(nc, tc, io)
            ident_f = cst.tile([P, P], F32, name="ident_f")
            make_identity(nc, ident_f[:])
            ctx.ident_r = cst.tile([P, P], F32R, name="ident_r")
            nc.vector.tensor_copy(ctx.ident_r[:], ident_f[:])
            oc_f = cst.tile([P, 1], F32, name="oc_f")
            nc.vector.memset(oc_f[:], 1.0)
            ctx.ones_col = cst.tile([P, 1], F32R, name="ones_col")
            nc.vector.tensor_copy(ctx.ones_col[:], oc_f[:])
            or_f = cst.tile([1, P], F32, name="or_f")
            nc.vector.memset(or_f[:], 1.0)
            ctx.ones_row = cst.tile([1, P], F32R, name="ones_row")
            nc.vector.tensor_copy(ctx.ones_row[:], or_f[:])
            ctx.eps_b = cst.tile([1, 1], F32, name="eps_b")
            nc.vector.memset(ctx.eps_b[:], 1e-5)

            # load x -> feature-major x^T
            x = xp.tile([P, DT * T], F32R, name="x")
            with (
                tc.tile_pool(name="iop", bufs=3) as iop,
                tc.tile_pool(name="iops", bufs=2, space="PSUM") as iops,
            ):
                for tt in range(8):  # tt = b*4 + nt
                    b, nt = divmod(tt, 4)
                    xtok = iop.tile([P, D], F32R, tag="xtok")
                    nc.sync.dma_start(xtok[:], io["x_local"][b, nt * P:(nt + 1) * P, :].bitcast(F32R))
                    for dg in range(2):
                        tps = iops.tile([P, 512], F32R, tag="xt")
                        for i in range(4):
                            dt = dg * 4 + i
                            nc.tensor.transpose(tps[:, i * P:(i + 1) * P],
                                                xtok[:, dt * P:(dt + 1) * P], ctx.ident_r[:])
                        nc.vector.tensor_copy(
                            x[:].rearrange("p (dt t) -> p dt t", dt=DT)[:, dg * 4:(dg + 1) * 4,
                                                                        b * 512 + nt * P:][:, :, :P],
                            tps[:].rearrange("p (i t) -> p i t", i=4).bitcast(F32))

            for li in range(L):
                _build_layer(ctx, li, "lin", x)
            for li in range(L):
                _build_layer(ctx, li, "conv", x)

            # write out: transpose back to token-major
            with (
                tc.tile_pool(name="oop", bufs=3) as oop,
                tc.tile_pool(name="oops", bufs=2, space="PSUM") as oops,
            ):
                for tt in range(8):
                    b, nt = divmod(tt, 4)
                    ytok = oop.tile([P, D], F32, tag="ytok")
                    for dg in range(2):
                        tps = oops.tile([P, 512], F32R, tag="yt")
                        for i in range(4):
                            dt = dg * 4 + i
                            nc.tensor.transpose(tps[:, i * P:(i + 1) * P],
                                                x[:, dt * T + b * 512 + nt * P:][:, :P],
                                                ctx.ident_r[:])
                        nc.vector.tensor_copy(ytok[:, dg * 512:(dg + 1) * 512], tps[:].bitcast(F32))
                    nc.sync.dma_start(io["y_local"][b, nt * P:(nt + 1) * P, :], ytok[:])
    nc.compile()
    return nc


_PROGRAM = None


def _get_program():
    global _PROGRAM
    if _PROGRAM is None:
        _PROGRAM = build_program()
    return _PROGRAM


def _make_in_maps(inputs):
    in_maps = []
    for c in range(NC):
        m = {"x_local": np.ascontiguousarray(inputs["x"][:, c * NL:(c + 1) * NL, :], dtype=np.float32)}
        for li in range(L):
            for kind in ("lin", "conv"):
                pre = f"{kind}{li}_"
                for nm in PARAM_NAMES:
                    v = np.asarray(inputs[f"{kind}_{nm}"][li], dtype=np.float32)
                    if nm in ("pk", "pv"):
                        if kind == "lin":
                            v = np.ascontiguousarray(v[c * NL:(c + 1) * NL, :])
                        else:
                            v = np.ascontiguousarray(v[:, c * P:(c + 1) * P, :])
                    m[pre + nm] = v
        in_maps.append(m)
    return in_maps


def kernel(**inputs):
    nc = _get_program()
    in_maps = _make_in_maps(inputs)
    res = run_bass_kernel_spmd(nc, in_maps, core_ids=list(range(NC)))
    out = np.concatenate([res.results[c]["y_local"] for c in range(NC)], axis=1)
    return out.astype(np.float32)



# revision 25
# speedup vs baseline: 27.0327x; 27.0327x over previous
"""ConvLinformer forward on 8 Trainium2 NeuronCores (Bass/Tile).

Sharding: 8-way over sequence (512 tokens/core/batch). Weights replicated
(bf16), except the conv kernels [O,C,S] which are channel(C)-sliced per core;
the conv contraction is channel-sharded via AllToAll of ke/ve activations
followed by AllReduce of the k_/v_ partials (Linformer layers use the same
AllReduce for their sequence-projection partials).

Overlap strategy: collectives are split per-tensor (k vs v) and launched as
soon as their producer finishes; the q projection and the attention score
phase run while the AllReduces are in flight. Attention is two-phase: scores/
softmax/transpose for all (b,h) first (needs k only), then all AV matmuls
(needs v).

Layout: residual kept feature-major f32: x^T = [128, (dt:8, b:2, tl:512)].
All other activations and all weights bf16; PSUM accumulation f32.

Self-contained: shapes hardcoded; host shards inputs / gathers outputs.
"""

import ml_dtypes
import numpy as np

import concourse.bacc as bacc
import concourse.mybir as mybir
import concourse.tile as tile
from concourse.bass_utils import run_bass_kernel_spmd
from concourse.masks import make_identity

P = 128
B, N, D, H, DH, K, S, DFF, L = 2, 4096, 1024, 8, 128, 256, 16, 4096, 2
NC = 8
NL = N // NC          # 512 local tokens per batch
T = B * NL            # 1024 local tokens, free layout (b, tl)
DT = D // P           # 8 feature tiles
DFT = DFF // P        # 32 dff tiles
KT = K // P           # 2 kv-position tiles
SCALE = float(DH) ** -0.5

F32 = mybir.dt.float32
F32R = mybir.dt.float32r
BF16 = mybir.dt.bfloat16
AX = mybir.AxisListType
OP = mybir.AluOpType
AF = mybir.ActivationFunctionType

PARAM_NAMES = [
    "ln1_g", "ln1_b", "wq", "wk", "wv", "pk", "pv", "wo", "bo",
    "ln2_g", "ln2_b", "w1", "b1", "w2", "b2",
]
BF16_PARAMS = {"wq", "wk", "wv", "wo", "w1", "w2", "pk", "pv"}


def _param_shape(kind, nm):
    if nm in ("ln1_g", "ln1_b", "bo", "ln2_g", "ln2_b", "b2"):
        return (D,)
    if nm == "b1":
        return (DFF,)
    if nm in ("wq", "wk", "wv", "wo"):
        return (D, D)
    if nm == "w1":
        return (D, DFF)
    if nm == "w2":
        return (DFF, D)
    # pk / pv: per-core slice
    return (NL, K) if kind == "lin" else (D, P, S)


def _blob_layout():
    """Fixed packing order -> [(key, kind, nm, offset, shape)] per dtype blob."""
    bf, fl = [], []
    off_b = off_f = 0
    for kind in ("lin", "conv"):
        for li in range(L):
            for nm in PARAM_NAMES:
                shape = _param_shape(kind, nm)
                n = int(np.prod(shape))
                key = f"{kind}{li}_{nm}"
                if nm in BF16_PARAMS:
                    bf.append((key, kind, nm, off_b, shape))
                    off_b += n
                else:
                    fl.append((key, kind, nm, off_f, shape))
                    off_f += n
    return bf, fl, off_b, off_f


def _declare_io(nc):
    bf, fl, nb, nf = _blob_layout()
    d = {}
    d["x_local"] = nc.dram_tensor("x_local", [B, NL, D], F32, kind="ExternalInput").ap()
    wb = nc.dram_tensor("wb", [nb], BF16, kind="ExternalInput").ap()
    cb = nc.dram_tensor("cb", [nf], F32, kind="ExternalInput").ap()
    for key, kind, nm, off, shape in bf:
        ap = wb[off:off + int(np.prod(shape))]
        if len(shape) == 2:
            ap = ap.rearrange("(r c) -> r c", c=shape[1])
        elif len(shape) == 3:
            ap = ap.rearrange("(o c s) -> o c s", c=shape[1], s=shape[2])
        d[key] = ap
    for key, kind, nm, off, shape in fl:
        d[key] = cb[off:off + int(np.prod(shape))]
    d["y_local"] = nc.dram_tensor("y_local", [B, NL, D], F32, kind="ExternalOutput").ap()
    return d


class Ctx:
    def __init__(self, nc, tc, io):
        self.nc, self.tc, self.io = nc, tc, io


def _load_col(ctx, dram_vec, width, pool, name):
    """Load a [width*128] dram vector as a [128, width] column tile (f32)."""
    nc = ctx.nc
    t = pool.tile([P, width], F32, name=name)
    nc.sync.dma_start(t[:], dram_vec.rearrange("(w p) -> p w", p=P))
    return t


def _layernorm(ctx, x, g_col, b_col, xn, pfx):
    """xn = LN(x) * g + b -> bf16 feature-major [128, DT*T]."""
    nc, tc = ctx.nc, ctx.tc
    with (
        tc.tile_pool(name=pfx + "sb", bufs=2) as sb,
        tc.tile_pool(name=pfx + "xq", bufs=1) as xqp,
        tc.tile_pool(name=pfx + "ps", bufs=2, space="PSUM") as ps,
        tc.tile_pool(name=pfx + "bps", bufs=2, space="PSUM") as bps,
    ):
        xsq = xqp.tile([P, DT * T], BF16, name="xsq")
        xsq_v = xsq[:].rearrange("p (dt c t) -> p dt c t", dt=DT, c=2)
        x_v = x[:].bitcast(F32).rearrange("p (dt c t) -> p dt c t", dt=DT, c=2)
        for c in range(2):
            nc.scalar.activation(xsq_v[:, :, c], x_v[:, :, c], AF.Square)
        s0_row = sb.tile([1, T], F32R, name="s0row", bufs=1)
        s1_row = sb.tile([1, T], F32R, name="s1row", bufs=1)
        for c in range(2):  # token chunks of 512 (c == batch)
            st1 = ps.tile([1, 512], F32, tag="st")
            st2 = ps.tile([1, 512], F32, tag="st")
            for dt in range(DT):
                nc.tensor.matmul(st1[:], ctx.ones_col[:], x[:, dt * T + c * 512:][:, :512],
                                 start=(dt == 0), stop=(dt == DT - 1))
            for dt in range(DT):
                nc.tensor.matmul(st2[:], ctx.ones_colb[:], xsq[:, dt * T + c * 512:][:, :512],
                                 start=(dt == 0), stop=(dt == DT - 1))
            m_row = sb.tile([1, 512], F32, tag="m")
            nc.vector.tensor_scalar_mul(m_row[:], st1[:], 1.0 / D)
            msq = sb.tile([1, 512], F32, tag="msq")
            nc.vector.tensor_mul(msq[:], m_row[:], m_row[:])
            var = sb.tile([1, 512], F32, tag="var")
            nc.vector.scalar_tensor_tensor(var[:], st2[:], 1.0 / D, msq[:], OP.mult, OP.subtract)
            sd = sb.tile([1, 512], F32, tag="sd")
            nc.scalar.activation(sd[:], var[:], AF.Sqrt, bias=ctx.eps_b[:], scale=1.0)
            r_row = sb.tile([1, 512], F32, tag="r")
            nc.vector.reciprocal(r_row[:], sd[:])
            nc.vector.tensor_copy(s1_row[:, c * 512:][:, :512], r_row[:])
            nc.vector.scalar_tensor_tensor(
                s0_row[:, c * 512:][:, :512], m_row[:], -1.0,
                r_row[:], OP.mult, OP.mult)
        for c in range(2):
            s0bc = bps.tile([P, 512], F32, tag="bc0")
            s1bc = bps.tile([P, 512], F32, tag="bc1")
            nc.tensor.matmul(s0bc[:], ctx.ones_row[:], s0_row[:, c * 512:][:, :512],
                             start=True, stop=True)
            nc.tensor.matmul(s1bc[:], ctx.ones_row[:], s1_row[:, c * 512:][:, :512],
                             start=True, stop=True)
            # stage broadcasts in SBUF so GpSimd can help with the apply
            s0sb = sb.tile([P, 512], F32, tag="s0sb")
            nc.scalar.activation(s0sb[:], s0bc[:], AF.Copy)
            s1sb = sb.tile([P, 512], F32, tag="s1sb")
            nc.scalar.activation(s1sb[:], s1bc[:], AF.Copy)
            for dt in range(DT):
                sl = slice(dt * T + c * 512, dt * T + c * 512 + 512)
                p1 = sb.tile([P, 512], BF16, tag="p1")
                nc.vector.scalar_tensor_tensor(p1[:], x[:, sl].bitcast(F32),
                                               g_col[:, dt:dt + 1], s1sb[:], OP.mult, OP.mult)
                p2 = sb.tile([P, 512], BF16, tag="p2")
                nc.scalar.activation(p2[:], s0sb[:], AF.Identity,
                                     bias=b_col[:, dt:dt + 1], scale=g_col[:, dt:dt + 1])
                nc.gpsimd.tensor_add(xn[:, sl], p1[:], p2[:])


def _proj_T(ctx, w_dram, src, out_cb, pfx):
    """Feature-major projection: psum[ot, c] = sum_dt W[dt,ot].T @ src[dt,c]."""
    nc, tc = ctx.nc, ctx.tc
    with (
        tc.tile_pool(name=pfx + "w", bufs=DT + 1) as wp,
        tc.tile_pool(name=pfx + "ps", bufs=3, space="PSUM") as ps,
    ):
        w_sb = []
        for dt in range(DT):
            wt = wp.tile([P, D], BF16, tag="w", name=f"w{dt}")
            nc.sync.dma_start(wt[:], w_dram[dt * P:(dt + 1) * P, :])
            w_sb.append(wt)
        for c in range(2):
            for ot in range(DT):
                pp = ps.tile([P, 512], F32, tag="pj")
                for dt in range(DT):
                    nc.tensor.matmul(pp[:], w_sb[dt][:, ot * P:(ot + 1) * P],
                                     src[:, dt * T + c * 512:][:, :512],
                                     start=(dt == 0), stop=(dt == DT - 1))
                out_cb(ot, c, pp)


def _lin_kv_one(ctx, li, xn, ten, cc_in, pfx):
    """Linformer k (ten=0) or v (ten=1): full token-major, project, stage."""
    nc, tc = ctx.nc, ctx.tc
    io = ctx.io
    pre = f"lin{li}_"
    with (
        tc.tile_pool(name=pfx + "w", bufs=DT) as wp,
        tc.tile_pool(name=pfx + "kv", bufs=2) as kvp,
        tc.tile_pool(name=pfx + "p", bufs=1) as pp_,
        tc.tile_pool(name=pfx + "ar", bufs=1) as arp,
        tc.tile_pool(name=pfx + "ps", bufs=2, space="PSUM") as ps,
        tc.tile_pool(name=pfx + "ps2", bufs=2, space="PSUM") as ps2,
    ):
        pname = pre + ("pk" if ten == 0 else "pv")
        p_sb = pp_.tile([P, 4 * K], BF16, name="p_sb")
        nc.sync.dma_start(p_sb[:].rearrange("p (nt k) -> p nt k", nt=4),
                          io[pname].rearrange("(nt p) k -> p nt k", p=P))
        wname = pre + ("wk" if ten == 0 else "wv")
        w_sb = []
        for dt in range(DT):
            wt = wp.tile([P, D], BF16, tag="w", name=f"w{dt}")
            nc.sync.dma_start(wt[:], io[wname][dt * P:(dt + 1) * P, :])
            w_sb.append(wt)
        arh = arp.tile([P, 4096], BF16, tag="arh", name="arh")
        for b in range(B):
            full = kvp.tile([P, 4 * D], BF16, tag="full", name="full")
            for nt in range(4):
                for c2 in range(2):
                    fp = ps.tile([P, 512], F32, tag="pf")
                    for dt in range(DT):
                        lhs = xn[:, dt * T + b * 512 + nt * P:][:, :P]
                        nc.tensor.matmul(fp[:], lhs, w_sb[dt][:, c2 * 512:][:, :512],
                                         start=(dt == 0), stop=(dt == DT - 1))
                    nc.vector.tensor_copy(full[:, nt * D + c2 * 512:][:, :512], fp[:])
            if ten == 0:
                # k_^T partials: [dt][128, K] per b (feature-major)
                for dt in range(DT):
                    kp = ps2.tile([P, K], F32, tag="kp")
                    for nt in range(4):
                        nc.tensor.matmul(kp[:], full[:, nt * D + dt * P:][:, :P],
                                         p_sb[:, nt * K:][:, :K],
                                         start=(nt == 0), stop=(nt == 3))
                    nc.vector.tensor_copy(arh[:, b * (DT * K) + dt * K:][:, :K], kp[:])
            else:
                # v_ token-major partials: [kt][128, D] per b
                for kt in range(KT):
                    for c2 in range(2):
                        vp = ps2.tile([P, 512], F32, tag="vp")
                        for nt in range(4):
                            nc.tensor.matmul(vp[:], p_sb[:, nt * K + kt * P:][:, :P],
                                             full[:, nt * D + c2 * 512:][:, :512],
                                             start=(nt == 0), stop=(nt == 3))
                        nc.vector.tensor_copy(
                            arh[:, (b * KT + kt) * D + c2 * 512:][:, :512], vp[:])
        nc.sync.dma_start(cc_in[:], arh[:])


def _conv_wt_prep(ctx, li, ten, wtp, pfx):
    """Transpose conv kernel slice [O,128c,S] -> wt [128c, (s, o)] bf16."""
    nc, tc = ctx.nc, ctx.tc
    io = ctx.io
    pre = f"conv{li}_"
    wname = pre + ("pk" if ten == 0 else "pv")
    wt_sb = wtp.tile([P, S * D], BF16, tag=f"wt{ten}", name=f"wt{ten}")
    with (
        tc.tile_pool(name=pfx + "wn", bufs=2) as wnp,
        tc.tile_pool(name=pfx + "ps", bufs=2, space="PSUM") as ps,
    ):
        for ot in range(DT):
            wn = wnp.tile([P, P * S], BF16, tag="wn")
            nc.sync.dma_start(
                wn[:], io[wname][ot * P:(ot + 1) * P].rearrange("o c s -> o (c s)"))
            for s4 in range(4):
                tp_ps = ps.tile([P, 512], BF16, tag="wtp")
                for si in range(4):
                    s = s4 * 4 + si
                    nc.tensor.transpose(
                        tp_ps[:, si * P:(si + 1) * P],
                        wn[:].rearrange("o (c s) -> o s c", s=S)[:, s],
                        ctx.ident_b[:])
                nc.vector.tensor_copy(
                    wt_sb[:].rearrange("c (s o) -> c s o", s=S)[:, s4 * 4:(s4 + 1) * 4, ot * P:(ot + 1) * P],
                    tp_ps[:].rearrange("c (si o) -> c si o", si=4))
    return wt_sb


def _conv_kv_one(ctx, ten, a2a_out, wt_sb, cc_in, pfx):
    """Contract local c-slice of the A2A'd ke/ve with the sliced kernel."""
    nc, tc = ctx.nc, ctx.tc
    with (
        tc.tile_pool(name=pfx + "cs", bufs=1) as csp,
        tc.tile_pool(name=pfx + "ar", bufs=1) as arp,
        tc.tile_pool(name=pfx + "cps", bufs=2, space="PSUM") as cps,
    ):
        # readback: [128c, (b, peer, tl)]
        ecs = csp.tile([P, B * N], BF16, tag="ecs", name="ecs")
        nc.sync.dma_start(
            ecs[:].rearrange("c (b j t) -> c b j t", b=B, j=NC),
            a2a_out[:].rearrange("j c (b t) -> c b j t", b=B))
        arh = arp.tile([P, 4096], BF16, tag="arh", name="arh")
        if ten == 0:
            # k_^T feature-major: [ot][128, (b, K)]
            for ot in range(DT):
                kp = cps.tile([P, 512], F32, tag="ck")
                for s in range(S):
                    rhs = ecs[:].rearrange("c (b j t) -> c b j t", b=B, j=NC)[:, :, :, s::S]
                    nc.tensor.matmul(kp[:].rearrange("o (b k) -> o b k", b=B),
                                     wt_sb[:, s * D + ot * P:][:, :P], rhs,
                                     start=(s == 0), stop=(s == S - 1))
                nc.vector.tensor_copy(
                    arh[:].rearrange("p (b dt k) -> p b dt k", b=B, dt=DT)[:, :, ot, :],
                    kp[:].rearrange("o (b k) -> o b k", b=B))
        else:
            # v_ token-major: [b, kt][128, D]
            for b in range(B):
                for kt in range(KT):
                    for c2 in range(2):
                        vp = cps.tile([P, 512], F32, tag="cv")
                        for s in range(S):
                            lhs = ecs[:].rearrange(
                                "c (b j t) -> c b j t", b=B, j=NC)[:, b, kt * 4:(kt + 1) * 4, s::S]
                            nc.tensor.matmul(vp[:], lhs,
                                             wt_sb[:, s * D + c2 * 512:][:, :512],
                                             start=(s == 0), stop=(s == S - 1))
                        nc.vector.tensor_copy(
                            arh[:, (b * KT + kt) * D + c2 * 512:][:, :512],
                            vp[:])
        nc.sync.dma_start(cc_in[:], arh[:])


def _attention_scores(ctx, qo_sb, kv_k, at_all, inv_all, pfx):
    """Phase S: per (b,h): transposed scores A^T = exp(K^T q / sqrt(d)) and
    softmax denominators (needs k only). A^T lands in at_all kv-major."""
    nc, tc = ctx.nc, ctx.tc
    with (
        tc.tile_pool(name=pfx + "ps", bufs=3, space="PSUM") as ps_s,
        tc.tile_pool(name=pfx + "pd", bufs=2, space="PSUM") as ps_d,
    ):
        for b in range(B):
            for h in range(H):
                bh = b * H + h
                for kt in range(KT):
                    st_ps = ps_s.tile([P, 512], F32, tag="st")
                    nc.tensor.matmul(st_ps[:],
                                     kv_k[:, b * (DT * K) + h * K + kt * P:][:, :P],
                                     qo_sb[:, h * T + b * 512:][:, :512],
                                     start=True, stop=True)
                    nc.scalar.activation(at_all[:, (bh * KT + kt) * 512:][:, :512],
                                         st_ps[:], AF.Exp, scale=SCALE)
                den_ps = ps_d.tile([1, 512], F32, tag="den")
                for kt in range(KT):
                    nc.tensor.matmul(den_ps[:], ctx.ones_colb[:],
                                     at_all[:, (bh * KT + kt) * 512:][:, :512],
                                     start=(kt == 0), stop=(kt == KT - 1))
                with nc.allow_low_precision(reason="softmax denom broadcast via f32r matmul"):
                    nc.vector.reciprocal(inv_all[:, bh * 512:][:, :512], den_ps[:])


def _attention_av(ctx, qo_sb, kv_v, at_all, inv_all, pfx):
    """Phase O: o = A^T.T @ v per (b,h), normalized by 1/den; overwrites q."""
    nc, tc = ctx.nc, ctx.tc
    with (
        tc.tile_pool(name=pfx + "po", bufs=2, space="PSUM") as ps_o,
        tc.tile_pool(name=pfx + "pb", bufs=2, space="PSUM") as ps_b,
        tc.tile_pool(name=pfx + "ib", bufs=2) as ibp,
    ):
        for b in range(B):
            for h in range(H):
                bh = b * H + h
                oo = ps_o.tile([P, 512], F32, tag="oo")
                for kt in range(KT):
                    nc.tensor.matmul(oo[:],
                                     kv_v[:, (b * KT + kt) * D + h * P:][:, :P],
                                     at_all[:, (bh * KT + kt) * 512:][:, :512],
                                     start=(kt == 0), stop=(kt == KT - 1))
                invbc = ps_b.tile([P, 512], F32, tag="invbc")
                nc.tensor.matmul(invbc[:], ctx.ones_row[:],
                                 inv_all[:, bh * 512:][:, :512], start=True, stop=True)
                invsb = ibp.tile([P, 512], BF16, tag="invsb")
                nc.scalar.activation(invsb[:], invbc[:], AF.Copy)
                nc.vector.tensor_mul(qo_sb[:, h * T + b * 512:][:, :512], oo[:], invsb[:])


def _ffn(ctx, pre, x, xn2, clp, pfx):
    nc, tc = ctx.nc, ctx.tc
    io = ctx.io
    with (
        tc.tile_pool(name=pfx + "h", bufs=1) as hp,
        tc.tile_pool(name=pfx + "w1", bufs=DT + 2) as w1p,
        tc.tile_pool(name=pfx + "w2", bufs=2) as w2p,
        tc.tile_pool(name=pfx + "ph", bufs=3, space="PSUM") as ps_h,
        tc.tile_pool(name=pfx + "pf", bufs=2, space="PSUM") as ps_f,
    ):
        b1_col = _load_col(ctx, io[pre + "b1"], DFT, clp, "b1c")
        b2_col = _load_col(ctx, io[pre + "b2"], DT, clp, "b2c")
        h_sb = hp.tile([P, DFT * T], BF16, name="h_sb")
        for fc in range(8):
            w1_t = []
            for dt in range(DT):
                wt = w1p.tile([P, 512], BF16, tag="w1")
                nc.scalar.dma_start(
                    wt[:], io[pre + "w1"][dt * P:(dt + 1) * P, fc * 512:(fc + 1) * 512])
                w1_t.append(wt)
            for fi in range(4):
                ft = fc * 4 + fi
                for c in range(2):
                    hh = ps_h.tile([P, 512], F32, tag="hh")
                    for dt in range(DT):
                        nc.tensor.matmul(hh[:], w1_t[dt][:, fi * P:(fi + 1) * P],
                                         xn2[:, dt * T + c * 512:][:, :512],
                                         start=(dt == 0), stop=(dt == DT - 1))
                    nc.scalar.activation(h_sb[:, ft * T + c * 512:][:, :512], hh[:],
                                         AF.Gelu, bias=b1_col[:, ft:ft + 1], scale=1.0)
        for c in range(2):  # c-outer: chunk 0 of x finishes early for next LN
            for ot in range(DT):
                w2t = w2p.tile([P, DFT * P], BF16, tag="w2")
                nc.scalar.dma_start(
                    w2t[:].rearrange("p (ft o) -> p ft o", ft=DFT),
                    io[pre + "w2"][:, ot * P:(ot + 1) * P].rearrange("(ft p) o -> p ft o", p=P))
                ff = ps_f.tile([P, 512], F32, tag="ff")
                for ft in range(DFT):
                    nc.tensor.matmul(ff[:], w2t[:, ft * P:(ft + 1) * P],
                                     h_sb[:, ft * T + c * 512:][:, :512],
                                     start=(ft == 0), stop=(ft == DFT - 1))
                sl = slice(ot * T + c * 512, ot * T + c * 512 + 512)
                nc.vector.scalar_tensor_tensor(x[:, sl], ff[:],
                                               b2_col[:, ot:ot + 1], x[:, sl].bitcast(F32),
                                               OP.add, OP.add)


def _build_layer(ctx, li, kind, x):
    nc, tc = ctx.nc, ctx.tc
    io = ctx.io
    pre = f"{kind}{li}_"
    pfx = pre
    rg = [list(range(NC))]
    with (
        tc.tile_pool(name=pfx + "cl", bufs=1) as clp,
        tc.tile_pool(name=pfx + "wa", bufs=1) as wap,
        tc.tile_pool(name=pfx + "dram", bufs=1, space="DRAM") as dp,
    ):
        g1_col = _load_col(ctx, io[pre + "ln1_g"], DT, clp, "g1c")
        b1c_col = _load_col(ctx, io[pre + "ln1_b"], DT, clp, "b1cc")
        bo_col = _load_col(ctx, io[pre + "bo"], DT, clp, "boc")

        xn = wap.tile([P, DT * T], BF16, tag="workA", name="xn")
        _layernorm(ctx, x, g1_col, b1c_col, xn, pfx + "ln1")

        cc_k_in = dp.tile([P, 4096], BF16, tag="cck_in", name="cck_in")
        cc_k_out = dp.tile([P, 4096], BF16, tag="cck_out", name="cck_out", addr_space="Shared")
        cc_v_in = dp.tile([P, 4096], BF16, tag="ccv_in", name="ccv_in")
        cc_v_out = dp.tile([P, 4096], BF16, tag="ccv_out", name="ccv_out", addr_space="Shared")

        with tc.tile_pool(name=pfx + "qo", bufs=1) as qop:
            qo_sb = qop.tile([P, DT * T], BF16, tag="qo", name="qo_sb")

            def q_cb(ot, c, pp):
                nc.vector.tensor_copy(qo_sb[:, ot * T + c * 512:][:, :512], pp[:])

            if kind == "lin":
                # k partials -> AR_k; v partials -> AR_v; then q (overlaps ARs)
                _lin_kv_one(ctx, li, xn, 0, cc_k_in, pfx + "kvk")
                nc.gpsimd.collective_compute(
                    "AllReduce", OP.add, replica_groups=rg,
                    ins=[cc_k_in[:]], outs=[cc_k_out[:]])
                _lin_kv_one(ctx, li, xn, 1, cc_v_in, pfx + "kvv")
                nc.gpsimd.collective_compute(
                    "AllReduce", OP.add, replica_groups=rg,
                    ins=[cc_v_in[:]], outs=[cc_v_out[:]])
                _proj_T(ctx, io[pre + "wq"], xn, q_cb, pfx + "q")
            else:
                a2a_k_in = dp.tile([NC, P, T], BF16, tag="a2ak_in", name="a2ak_in")
                a2a_k_out = dp.tile([NC, P, T], BF16, tag="a2ak_out", name="a2ak_out")
                a2a_v_in = dp.tile([NC, P, T], BF16, tag="a2av_in", name="a2av_in")
                a2a_v_out = dp.tile([NC, P, T], BF16, tag="a2av_out", name="a2av_out")
                with (
                    tc.tile_pool(name=pfx + "kest", bufs=3) as ksp,
                    tc.tile_pool(name=pfx + "wt", bufs=1) as wtp,
                ):
                    def mk_cb(dst):
                        def cb(ot, c, pp):
                            st = ksp.tile([P, 512], BF16, tag="kest", name="kest")
                            nc.vector.tensor_copy(st[:], pp[:])
                            nc.sync.dma_start(dst[ot, :, c * 512:(c + 1) * 512], st[:])
                        return cb
                    _proj_T(ctx, io[pre + "wk"], xn, mk_cb(a2a_k_in), pfx + "ke")
                    nc.gpsimd.collective_compute(
                        "AllToAll", OP.bypass, replica_groups=rg,
                        ins=[a2a_k_in[:]], outs=[a2a_k_out[:]])
                    _proj_T(ctx, io[pre + "wv"], xn, mk_cb(a2a_v_in), pfx + "ve")
                    nc.gpsimd.collective_compute(
                        "AllToAll", OP.bypass, replica_groups=rg,
                        ins=[a2a_v_in[:]], outs=[a2a_v_out[:]])
                    _proj_T(ctx, io[pre + "wq"], xn, q_cb, pfx + "q")
                    wt_k = _conv_wt_prep(ctx, li, 0, wtp, pfx + "wpk")
                    wt_v = _conv_wt_prep(ctx, li, 1, wtp, pfx + "wpv")
                    _conv_kv_one(ctx, 0, a2a_k_out, wt_k, cc_k_in, pfx + "ckk")
                    nc.gpsimd.collective_compute(
                        "AllReduce", OP.add, replica_groups=rg,
                        ins=[cc_k_in[:]], outs=[cc_k_out[:]])
                    _conv_kv_one(ctx, 1, a2a_v_out, wt_v, cc_v_in, pfx + "ckv")
                    nc.gpsimd.collective_compute(
                        "AllReduce", OP.add, replica_groups=rg,
                        ins=[cc_v_in[:]], outs=[cc_v_out[:]])

            with (
                tc.tile_pool(name=pfx + "kvp", bufs=1) as kvp,
                tc.tile_pool(name=pfx + "atp", bufs=1) as atp,
            ):
                kv_k = kvp.tile([P, 4096], BF16, tag="kvk", name="kv_k")
                nc.sync.dma_start(kv_k[:], cc_k_out[:])
                at_all = atp.tile([P, B * H * KT * 512], BF16, name="at_all")
                inv_all = atp.tile([1, B * H * 512], F32R, name="inv_all")
                _attention_scores(ctx, qo_sb, kv_k, at_all, inv_all, pfx + "atS")

                kv_v = kvp.tile([P, 4096], BF16, tag="kvv", name="kv_v")
                nc.sync.dma_start(kv_v[:], cc_v_out[:])
                _attention_av(ctx, qo_sb, kv_v, at_all, inv_all, pfx + "atO")

            def wo_cb(ot, c, pp):
                sl = slice(ot * T + c * 512, ot * T + c * 512 + 512)
                nc.vector.scalar_tensor_tensor(x[:, sl], pp[:],
                                               bo_col[:, ot:ot + 1], x[:, sl].bitcast(F32),
                                               OP.add, OP.add)

            _proj_T(ctx, io[pre + "wo"], qo_sb, wo_cb, pfx + "wo")

        g2_col = _load_col(ctx, io[pre + "ln2_g"], DT, clp, "g2c")
        b2c_col = _load_col(ctx, io[pre + "ln2_b"], DT, clp, "b2cc")
        xn2 = wap.tile([P, DT * T], BF16, tag="workA", name="xn2")
        _layernorm(ctx, x, g2_col, b2c_col, xn2, pfx + "ln2")
        _ffn(ctx, pre, x, xn2, clp, pfx + "ffn")


def build_program():
    nc = bacc.Bacc("TRN2", target_bir_lowering=False, debug=False, num_devices=NC)
    io = _declare_io(nc)
    with tile.TileContext(nc) as tc:
        with (
            tc.tile_pool(name="cst", bufs=1) as cst,
            tc.tile_pool(name="xp", bufs=1) as xp,
        ):
            ctx = Ctx(nc, tc, io)
            ident_f = cst.tile([P, P], F32, name="ident_f")
            make_identity(nc, ident_f[:])
            ctx.ident_r = cst.tile([P, P], F32R, name="ident_r")
            nc.vector.tensor_copy(ctx.ident_r[:], ident_f[:])
            ctx.ident_b = cst.tile([P, P], BF16, name="ident_b")
            nc.vector.tensor_copy(ctx.ident_b[:], ident_f[:])
            oc_f = cst.tile([P, 1], F32, name="oc_f")
            nc.vector.memset(oc_f[:], 1.0)
            ctx.ones_col = cst.tile([P, 1], F32R, name="ones_col")
            nc.vector.tensor_copy(ctx.ones_col[:], oc_f[:])
            ctx.ones_colb = cst.tile([P, 1], BF16, name="ones_colb")
            nc.vector.tensor_copy(ctx.ones_colb[:], oc_f[:])
            or_f = cst.tile([1, P], F32, name="or_f")
            nc.vector.memset(or_f[:], 1.0)
            ctx.ones_row = cst.tile([1, P], F32R, name="ones_row")
            nc.vector.tensor_copy(ctx.ones_row[:], or_f[:])
            ctx.eps_b = cst.tile([1, 1], F32, name="eps_b")
            nc.vector.memset(ctx.eps_b[:], 1e-5)

            # load x -> feature-major x^T
            x = xp.tile([P, DT * T], F32R, name="x")
            with (
                tc.tile_pool(name="iop", bufs=3) as iop,
                tc.tile_pool(name="iops", bufs=2, space="PSUM") as iops,
            ):
                for tt in range(8):  # tt = b*4 + nt
                    b, nt = divmod(tt, 4)
                    xtok = iop.tile([P, D], F32R, tag="xtok")
                    nc.sync.dma_start(xtok[:], io["x_local"][b, nt * P:(nt + 1) * P, :].bitcast(F32R))
                    for dg in range(2):
                        tps = iops.tile([P, 512], F32R, tag="xt")
                        for i in range(4):
                            dt = dg * 4 + i
                            nc.tensor.transpose(tps[:, i * P:(i + 1) * P],
                                                xtok[:, dt * P:(dt + 1) * P], ctx.ident_r[:])
                        nc.vector.tensor_copy(
                            x[:].rearrange("p (dt t) -> p dt t", dt=DT)[:, dg * 4:(dg + 1) * 4,
                                                                        b * 512 + nt * P:][:, :, :P],
                            tps[:].rearrange("p (i t) -> p i t", i=4).bitcast(F32))

            for li in range(L):
                _build_layer(ctx, li, "lin", x)
            for li in range(L):
                _build_layer(ctx, li, "conv", x)

            # write out: transpose back to token-major
            with (
                tc.tile_pool(name="oop", bufs=3) as oop,
                tc.tile_pool(name="oops", bufs=2, space="PSUM") as oops,
            ):
                for tt in range(8):
                    b, nt = divmod(tt, 4)
                    ytok = oop.tile([P, D], F32, tag="ytok")
                    for dg in range(2):
                        tps = oops.tile([P, 512], F32R, tag="yt")
                        for i in range(4):
                            dt = dg * 4 + i
                            nc.tensor.transpose(tps[:, i * P:(i + 1) * P],
                                                x[:, dt * T + b * 512 + nt * P:][:, :P],
                                                ctx.ident_r[:])
                        nc.vector.tensor_copy(ytok[:, dg * 512:(dg + 1) * 512], tps[:].bitcast(F32))
                    nc.sync.dma_start(io["y_local"][b, nt * P:(nt + 1) * P, :], ytok[:])
    nc.compile()
    return nc


_PROGRAM = None


def _get_program():
    global _PROGRAM
    if _PROGRAM is None:
        _PROGRAM = build_program()
    return _PROGRAM


def _make_in_maps(inputs):
    bf, fl, nb, nf = _blob_layout()
    in_maps = []
    for c in range(NC):
        wb = np.empty(nb, dtype=ml_dtypes.bfloat16)
        cb = np.empty(nf, dtype=np.float32)
        for entries, blob in ((bf, wb), (fl, cb)):
            for key, kind, nm, off, shape in entries:
                li = int(key[len(kind):key.index("_")])
                v = np.asarray(inputs[f"{kind}_{nm}"][li], dtype=np.float32)
                if nm in ("pk", "pv"):
                    if kind == "lin":
                        v = v[c * NL:(c + 1) * NL, :]
                    else:
                        v = v[:, c * P:(c + 1) * P, :]
                blob[off:off + int(np.prod(shape))] = v.ravel().astype(blob.dtype)
        m = {
            "x_local": np.ascontiguousarray(inputs["x"][:, c * NL:(c + 1) * NL, :], dtype=np.float32),
            "wb": wb,
            "cb": cb,
        }
        in_maps.append(m)
    return in_maps


def kernel(**inputs):
    nc = _get_program()
    in_maps = _make_in_maps(inputs)
    res = run_bass_kernel_spmd(nc, in_maps, core_ids=list(range(NC)))
    out = np.concatenate([res.results[c]["y_local"] for c in range(NC)], axis=1)
    return out.astype(np.float32)


# revision 27
# speedup vs baseline: 27.3023x; 1.0100x over previous
"""ConvLinformer forward on 8 Trainium2 NeuronCores (Bass/Tile).

Sharding: 8-way over sequence (512 tokens/core/batch). Weights replicated
(bf16), except the conv kernels [O,C,S] which are channel(C)-sliced per core;
the conv contraction is channel-sharded via AllToAll of ke/ve activations
followed by AllReduce of the k_/v_ partials (Linformer layers use the same
AllReduce for their sequence-projection partials).

Overlap strategy: collectives are split per-tensor (k vs v) and launched as
soon as their producer finishes; the q projection and the attention score
phase run while the AllReduces are in flight. Attention is two-phase: scores/
softmax/transpose for all (b,h) first (needs k only), then all AV matmuls
(needs v).

Layout: residual kept feature-major f32: x^T = [128, (dt:8, b:2, tl:512)].
All other activations and all weights bf16; PSUM accumulation f32.

Self-contained: shapes hardcoded; host shards inputs / gathers outputs.
"""

import ml_dtypes
import numpy as np

import concourse.bacc as bacc
import concourse.mybir as mybir
import concourse.tile as tile
from concourse.bass_utils import run_bass_kernel_spmd
from concourse.masks import make_identity

P = 128
B, N, D, H, DH, K, S, DFF, L = 2, 4096, 1024, 8, 128, 256, 16, 4096, 2
NC = 8
NL = N // NC          # 512 local tokens per batch
T = B * NL            # 1024 local tokens, free layout (b, tl)
DT = D // P           # 8 feature tiles
DFT = DFF // P        # 32 dff tiles
KT = K // P           # 2 kv-position tiles
SCALE = float(DH) ** -0.5

F32 = mybir.dt.float32
F32R = mybir.dt.float32r
BF16 = mybir.dt.bfloat16
AX = mybir.AxisListType
OP = mybir.AluOpType
AF = mybir.ActivationFunctionType

PARAM_NAMES = [
    "ln1_g", "ln1_b", "wq", "wk", "wv", "pk", "pv", "wo", "bo",
    "ln2_g", "ln2_b", "w1", "b1", "w2", "b2",
]
BF16_PARAMS = {"wq", "wk", "wv", "wo", "w1", "w2", "pk", "pv"}


def _param_shape(kind, nm):
    if nm in ("ln1_g", "ln1_b", "bo", "ln2_g", "ln2_b", "b2"):
        return (D,)
    if nm == "b1":
        return (DFF,)
    if nm in ("wq", "wk", "wv", "wo"):
        return (D, D)
    if nm == "w1":
        return (D, DFF)
    if nm == "w2":
        return (DFF, D)
    # pk / pv: per-core slice
    return (NL, K) if kind == "lin" else (D, P, S)


def _blob_layout():
    """Fixed packing order -> [(key, kind, nm, offset, shape)] per dtype blob."""
    bf, fl = [], []
    off_b = off_f = 0
    for kind in ("lin", "conv"):
        for li in range(L):
            for nm in PARAM_NAMES:
                shape = _param_shape(kind, nm)
                n = int(np.prod(shape))
                key = f"{kind}{li}_{nm}"
                if nm in BF16_PARAMS:
                    bf.append((key, kind, nm, off_b, shape))
                    off_b += n
                else:
                    fl.append((key, kind, nm, off_f, shape))
                    off_f += n
    return bf, fl, off_b, off_f


def _declare_io(nc):
    bf, fl, nb, nf = _blob_layout()
    d = {}
    d["x_local"] = nc.dram_tensor("x_local", [B, NL, D], F32, kind="ExternalInput").ap()
    wb = nc.dram_tensor("wb", [nb], BF16, kind="ExternalInput").ap()
    cb = nc.dram_tensor("cb", [nf], F32, kind="ExternalInput").ap()
    for key, kind, nm, off, shape in bf:
        ap = wb[off:off + int(np.prod(shape))]
        if len(shape) == 2:
            ap = ap.rearrange("(r c) -> r c", c=shape[1])
        elif len(shape) == 3:
            ap = ap.rearrange("(o c s) -> o c s", c=shape[1], s=shape[2])
        d[key] = ap
    for key, kind, nm, off, shape in fl:
        d[key] = cb[off:off + int(np.prod(shape))]
    d["y_local"] = nc.dram_tensor("y_local", [B, NL, D], F32, kind="ExternalOutput").ap()
    return d


class Ctx:
    def __init__(self, nc, tc, io):
        self.nc, self.tc, self.io = nc, tc, io


def _load_col(ctx, dram_vec, width, pool, name):
    """Load a [width*128] dram vector as a [128, width] column tile (f32)."""
    nc = ctx.nc
    t = pool.tile([P, width], F32, name=name)
    nc.sync.dma_start(t[:], dram_vec.rearrange("(w p) -> p w", p=P))
    return t


def _layernorm(ctx, x, g_col, b_col, xn, pfx):
    """xn = LN(x) * g + b -> bf16 feature-major [128, DT*T]."""
    nc, tc = ctx.nc, ctx.tc
    with (
        tc.tile_pool(name=pfx + "sb", bufs=2) as sb,
        tc.tile_pool(name=pfx + "xq", bufs=1) as xqp,
        tc.tile_pool(name=pfx + "ps", bufs=2, space="PSUM") as ps,
        tc.tile_pool(name=pfx + "bps", bufs=2, space="PSUM") as bps,
    ):
        xsq = xqp.tile([P, DT * T], BF16, name="xsq")
        xsq_v = xsq[:].rearrange("p (dt c t) -> p dt c t", dt=DT, c=2)
        x_v = x[:].bitcast(F32).rearrange("p (dt c t) -> p dt c t", dt=DT, c=2)
        for c in range(2):
            nc.scalar.activation(xsq_v[:, :, c], x_v[:, :, c], AF.Square)
        s0_row = sb.tile([1, T], F32R, name="s0row", bufs=1)
        s1_row = sb.tile([1, T], F32R, name="s1row", bufs=1)
        for c in range(2):  # token chunks of 512 (c == batch)
            st1 = ps.tile([1, 512], F32, tag="st")
            st2 = ps.tile([1, 512], F32, tag="st")
            for dt in range(DT):
                nc.tensor.matmul(st1[:], ctx.ones_col[:], x[:, dt * T + c * 512:][:, :512],
                                 start=(dt == 0), stop=(dt == DT - 1))
            for dt in range(DT):
                nc.tensor.matmul(st2[:], ctx.ones_colb[:], xsq[:, dt * T + c * 512:][:, :512],
                                 start=(dt == 0), stop=(dt == DT - 1))
            m_row = sb.tile([1, 512], F32, tag="m")
            nc.vector.tensor_scalar_mul(m_row[:], st1[:], 1.0 / D)
            msq = sb.tile([1, 512], F32, tag="msq")
            nc.vector.tensor_mul(msq[:], m_row[:], m_row[:])
            var = sb.tile([1, 512], F32, tag="var")
            nc.vector.scalar_tensor_tensor(var[:], st2[:], 1.0 / D, msq[:], OP.mult, OP.subtract)
            sd = sb.tile([1, 512], F32, tag="sd")
            nc.scalar.activation(sd[:], var[:], AF.Sqrt, bias=ctx.eps_b[:], scale=1.0)
            r_row = sb.tile([1, 512], F32, tag="r")
            nc.vector.reciprocal(r_row[:], sd[:])
            nc.vector.tensor_copy(s1_row[:, c * 512:][:, :512], r_row[:])
            nc.vector.scalar_tensor_tensor(
                s0_row[:, c * 512:][:, :512], m_row[:], -1.0,
                r_row[:], OP.mult, OP.mult)
        for c in range(2):
            s0bc = bps.tile([P, 512], F32, tag="bc0")
            s1bc = bps.tile([P, 512], F32, tag="bc1")
            nc.tensor.matmul(s0bc[:], ctx.ones_row[:], s0_row[:, c * 512:][:, :512],
                             start=True, stop=True)
            nc.tensor.matmul(s1bc[:], ctx.ones_row[:], s1_row[:, c * 512:][:, :512],
                             start=True, stop=True)
            # stage broadcasts in SBUF so GpSimd can help with the apply
            s0sb = sb.tile([P, 512], F32, tag="s0sb")
            nc.scalar.activation(s0sb[:], s0bc[:], AF.Copy)
            s1sb = sb.tile([P, 512], F32, tag="s1sb")
            nc.scalar.activation(s1sb[:], s1bc[:], AF.Copy)
            for dt in range(DT):
                sl = slice(dt * T + c * 512, dt * T + c * 512 + 512)
                p1 = sb.tile([P, 512], BF16, tag="p1")
                nc.vector.scalar_tensor_tensor(p1[:], x[:, sl].bitcast(F32),
                                               g_col[:, dt:dt + 1], s1sb[:], OP.mult, OP.mult)
                p2 = sb.tile([P, 512], BF16, tag="p2")
                nc.scalar.activation(p2[:], s0sb[:], AF.Identity,
                                     bias=b_col[:, dt:dt + 1], scale=g_col[:, dt:dt + 1])
                nc.gpsimd.tensor_add(xn[:, sl], p1[:], p2[:])


def _proj_T(ctx, w_dram, src, out_cb, pfx):
    """Feature-major projection: psum[ot, c] = sum_dt W[dt,ot].T @ src[dt,c]."""
    nc, tc = ctx.nc, ctx.tc
    with (
        tc.tile_pool(name=pfx + "w", bufs=DT + 1) as wp,
        tc.tile_pool(name=pfx + "ps", bufs=3, space="PSUM") as ps,
    ):
        w_sb = []
        for dt in range(DT):
            wt = wp.tile([P, D], BF16, tag="w", name=f"w{dt}")
            nc.sync.dma_start(wt[:], w_dram[dt * P:(dt + 1) * P, :])
            w_sb.append(wt)
        for c in range(2):
            for ot in range(DT):
                pp = ps.tile([P, 512], F32, tag="pj")
                for dt in range(DT):
                    nc.tensor.matmul(pp[:], w_sb[dt][:, ot * P:(ot + 1) * P],
                                     src[:, dt * T + c * 512:][:, :512],
                                     start=(dt == 0), stop=(dt == DT - 1))
                out_cb(ot, c, pp)


def _lin_kv_one(ctx, li, xn, ten, cc_in, pfx):
    """Linformer k (ten=0) or v (ten=1): full token-major, project, stage."""
    nc, tc = ctx.nc, ctx.tc
    io = ctx.io
    pre = f"lin{li}_"
    with (
        tc.tile_pool(name=pfx + "w", bufs=DT) as wp,
        tc.tile_pool(name=pfx + "kv", bufs=2) as kvp,
        tc.tile_pool(name=pfx + "p", bufs=1) as pp_,
        tc.tile_pool(name=pfx + "ar", bufs=1) as arp,
        tc.tile_pool(name=pfx + "ps", bufs=2, space="PSUM") as ps,
        tc.tile_pool(name=pfx + "ps2", bufs=2, space="PSUM") as ps2,
    ):
        pname = pre + ("pk" if ten == 0 else "pv")
        p_sb = pp_.tile([P, 4 * K], BF16, name="p_sb")
        nc.sync.dma_start(p_sb[:].rearrange("p (nt k) -> p nt k", nt=4),
                          io[pname].rearrange("(nt p) k -> p nt k", p=P))
        wname = pre + ("wk" if ten == 0 else "wv")
        w_sb = []
        for dt in range(DT):
            wt = wp.tile([P, D], BF16, tag="w", name=f"w{dt}")
            nc.sync.dma_start(wt[:], io[wname][dt * P:(dt + 1) * P, :])
            w_sb.append(wt)
        arh = arp.tile([P, 4096], BF16, tag="arh", name="arh")
        for b in range(B):
            full = kvp.tile([P, 4 * D], BF16, tag="full", name="full")
            for nt in range(4):
                for c2 in range(2):
                    fp = ps.tile([P, 512], F32, tag="pf")
                    for dt in range(DT):
                        lhs = xn[:, dt * T + b * 512 + nt * P:][:, :P]
                        nc.tensor.matmul(fp[:], lhs, w_sb[dt][:, c2 * 512:][:, :512],
                                         start=(dt == 0), stop=(dt == DT - 1))
                    nc.vector.tensor_copy(full[:, nt * D + c2 * 512:][:, :512], fp[:])
            if ten == 0:
                # k_^T partials: [dt][128, K] per b (feature-major)
                for dt in range(DT):
                    kp = ps2.tile([P, K], F32, tag="kp")
                    for nt in range(4):
                        nc.tensor.matmul(kp[:], full[:, nt * D + dt * P:][:, :P],
                                         p_sb[:, nt * K:][:, :K],
                                         start=(nt == 0), stop=(nt == 3))
                    nc.vector.tensor_copy(arh[:, b * (DT * K) + dt * K:][:, :K], kp[:])
            else:
                # v_ token-major partials: [kt][128, D] per b
                for kt in range(KT):
                    for c2 in range(2):
                        vp = ps2.tile([P, 512], F32, tag="vp")
                        for nt in range(4):
                            nc.tensor.matmul(vp[:], p_sb[:, nt * K + kt * P:][:, :P],
                                             full[:, nt * D + c2 * 512:][:, :512],
                                             start=(nt == 0), stop=(nt == 3))
                        nc.vector.tensor_copy(
                            arh[:, (b * KT + kt) * D + c2 * 512:][:, :512], vp[:])
        nc.sync.dma_start(cc_in[:], arh[:])


def _conv_wt_prep(ctx, li, ten, wtp, pfx):
    """Transpose conv kernel slice [O,128c,S] -> wt [128c, (s, o)] bf16."""
    nc, tc = ctx.nc, ctx.tc
    io = ctx.io
    pre = f"conv{li}_"
    wname = pre + ("pk" if ten == 0 else "pv")
    wt_sb = wtp.tile([P, S * D], BF16, tag=f"wt{ten}", name=f"wt{ten}")
    with (
        tc.tile_pool(name=pfx + "wn", bufs=2) as wnp,
        tc.tile_pool(name=pfx + "ps", bufs=2, space="PSUM") as ps,
    ):
        for ot in range(DT):
            wn = wnp.tile([P, P * S], BF16, tag="wn")
            nc.sync.dma_start(
                wn[:], io[wname][ot * P:(ot + 1) * P].rearrange("o c s -> o (c s)"))
            for s4 in range(4):
                tp_ps = ps.tile([P, 512], BF16, tag="wtp")
                for si in range(4):
                    s = s4 * 4 + si
                    nc.tensor.transpose(
                        tp_ps[:, si * P:(si + 1) * P],
                        wn[:].rearrange("o (c s) -> o s c", s=S)[:, s],
                        ctx.ident_b[:])
                nc.vector.tensor_copy(
                    wt_sb[:].rearrange("c (s o) -> c s o", s=S)[:, s4 * 4:(s4 + 1) * 4, ot * P:(ot + 1) * P],
                    tp_ps[:].rearrange("c (si o) -> c si o", si=4))
    return wt_sb


def _conv_kv_one(ctx, ten, a2a_out, wt_sb, cc_in, pfx):
    """Contract local c-slice of the A2A'd ke/ve with the sliced kernel."""
    nc, tc = ctx.nc, ctx.tc
    with (
        tc.tile_pool(name=pfx + "cs", bufs=1) as csp,
        tc.tile_pool(name=pfx + "ar", bufs=1) as arp,
        tc.tile_pool(name=pfx + "cps", bufs=2, space="PSUM") as cps,
    ):
        # readback: [128c, (b, peer, tl)]
        ecs = csp.tile([P, B * N], BF16, tag="ecs", name="ecs")
        nc.sync.dma_start(
            ecs[:].rearrange("c (b j t) -> c b j t", b=B, j=NC),
            a2a_out[:].rearrange("j c (b t) -> c b j t", b=B))
        arh = arp.tile([P, 4096], BF16, tag="arh", name="arh")
        if ten == 0:
            # k_^T feature-major: [ot][128, (b, K)]
            for ot in range(DT):
                kp = cps.tile([P, 512], F32, tag="ck")
                for s in range(S):
                    rhs = ecs[:].rearrange("c (b j t) -> c b j t", b=B, j=NC)[:, :, :, s::S]
                    nc.tensor.matmul(kp[:].rearrange("o (b k) -> o b k", b=B),
                                     wt_sb[:, s * D + ot * P:][:, :P], rhs,
                                     start=(s == 0), stop=(s == S - 1))
                nc.vector.tensor_copy(
                    arh[:].rearrange("p (b dt k) -> p b dt k", b=B, dt=DT)[:, :, ot, :],
                    kp[:].rearrange("o (b k) -> o b k", b=B))
        else:
            # v_ token-major: [b, kt][128, D]
            for b in range(B):
                for kt in range(KT):
                    for c2 in range(2):
                        vp = cps.tile([P, 512], F32, tag="cv")
                        for s in range(S):
                            lhs = ecs[:].rearrange(
                                "c (b j t) -> c b j t", b=B, j=NC)[:, b, kt * 4:(kt + 1) * 4, s::S]
                            nc.tensor.matmul(vp[:], lhs,
                                             wt_sb[:, s * D + c2 * 512:][:, :512],
                                             start=(s == 0), stop=(s == S - 1))
                        nc.vector.tensor_copy(
                            arh[:, (b * KT + kt) * D + c2 * 512:][:, :512],
                            vp[:])
        nc.sync.dma_start(cc_in[:], arh[:])


def _attention_scores(ctx, qo_sb, kv_k, at_all, inv_all, pfx):
    """Phase S: per (b,h): transposed scores A^T = exp(K^T q / sqrt(d)) and
    softmax denominators (needs k only). A^T lands in at_all kv-major."""
    nc, tc = ctx.nc, ctx.tc
    with (
        tc.tile_pool(name=pfx + "ps", bufs=3, space="PSUM") as ps_s,
        tc.tile_pool(name=pfx + "pd", bufs=2, space="PSUM") as ps_d,
    ):
        for b in range(B):
            for h in range(H):
                bh = b * H + h
                for kt in range(KT):
                    st_ps = ps_s.tile([P, 512], F32, tag="st")
                    nc.tensor.matmul(st_ps[:],
                                     kv_k[:, b * (DT * K) + h * K + kt * P:][:, :P],
                                     qo_sb[:, h * T + b * 512:][:, :512],
                                     start=True, stop=True)
                    nc.scalar.activation(at_all[:, (bh * KT + kt) * 512:][:, :512],
                                         st_ps[:], AF.Exp, scale=SCALE)
                den_ps = ps_d.tile([1, 512], F32, tag="den")
                for kt in range(KT):
                    nc.tensor.matmul(den_ps[:], ctx.ones_colb[:],
                                     at_all[:, (bh * KT + kt) * 512:][:, :512],
                                     start=(kt == 0), stop=(kt == KT - 1))
                with nc.allow_low_precision(reason="softmax denom broadcast via f32r matmul"):
                    nc.vector.reciprocal(inv_all[:, bh * 512:][:, :512], den_ps[:])


def _attention_av(ctx, qo_sb, kv_v, at_all, inv_all, pfx):
    """Phase O: o = A^T.T @ v per (b,h), normalized by 1/den; overwrites q."""
    nc, tc = ctx.nc, ctx.tc
    with (
        tc.tile_pool(name=pfx + "po", bufs=2, space="PSUM") as ps_o,
        tc.tile_pool(name=pfx + "pb", bufs=2, space="PSUM") as ps_b,
        tc.tile_pool(name=pfx + "ib", bufs=2) as ibp,
    ):
        for b in range(B):
            for h in range(H):
                bh = b * H + h
                oo = ps_o.tile([P, 512], F32, tag="oo")
                for kt in range(KT):
                    nc.tensor.matmul(oo[:],
                                     kv_v[:, (b * KT + kt) * D + h * P:][:, :P],
                                     at_all[:, (bh * KT + kt) * 512:][:, :512],
                                     start=(kt == 0), stop=(kt == KT - 1))
                invbc = ps_b.tile([P, 512], F32, tag="invbc")
                nc.tensor.matmul(invbc[:], ctx.ones_row[:],
                                 inv_all[:, bh * 512:][:, :512], start=True, stop=True)
                invsb = ibp.tile([P, 512], BF16, tag="invsb")
                nc.scalar.activation(invsb[:], invbc[:], AF.Copy)
                nc.vector.tensor_mul(qo_sb[:, h * T + b * 512:][:, :512], oo[:], invsb[:])


def _ffn(ctx, pre, x, xn2, clp, pfx):
    nc, tc = ctx.nc, ctx.tc
    io = ctx.io
    with (
        tc.tile_pool(name=pfx + "h", bufs=1) as hp,
        tc.tile_pool(name=pfx + "w1", bufs=DT + 2) as w1p,
        tc.tile_pool(name=pfx + "w2", bufs=2) as w2p,
        tc.tile_pool(name=pfx + "ph", bufs=3, space="PSUM") as ps_h,
        tc.tile_pool(name=pfx + "pf", bufs=2, space="PSUM") as ps_f,
    ):
        b1_col = _load_col(ctx, io[pre + "b1"], DFT, clp, "b1c")
        b2_col = _load_col(ctx, io[pre + "b2"], DT, clp, "b2c")
        h_sb = hp.tile([P, DFT * T], BF16, name="h_sb")
        for fc in range(8):
            w1_t = []
            for dt in range(DT):
                wt = w1p.tile([P, 512], BF16, tag="w1")
                nc.scalar.dma_start(
                    wt[:], io[pre + "w1"][dt * P:(dt + 1) * P, fc * 512:(fc + 1) * 512])
                w1_t.append(wt)
            for fi in range(4):
                ft = fc * 4 + fi
                for c in range(2):
                    hh = ps_h.tile([P, 512], F32, tag="hh")
                    for dt in range(DT):
                        nc.tensor.matmul(hh[:], w1_t[dt][:, fi * P:(fi + 1) * P],
                                         xn2[:, dt * T + c * 512:][:, :512],
                                         start=(dt == 0), stop=(dt == DT - 1))
                    nc.scalar.activation(h_sb[:, ft * T + c * 512:][:, :512], hh[:],
                                         AF.Gelu, bias=b1_col[:, ft:ft + 1], scale=1.0)
        for c in range(2):  # c-outer: chunk 0 of x finishes early for next LN
            for ot in range(DT):
                w2t = w2p.tile([P, DFT * P], BF16, tag="w2")
                nc.scalar.dma_start(
                    w2t[:].rearrange("p (ft o) -> p ft o", ft=DFT),
                    io[pre + "w2"][:, ot * P:(ot + 1) * P].rearrange("(ft p) o -> p ft o", p=P))
                ff = ps_f.tile([P, 512], F32, tag="ff")
                for ft in range(DFT):
                    nc.tensor.matmul(ff[:], w2t[:, ft * P:(ft + 1) * P],
                                     h_sb[:, ft * T + c * 512:][:, :512],
                                     start=(ft == 0), stop=(ft == DFT - 1))
                sl = slice(ot * T + c * 512, ot * T + c * 512 + 512)
                nc.vector.scalar_tensor_tensor(x[:, sl], ff[:],
                                               b2_col[:, ot:ot + 1], x[:, sl].bitcast(F32),
                                               OP.add, OP.add)


def _build_layer(ctx, li, kind, x):
    nc, tc = ctx.nc, ctx.tc
    io = ctx.io
    pre = f"{kind}{li}_"
    pfx = pre
    rg = [list(range(NC))]
    with (
        tc.tile_pool(name=pfx + "cl", bufs=1) as clp,
        tc.tile_pool(name=pfx + "wa", bufs=1) as wap,
        tc.tile_pool(name=pfx + "dram", bufs=1, space="DRAM") as dp,
    ):
        g1_col = _load_col(ctx, io[pre + "ln1_g"], DT, clp, "g1c")
        b1c_col = _load_col(ctx, io[pre + "ln1_b"], DT, clp, "b1cc")
        bo_col = _load_col(ctx, io[pre + "bo"], DT, clp, "boc")

        xn = wap.tile([P, DT * T], BF16, tag="workA", name="xn")
        _layernorm(ctx, x, g1_col, b1c_col, xn, pfx + "ln1")

        cc_k_in = dp.tile([P, 4096], BF16, tag="cck_in", name="cck_in")
        cc_k_out = dp.tile([P, 4096], BF16, tag="cck_out", name="cck_out", addr_space="Shared")
        cc_v_in = dp.tile([P, 4096], BF16, tag="ccv_in", name="ccv_in")
        cc_v_out = dp.tile([P, 4096], BF16, tag="ccv_out", name="ccv_out", addr_space="Shared")

        with tc.tile_pool(name=pfx + "qo", bufs=1) as qop:
            qo_sb = qop.tile([P, DT * T], BF16, tag="qo", name="qo_sb")

            def q_cb(ot, c, pp):
                nc.vector.tensor_copy(qo_sb[:, ot * T + c * 512:][:, :512], pp[:])

            if kind == "lin":
                # k partials -> AR_k; v partials -> AR_v; then q (overlaps ARs)
                _lin_kv_one(ctx, li, xn, 0, cc_k_in, pfx + "kvk")
                nc.gpsimd.collective_compute(
                    "AllReduce", OP.add, replica_groups=rg,
                    ins=[cc_k_in[:]], outs=[cc_k_out[:]])
                _lin_kv_one(ctx, li, xn, 1, cc_v_in, pfx + "kvv")
                nc.gpsimd.collective_compute(
                    "AllReduce", OP.add, replica_groups=rg,
                    ins=[cc_v_in[:]], outs=[cc_v_out[:]])
                _proj_T(ctx, io[pre + "wq"], xn, q_cb, pfx + "q")
            else:
                a2a_k_in = dp.tile([NC, P, T], BF16, tag="a2ak_in", name="a2ak_in")
                a2a_k_out = dp.tile([NC, P, T], BF16, tag="a2ak_out", name="a2ak_out")
                a2a_v_in = dp.tile([NC, P, T], BF16, tag="a2av_in", name="a2av_in")
                a2a_v_out = dp.tile([NC, P, T], BF16, tag="a2av_out", name="a2av_out")
                with (
                    tc.tile_pool(name=pfx + "kest", bufs=3) as ksp,
                    tc.tile_pool(name=pfx + "wt", bufs=1) as wtp,
                ):
                    def mk_cb(dst):
                        def cb(ot, c, pp):
                            st = ksp.tile([P, 512], BF16, tag="kest", name="kest")
                            nc.vector.tensor_copy(st[:], pp[:])
                            nc.sync.dma_start(dst[ot, :, c * 512:(c + 1) * 512], st[:])
                        return cb
                    _proj_T(ctx, io[pre + "wk"], xn, mk_cb(a2a_k_in), pfx + "ke")
                    nc.gpsimd.collective_compute(
                        "AllToAll", OP.bypass, replica_groups=rg,
                        ins=[a2a_k_in[:]], outs=[a2a_k_out[:]])
                    _proj_T(ctx, io[pre + "wv"], xn, mk_cb(a2a_v_in), pfx + "ve")
                    nc.gpsimd.collective_compute(
                        "AllToAll", OP.bypass, replica_groups=rg,
                        ins=[a2a_v_in[:]], outs=[a2a_v_out[:]])
                    _proj_T(ctx, io[pre + "wq"], xn, q_cb, pfx + "q")
                    wt_k = _conv_wt_prep(ctx, li, 0, wtp, pfx + "wpk")
                    wt_v = _conv_wt_prep(ctx, li, 1, wtp, pfx + "wpv")
                    _conv_kv_one(ctx, 0, a2a_k_out, wt_k, cc_k_in, pfx + "ckk")
                    nc.gpsimd.collective_compute(
                        "AllReduce", OP.add, replica_groups=rg,
                        ins=[cc_k_in[:]], outs=[cc_k_out[:]])
                    _conv_kv_one(ctx, 1, a2a_v_out, wt_v, cc_v_in, pfx + "ckv")
                    nc.gpsimd.collective_compute(
                        "AllReduce", OP.add, replica_groups=rg,
                        ins=[cc_v_in[:]], outs=[cc_v_out[:]])

            with (
                tc.tile_pool(name=pfx + "kvp", bufs=1) as kvp,
                tc.tile_pool(name=pfx + "atp", bufs=1) as atp,
            ):
                kv_k = kvp.tile([P, 4096], BF16, tag="kvk", name="kv_k")
                nc.sync.dma_start(kv_k[:], cc_k_out[:])
                at_all = atp.tile([P, B * H * KT * 512], BF16, name="at_all")
                inv_all = atp.tile([1, B * H * 512], F32R, name="inv_all")
                _attention_scores(ctx, qo_sb, kv_k, at_all, inv_all, pfx + "atS")

                kv_v = kvp.tile([P, 4096], BF16, tag="kvv", name="kv_v")
                nc.sync.dma_start(kv_v[:], cc_v_out[:])
                _attention_av(ctx, qo_sb, kv_v, at_all, inv_all, pfx + "atO")

            def wo_cb(ot, c, pp):
                sl = slice(ot * T + c * 512, ot * T + c * 512 + 512)
                nc.vector.scalar_tensor_tensor(x[:, sl], pp[:],
                                               bo_col[:, ot:ot + 1], x[:, sl].bitcast(F32),
                                               OP.add, OP.add)

            _proj_T(ctx, io[pre + "wo"], qo_sb, wo_cb, pfx + "wo")

        g2_col = _load_col(ctx, io[pre + "ln2_g"], DT, clp, "g2c")
        b2c_col = _load_col(ctx, io[pre + "ln2_b"], DT, clp, "b2cc")
        xn2 = wap.tile([P, DT * T], BF16, tag="workA", name="xn2")
        _layernorm(ctx, x, g2_col, b2c_col, xn2, pfx + "ln2")
        _ffn(ctx, pre, x, xn2, clp, pfx + "ffn")


def build_program():
    nc = bacc.Bacc("TRN2", target_bir_lowering=False, debug=False, num_devices=NC)
    io = _declare_io(nc)
    with tile.TileContext(nc) as tc:
        with (
            tc.tile_pool(name="cst", bufs=1) as cst,
            tc.tile_pool(name="xp", bufs=1) as xp,
        ):
            ctx = Ctx(nc, tc, io)
            ident_f = cst.tile([P, P], F32, name="ident_f")
            make_identity(nc, ident_f[:])
            ctx.ident_r = cst.tile([P, P], F32R, name="ident_r")
            nc.vector.tensor_copy(ctx.ident_r[:], ident_f[:])
            ctx.ident_b = cst.tile([P, P], BF16, name="ident_b")
            nc.vector.tensor_copy(ctx.ident_b[:], ident_f[:])
            oc_f = cst.tile([P, 1], F32, name="oc_f")
            nc.vector.memset(oc_f[:], 1.0)
            ctx.ones_col = cst.tile([P, 1], F32R, name="ones_col")
            nc.vector.tensor_copy(ctx.ones_col[:], oc_f[:])
            ctx.ones_colb = cst.tile([P, 1], BF16, name="ones_colb")
            nc.vector.tensor_copy(ctx.ones_colb[:], oc_f[:])
            or_f = cst.tile([1, P], F32, name="or_f")
            nc.vector.memset(or_f[:], 1.0)
            ctx.ones_row = cst.tile([1, P], F32R, name="ones_row")
            nc.vector.tensor_copy(ctx.ones_row[:], or_f[:])
            ctx.eps_b = cst.tile([1, 1], F32, name="eps_b")
            nc.vector.memset(ctx.eps_b[:], 1e-5)

            # load x -> feature-major x^T
            x = xp.tile([P, DT * T], F32R, name="x")
            with (
                tc.tile_pool(name="iop", bufs=3) as iop,
                tc.tile_pool(name="iops", bufs=2, space="PSUM") as iops,
            ):
                for tt in range(8):  # tt = b*4 + nt
                    b, nt = divmod(tt, 4)
                    xtok = iop.tile([P, D], F32R, tag="xtok")
                    nc.sync.dma_start(xtok[:], io["x_local"][b, nt * P:(nt + 1) * P, :].bitcast(F32R))
                    for dg in range(2):
                        tps = iops.tile([P, 512], F32R, tag="xt")
                        for i in range(4):
                            dt = dg * 4 + i
                            nc.tensor.transpose(tps[:, i * P:(i + 1) * P],
                                                xtok[:, dt * P:(dt + 1) * P], ctx.ident_r[:])
                        nc.vector.tensor_copy(
                            x[:].rearrange("p (dt t) -> p dt t", dt=DT)[:, dg * 4:(dg + 1) * 4,
                                                                        b * 512 + nt * P:][:, :, :P],
                            tps[:].rearrange("p (i t) -> p i t", i=4).bitcast(F32))

            for li in range(L):
                _build_layer(ctx, li, "lin", x)
            for li in range(L):
                _build_layer(ctx, li, "conv", x)

            # write out: transpose back to token-major
            with (
                tc.tile_pool(name="oop", bufs=3) as oop,
                tc.tile_pool(name="oops", bufs=2, space="PSUM") as oops,
            ):
                for tt in range(8):
                    b, nt = divmod(tt, 4)
                    ytok = oop.tile([P, D], F32, tag="ytok")
                    for dg in range(2):
                        tps = oops.tile([P, 512], F32R, tag="yt")
                        for i in range(4):
                            dt = dg * 4 + i
                            nc.tensor.transpose(tps[:, i * P:(i + 1) * P],
                                                x[:, dt * T + b * 512 + nt * P:][:, :P],
                                                ctx.ident_r[:])
                        nc.vector.tensor_copy(ytok[:, dg * 512:(dg + 1) * 512], tps[:].bitcast(F32))
                    nc.sync.dma_start(io["y_local"][b, nt * P:(nt + 1) * P, :], ytok[:])
    nc.compile()
    return nc


_PROGRAM = None


def _get_program():
    global _PROGRAM
    if _PROGRAM is None:
        _PROGRAM = build_program()
    return _PROGRAM


def _make_in_maps(inputs):
    bf, fl, nb, nf = _blob_layout()
    in_maps = []
    for c in range(NC):
        wb = np.empty(nb, dtype=ml_dtypes.bfloat16)
        cb = np.empty(nf, dtype=np.float32)
        for entries, blob in ((bf, wb), (fl, cb)):
            for key, kind, nm, off, shape in entries:
                li = int(key[len(kind):key.index("_")])
                v = np.asarray(inputs[f"{kind}_{nm}"][li], dtype=np.float32)
                if nm in ("pk", "pv"):
                    if kind == "lin":
                        v = v[c * NL:(c + 1) * NL, :]
                    else:
                        v = v[:, c * P:(c + 1) * P, :]
                blob[off:off + int(np.prod(shape))] = v.ravel().astype(blob.dtype)
        m = {
            "x_local": np.ascontiguousarray(inputs["x"][:, c * NL:(c + 1) * NL, :], dtype=np.float32),
            "wb": wb,
            "cb": cb,
        }
        in_maps.append(m)
    return in_maps


def kernel(**inputs):
    nc = _get_program()
    in_maps = _make_in_maps(inputs)
    res = run_bass_kernel_spmd(nc, in_maps, core_ids=list(range(NC)))
    out = np.concatenate([res.results[c]["y_local"] for c in range(NC)], axis=1)
    return out.astype(np.float32)


# revision 29
# speedup vs baseline: 27.3518x; 1.0018x over previous
"""ConvLinformer forward on 8 Trainium2 NeuronCores (Bass/Tile).

Sharding: 8-way over sequence (512 tokens/core/batch). Weights replicated
(bf16), except the conv kernels [O,C,S] which are channel(C)-sliced per core;
the conv contraction is channel-sharded via AllToAll of ke/ve activations
followed by AllReduce of the k_/v_ partials (Linformer layers use the same
AllReduce for their sequence-projection partials).

Overlap strategy: collectives are split per-tensor (k vs v) and launched as
soon as their producer finishes; the q projection and the attention score
phase run while the AllReduces are in flight. Attention is two-phase: scores/
softmax/transpose for all (b,h) first (needs k only), then all AV matmuls
(needs v).

Layout: residual kept feature-major f32: x^T = [128, (dt:8, b:2, tl:512)].
All other activations and all weights bf16; PSUM accumulation f32.

Self-contained: shapes hardcoded; host shards inputs / gathers outputs.
"""

import ml_dtypes
import numpy as np

import concourse.bacc as bacc
import concourse.mybir as mybir
import concourse.tile as tile
from concourse.bass_utils import run_bass_kernel_spmd
from concourse.masks import make_identity

P = 128
B, N, D, H, DH, K, S, DFF, L = 2, 4096, 1024, 8, 128, 256, 16, 4096, 2
NC = 8
NL = N // NC          # 512 local tokens per batch
T = B * NL            # 1024 local tokens, free layout (b, tl)
DT = D // P           # 8 feature tiles
DFT = DFF // P        # 32 dff tiles
KT = K // P           # 2 kv-position tiles
SCALE = float(DH) ** -0.5

F32 = mybir.dt.float32
F32R = mybir.dt.float32r
BF16 = mybir.dt.bfloat16
AX = mybir.AxisListType
OP = mybir.AluOpType
AF = mybir.ActivationFunctionType

PARAM_NAMES = [
    "ln1_g", "ln1_b", "wq", "wk", "wv", "pk", "pv", "wo", "bo",
    "ln2_g", "ln2_b", "w1", "b1", "w2", "b2",
]
BF16_PARAMS = {"wq", "wk", "wv", "wo", "w1", "w2", "pk", "pv"}


def _param_shape(kind, nm):
    if nm in ("ln1_g", "ln1_b", "bo", "ln2_g", "ln2_b", "b2"):
        return (D,)
    if nm == "b1":
        return (DFF,)
    if nm in ("wq", "wk", "wv", "wo"):
        return (D, D)
    if nm == "w1":
        return (D, DFF)
    if nm == "w2":
        return (DFF, D)
    # pk / pv: per-core slice
    return (NL, K) if kind == "lin" else (D, P, S)


def _blob_layout():
    """Fixed packing order -> [(key, kind, nm, offset, shape)] per dtype blob."""
    bf, fl = [], []
    off_b = off_f = 0
    for kind in ("lin", "conv"):
        for li in range(L):
            for nm in PARAM_NAMES:
                shape = _param_shape(kind, nm)
                n = int(np.prod(shape))
                key = f"{kind}{li}_{nm}"
                if nm in BF16_PARAMS:
                    bf.append((key, kind, nm, off_b, shape))
                    off_b += n
                else:
                    fl.append((key, kind, nm, off_f, shape))
                    off_f += n
    return bf, fl, off_b, off_f


def _declare_io(nc):
    bf, fl, nb, nf = _blob_layout()
    d = {}
    d["x_local"] = nc.dram_tensor("x_local", [B, NL, D], F32, kind="ExternalInput").ap()
    wb = nc.dram_tensor("wb", [nb], BF16, kind="ExternalInput").ap()
    cb = nc.dram_tensor("cb", [nf], F32, kind="ExternalInput").ap()
    for key, kind, nm, off, shape in bf:
        ap = wb[off:off + int(np.prod(shape))]
        if len(shape) == 2:
            ap = ap.rearrange("(r c) -> r c", c=shape[1])
        elif len(shape) == 3:
            ap = ap.rearrange("(o c s) -> o c s", c=shape[1], s=shape[2])
        d[key] = ap
    for key, kind, nm, off, shape in fl:
        d[key] = cb[off:off + int(np.prod(shape))]
    d["y_local"] = nc.dram_tensor("y_local", [B, NL, D], F32, kind="ExternalOutput").ap()
    return d


class Ctx:
    def __init__(self, nc, tc, io):
        self.nc, self.tc, self.io = nc, tc, io


def _load_col(ctx, dram_vec, width, pool, name):
    """Load a [width*128] dram vector as a [128, width] column tile (f32)."""
    nc = ctx.nc
    t = pool.tile([P, width], F32, name=name)
    nc.sync.dma_start(t[:], dram_vec.rearrange("(w p) -> p w", p=P))
    return t


def _layernorm(ctx, x, g_col, b_col, xn, pfx):
    """xn = LN(x) * g + b -> bf16 feature-major [128, DT*T]."""
    nc, tc = ctx.nc, ctx.tc
    with (
        tc.tile_pool(name=pfx + "sb", bufs=2) as sb,
        tc.tile_pool(name=pfx + "xq", bufs=1) as xqp,
        tc.tile_pool(name=pfx + "ps", bufs=2, space="PSUM") as ps,
        tc.tile_pool(name=pfx + "bps", bufs=2, space="PSUM") as bps,
    ):
        xsq = xqp.tile([P, DT * T], BF16, name="xsq")
        xsq_v = xsq[:].rearrange("p (dt c t) -> p dt c t", dt=DT, c=2)
        x_v = x[:].bitcast(F32).rearrange("p (dt c t) -> p dt c t", dt=DT, c=2)
        for c in range(2):
            nc.gpsimd.tensor_mul(xsq_v[:, :, c], x_v[:, :, c], x_v[:, :, c])
        s0_row = sb.tile([1, T], F32R, name="s0row", bufs=1)
        s1_row = sb.tile([1, T], F32R, name="s1row", bufs=1)
        for c in range(2):  # token chunks of 512 (c == batch)
            st1 = ps.tile([1, 512], F32, tag="st")
            st2 = ps.tile([1, 512], F32, tag="st")
            for dt in range(DT):
                nc.tensor.matmul(st1[:], ctx.ones_col[:], x[:, dt * T + c * 512:][:, :512],
                                 start=(dt == 0), stop=(dt == DT - 1))
            for dt in range(DT):
                nc.tensor.matmul(st2[:], ctx.ones_colb[:], xsq[:, dt * T + c * 512:][:, :512],
                                 start=(dt == 0), stop=(dt == DT - 1))
            m_row = sb.tile([1, 512], F32, tag="m")
            nc.vector.tensor_scalar_mul(m_row[:], st1[:], 1.0 / D)
            msq = sb.tile([1, 512], F32, tag="msq")
            nc.vector.tensor_mul(msq[:], m_row[:], m_row[:])
            var = sb.tile([1, 512], F32, tag="var")
            nc.vector.scalar_tensor_tensor(var[:], st2[:], 1.0 / D, msq[:], OP.mult, OP.subtract)
            sd = sb.tile([1, 512], F32, tag="sd")
            nc.scalar.activation(sd[:], var[:], AF.Sqrt, bias=ctx.eps_b[:], scale=1.0)
            r_row = sb.tile([1, 512], F32, tag="r")
            nc.vector.reciprocal(r_row[:], sd[:])
            nc.vector.tensor_copy(s1_row[:, c * 512:][:, :512], r_row[:])
            nc.vector.scalar_tensor_tensor(
                s0_row[:, c * 512:][:, :512], m_row[:], -1.0,
                r_row[:], OP.mult, OP.mult)
        for c in range(2):
            s0bc = bps.tile([P, 512], F32, tag="bc0")
            s1bc = bps.tile([P, 512], F32, tag="bc1")
            nc.tensor.matmul(s0bc[:], ctx.ones_row[:], s0_row[:, c * 512:][:, :512],
                             start=True, stop=True)
            nc.tensor.matmul(s1bc[:], ctx.ones_row[:], s1_row[:, c * 512:][:, :512],
                             start=True, stop=True)
            # stage broadcasts in SBUF so GpSimd can help with the apply
            s0sb = sb.tile([P, 512], F32, tag="s0sb")
            nc.scalar.activation(s0sb[:], s0bc[:], AF.Copy)
            s1sb = sb.tile([P, 512], F32, tag="s1sb")
            nc.scalar.activation(s1sb[:], s1bc[:], AF.Copy)
            for dt in range(DT):
                sl = slice(dt * T + c * 512, dt * T + c * 512 + 512)
                p1 = sb.tile([P, 512], BF16, tag="p1")
                nc.vector.scalar_tensor_tensor(p1[:], x[:, sl].bitcast(F32),
                                               g_col[:, dt:dt + 1], s1sb[:], OP.mult, OP.mult)
                p2 = sb.tile([P, 512], BF16, tag="p2")
                nc.scalar.activation(p2[:], s0sb[:], AF.Identity,
                                     bias=b_col[:, dt:dt + 1], scale=g_col[:, dt:dt + 1])
                nc.gpsimd.tensor_add(xn[:, sl], p1[:], p2[:])


def _proj_T(ctx, w_dram, src, out_cb, pfx):
    """Feature-major projection: psum[ot, c] = sum_dt W[dt,ot].T @ src[dt,c]."""
    nc, tc = ctx.nc, ctx.tc
    with (
        tc.tile_pool(name=pfx + "w", bufs=DT + 1) as wp,
        tc.tile_pool(name=pfx + "ps", bufs=3, space="PSUM") as ps,
    ):
        w_sb = []
        for dt in range(DT):
            wt = wp.tile([P, D], BF16, tag="w", name=f"w{dt}")
            nc.sync.dma_start(wt[:], w_dram[dt * P:(dt + 1) * P, :])
            w_sb.append(wt)
        for c in range(2):
            for ot in range(DT):
                pp = ps.tile([P, 512], F32, tag="pj")
                for dt in range(DT):
                    nc.tensor.matmul(pp[:], w_sb[dt][:, ot * P:(ot + 1) * P],
                                     src[:, dt * T + c * 512:][:, :512],
                                     start=(dt == 0), stop=(dt == DT - 1))
                out_cb(ot, c, pp)


def _lin_kv_one(ctx, li, xn, ten, cc_in, pfx):
    """Linformer k (ten=0) or v (ten=1): full token-major, project, stage."""
    nc, tc = ctx.nc, ctx.tc
    io = ctx.io
    pre = f"lin{li}_"
    with (
        tc.tile_pool(name=pfx + "w", bufs=DT) as wp,
        tc.tile_pool(name=pfx + "kv", bufs=2) as kvp,
        tc.tile_pool(name=pfx + "p", bufs=1) as pp_,
        tc.tile_pool(name=pfx + "ar", bufs=1) as arp,
        tc.tile_pool(name=pfx + "ps", bufs=2, space="PSUM") as ps,
        tc.tile_pool(name=pfx + "ps2", bufs=2, space="PSUM") as ps2,
    ):
        pname = pre + ("pk" if ten == 0 else "pv")
        p_sb = pp_.tile([P, 4 * K], BF16, name="p_sb")
        nc.sync.dma_start(p_sb[:].rearrange("p (nt k) -> p nt k", nt=4),
                          io[pname].rearrange("(nt p) k -> p nt k", p=P))
        wname = pre + ("wk" if ten == 0 else "wv")
        w_sb = []
        for dt in range(DT):
            wt = wp.tile([P, D], BF16, tag="w", name=f"w{dt}")
            nc.sync.dma_start(wt[:], io[wname][dt * P:(dt + 1) * P, :])
            w_sb.append(wt)
        arh = arp.tile([P, 4096], BF16, tag="arh", name="arh")
        for b in range(B):
            full = kvp.tile([P, 4 * D], BF16, tag="full", name="full")
            for nt in range(4):
                for c2 in range(2):
                    fp = ps.tile([P, 512], F32, tag="pf")
                    for dt in range(DT):
                        lhs = xn[:, dt * T + b * 512 + nt * P:][:, :P]
                        nc.tensor.matmul(fp[:], lhs, w_sb[dt][:, c2 * 512:][:, :512],
                                         start=(dt == 0), stop=(dt == DT - 1))
                    nc.vector.tensor_copy(full[:, nt * D + c2 * 512:][:, :512], fp[:])
            if ten == 0:
                # k_^T partials: [dt][128, K] per b (feature-major)
                for dt in range(DT):
                    kp = ps2.tile([P, K], F32, tag="kp")
                    for nt in range(4):
                        nc.tensor.matmul(kp[:], full[:, nt * D + dt * P:][:, :P],
                                         p_sb[:, nt * K:][:, :K],
                                         start=(nt == 0), stop=(nt == 3))
                    nc.vector.tensor_copy(arh[:, b * (DT * K) + dt * K:][:, :K], kp[:])
            else:
                # v_ token-major partials: [kt][128, D] per b
                for kt in range(KT):
                    for c2 in range(2):
                        vp = ps2.tile([P, 512], F32, tag="vp")
                        for nt in range(4):
                            nc.tensor.matmul(vp[:], p_sb[:, nt * K + kt * P:][:, :P],
                                             full[:, nt * D + c2 * 512:][:, :512],
                                             start=(nt == 0), stop=(nt == 3))
                        nc.vector.tensor_copy(
                            arh[:, (b * KT + kt) * D + c2 * 512:][:, :512], vp[:])
        nc.sync.dma_start(cc_in[:], arh[:])


def _conv_wt_prep(ctx, li, ten, wtp, pfx):
    """Transpose conv kernel slice [O,128c,S] -> wt [128c, (s, o)] bf16."""
    nc, tc = ctx.nc, ctx.tc
    io = ctx.io
    pre = f"conv{li}_"
    wname = pre + ("pk" if ten == 0 else "pv")
    wt_sb = wtp.tile([P, S * D], BF16, tag=f"wt{ten}", name=f"wt{ten}")
    with (
        tc.tile_pool(name=pfx + "wn", bufs=2) as wnp,
        tc.tile_pool(name=pfx + "ps", bufs=2, space="PSUM") as ps,
    ):
        for ot in range(DT):
            wn = wnp.tile([P, P * S], BF16, tag="wn")
            nc.sync.dma_start(
                wn[:], io[wname][ot * P:(ot + 1) * P].rearrange("o c s -> o (c s)"))
            for s4 in range(4):
                tp_ps = ps.tile([P, 512], BF16, tag="wtp")
                for si in range(4):
                    s = s4 * 4 + si
                    nc.tensor.transpose(
                        tp_ps[:, si * P:(si + 1) * P],
                        wn[:].rearrange("o (c s) -> o s c", s=S)[:, s],
                        ctx.ident_b[:])
                nc.vector.tensor_copy(
                    wt_sb[:].rearrange("c (s o) -> c s o", s=S)[:, s4 * 4:(s4 + 1) * 4, ot * P:(ot + 1) * P],
                    tp_ps[:].rearrange("c (si o) -> c si o", si=4))
    return wt_sb


def _conv_kv_one(ctx, ten, a2a_out, wt_sb, cc_in, pfx):
    """Contract local c-slice of the A2A'd ke/ve with the sliced kernel."""
    nc, tc = ctx.nc, ctx.tc
    with (
        tc.tile_pool(name=pfx + "cs", bufs=1) as csp,
        tc.tile_pool(name=pfx + "ar", bufs=1) as arp,
        tc.tile_pool(name=pfx + "cps", bufs=2, space="PSUM") as cps,
    ):
        # readback: [128c, (b, peer, tl)]
        ecs = csp.tile([P, B * N], BF16, tag="ecs", name="ecs")
        nc.sync.dma_start(
            ecs[:].rearrange("c (b j t) -> c b j t", b=B, j=NC),
            a2a_out[:].rearrange("j c (b t) -> c b j t", b=B))
        arh = arp.tile([P, 4096], BF16, tag="arh", name="arh")
        if ten == 0:
            # k_^T feature-major: [ot][128, (b, K)]
            for ot in range(DT):
                kp = cps.tile([P, 512], F32, tag="ck")
                for s in range(S):
                    rhs = ecs[:].rearrange("c (b j t) -> c b j t", b=B, j=NC)[:, :, :, s::S]
                    nc.tensor.matmul(kp[:].rearrange("o (b k) -> o b k", b=B),
                                     wt_sb[:, s * D + ot * P:][:, :P], rhs,
                                     start=(s == 0), stop=(s == S - 1))
                nc.vector.tensor_copy(
                    arh[:].rearrange("p (b dt k) -> p b dt k", b=B, dt=DT)[:, :, ot, :],
                    kp[:].rearrange("o (b k) -> o b k", b=B))
        else:
            # v_ token-major: [b, kt][128, D]
            for b in range(B):
                for kt in range(KT):
                    for c2 in range(2):
                        vp = cps.tile([P, 512], F32, tag="cv")
                        for s in range(S):
                            lhs = ecs[:].rearrange(
                                "c (b j t) -> c b j t", b=B, j=NC)[:, b, kt * 4:(kt + 1) * 4, s::S]
                            nc.tensor.matmul(vp[:], lhs,
                                             wt_sb[:, s * D + c2 * 512:][:, :512],
                                             start=(s == 0), stop=(s == S - 1))
                        nc.vector.tensor_copy(
                            arh[:, (b * KT + kt) * D + c2 * 512:][:, :512],
                            vp[:])
        nc.sync.dma_start(cc_in[:], arh[:])


def _attention_scores(ctx, qo_sb, kv_k, at_all, inv_all, pfx):
    """Phase S: per (b,h): transposed scores A^T = exp(K^T q / sqrt(d)) and
    softmax denominators (needs k only). A^T lands in at_all kv-major."""
    nc, tc = ctx.nc, ctx.tc
    with (
        tc.tile_pool(name=pfx + "ps", bufs=3, space="PSUM") as ps_s,
        tc.tile_pool(name=pfx + "pd", bufs=2, space="PSUM") as ps_d,
    ):
        for b in range(B):
            for h in range(H):
                bh = b * H + h
                for kt in range(KT):
                    st_ps = ps_s.tile([P, 512], F32, tag="st")
                    nc.tensor.matmul(st_ps[:],
                                     kv_k[:, b * (DT * K) + h * K + kt * P:][:, :P],
                                     qo_sb[:, h * T + b * 512:][:, :512],
                                     start=True, stop=True)
                    nc.scalar.activation(at_all[:, (bh * KT + kt) * 512:][:, :512],
                                         st_ps[:], AF.Exp, scale=SCALE)
                den_ps = ps_d.tile([1, 512], F32, tag="den")
                for kt in range(KT):
                    nc.tensor.matmul(den_ps[:], ctx.ones_colb[:],
                                     at_all[:, (bh * KT + kt) * 512:][:, :512],
                                     start=(kt == 0), stop=(kt == KT - 1))
                with nc.allow_low_precision(reason="softmax denom broadcast via f32r matmul"):
                    nc.vector.reciprocal(inv_all[:, bh * 512:][:, :512], den_ps[:])


def _attention_av(ctx, qo_sb, kv_v, at_all, inv_all, pfx):
    """Phase O: o = A^T.T @ v per (b,h), normalized by 1/den; overwrites q."""
    nc, tc = ctx.nc, ctx.tc
    with (
        tc.tile_pool(name=pfx + "po", bufs=2, space="PSUM") as ps_o,
        tc.tile_pool(name=pfx + "pb", bufs=2, space="PSUM") as ps_b,
        tc.tile_pool(name=pfx + "ib", bufs=2) as ibp,
    ):
        for b in range(B):
            for h in range(H):
                bh = b * H + h
                oo = ps_o.tile([P, 512], F32, tag="oo")
                for kt in range(KT):
                    nc.tensor.matmul(oo[:],
                                     kv_v[:, (b * KT + kt) * D + h * P:][:, :P],
                                     at_all[:, (bh * KT + kt) * 512:][:, :512],
                                     start=(kt == 0), stop=(kt == KT - 1))
                invbc = ps_b.tile([P, 512], F32, tag="invbc")
                nc.tensor.matmul(invbc[:], ctx.ones_row[:],
                                 inv_all[:, bh * 512:][:, :512], start=True, stop=True)
                invsb = ibp.tile([P, 512], BF16, tag="invsb")
                nc.scalar.activation(invsb[:], invbc[:], AF.Copy)
                nc.vector.tensor_mul(qo_sb[:, h * T + b * 512:][:, :512], oo[:], invsb[:])


def _ffn(ctx, pre, x, xn2, clp, pfx):
    nc, tc = ctx.nc, ctx.tc
    io = ctx.io
    with (
        tc.tile_pool(name=pfx + "h", bufs=1) as hp,
        tc.tile_pool(name=pfx + "w1", bufs=DT + 2) as w1p,
        tc.tile_pool(name=pfx + "w2", bufs=2) as w2p,
        tc.tile_pool(name=pfx + "ph", bufs=3, space="PSUM") as ps_h,
        tc.tile_pool(name=pfx + "pf", bufs=2, space="PSUM") as ps_f,
    ):
        b1_col = _load_col(ctx, io[pre + "b1"], DFT, clp, "b1c")
        b2_col = _load_col(ctx, io[pre + "b2"], DT, clp, "b2c")
        h_sb = hp.tile([P, DFT * T], BF16, name="h_sb")
        for fc in range(8):
            w1_t = []
            for dt in range(DT):
                wt = w1p.tile([P, 512], BF16, tag="w1")
                nc.scalar.dma_start(
                    wt[:], io[pre + "w1"][dt * P:(dt + 1) * P, fc * 512:(fc + 1) * 512])
                w1_t.append(wt)
            for fi in range(4):
                ft = fc * 4 + fi
                for c in range(2):
                    hh = ps_h.tile([P, 512], F32, tag="hh")
                    for dt in range(DT):
                        nc.tensor.matmul(hh[:], w1_t[dt][:, fi * P:(fi + 1) * P],
                                         xn2[:, dt * T + c * 512:][:, :512],
                                         start=(dt == 0), stop=(dt == DT - 1))
                    nc.scalar.activation(h_sb[:, ft * T + c * 512:][:, :512], hh[:],
                                         AF.Gelu, bias=b1_col[:, ft:ft + 1], scale=1.0)
        for c in range(2):  # c-outer: chunk 0 of x finishes early for next LN
            for ot in range(DT):
                w2t = w2p.tile([P, DFT * P], BF16, tag="w2")
                nc.scalar.dma_start(
                    w2t[:].rearrange("p (ft o) -> p ft o", ft=DFT),
                    io[pre + "w2"][:, ot * P:(ot + 1) * P].rearrange("(ft p) o -> p ft o", p=P))
                ff = ps_f.tile([P, 512], F32, tag="ff")
                for ft in range(DFT):
                    nc.tensor.matmul(ff[:], w2t[:, ft * P:(ft + 1) * P],
                                     h_sb[:, ft * T + c * 512:][:, :512],
                                     start=(ft == 0), stop=(ft == DFT - 1))
                sl = slice(ot * T + c * 512, ot * T + c * 512 + 512)
                nc.vector.scalar_tensor_tensor(x[:, sl], ff[:],
                                               b2_col[:, ot:ot + 1], x[:, sl].bitcast(F32),
                                               OP.add, OP.add)


def _build_layer(ctx, li, kind, x):
    nc, tc = ctx.nc, ctx.tc
    io = ctx.io
    pre = f"{kind}{li}_"
    pfx = pre
    rg = [list(range(NC))]
    with (
        tc.tile_pool(name=pfx + "cl", bufs=1) as clp,
        tc.tile_pool(name=pfx + "wa", bufs=1) as wap,
        tc.tile_pool(name=pfx + "dram", bufs=1, space="DRAM") as dp,
    ):
        g1_col = _load_col(ctx, io[pre + "ln1_g"], DT, clp, "g1c")
        b1c_col = _load_col(ctx, io[pre + "ln1_b"], DT, clp, "b1cc")
        bo_col = _load_col(ctx, io[pre + "bo"], DT, clp, "boc")

        xn = wap.tile([P, DT * T], BF16, tag="workA", name="xn")
        _layernorm(ctx, x, g1_col, b1c_col, xn, pfx + "ln1")

        cc_k_in = dp.tile([P, 4096], BF16, tag="cck_in", name="cck_in")
        cc_k_out = dp.tile([P, 4096], BF16, tag="cck_out", name="cck_out", addr_space="Shared")
        cc_v_in = dp.tile([P, 4096], BF16, tag="ccv_in", name="ccv_in")
        cc_v_out = dp.tile([P, 4096], BF16, tag="ccv_out", name="ccv_out", addr_space="Shared")

        with tc.tile_pool(name=pfx + "qo", bufs=1) as qop:
            qo_sb = qop.tile([P, DT * T], BF16, tag="qo", name="qo_sb")

            def q_cb(ot, c, pp):
                nc.vector.tensor_copy(qo_sb[:, ot * T + c * 512:][:, :512], pp[:])

            if kind == "lin":
                # k partials -> AR_k; v partials -> AR_v; then q (overlaps ARs)
                _lin_kv_one(ctx, li, xn, 0, cc_k_in, pfx + "kvk")
                nc.gpsimd.collective_compute(
                    "AllReduce", OP.add, replica_groups=rg,
                    ins=[cc_k_in[:]], outs=[cc_k_out[:]])
                _lin_kv_one(ctx, li, xn, 1, cc_v_in, pfx + "kvv")
                nc.gpsimd.collective_compute(
                    "AllReduce", OP.add, replica_groups=rg,
                    ins=[cc_v_in[:]], outs=[cc_v_out[:]])
                _proj_T(ctx, io[pre + "wq"], xn, q_cb, pfx + "q")
            else:
                a2a_k_in = dp.tile([NC, P, T], BF16, tag="a2ak_in", name="a2ak_in")
                a2a_k_out = dp.tile([NC, P, T], BF16, tag="a2ak_out", name="a2ak_out")
                a2a_v_in = dp.tile([NC, P, T], BF16, tag="a2av_in", name="a2av_in")
                a2a_v_out = dp.tile([NC, P, T], BF16, tag="a2av_out", name="a2av_out")
                with (
                    tc.tile_pool(name=pfx + "kest", bufs=3) as ksp,
                    tc.tile_pool(name=pfx + "wt", bufs=1) as wtp,
                ):
                    def mk_cb(dst):
                        def cb(ot, c, pp):
                            st = ksp.tile([P, 512], BF16, tag="kest", name="kest")
                            nc.vector.tensor_copy(st[:], pp[:])
                            nc.sync.dma_start(dst[ot, :, c * 512:(c + 1) * 512], st[:])
                        return cb
                    _proj_T(ctx, io[pre + "wk"], xn, mk_cb(a2a_k_in), pfx + "ke")
                    nc.gpsimd.collective_compute(
                        "AllToAll", OP.bypass, replica_groups=rg,
                        ins=[a2a_k_in[:]], outs=[a2a_k_out[:]])
                    _proj_T(ctx, io[pre + "wv"], xn, mk_cb(a2a_v_in), pfx + "ve")
                    nc.gpsimd.collective_compute(
                        "AllToAll", OP.bypass, replica_groups=rg,
                        ins=[a2a_v_in[:]], outs=[a2a_v_out[:]])
                    _proj_T(ctx, io[pre + "wq"], xn, q_cb, pfx + "q")
                    wt_k = _conv_wt_prep(ctx, li, 0, wtp, pfx + "wpk")
                    wt_v = _conv_wt_prep(ctx, li, 1, wtp, pfx + "wpv")
                    _conv_kv_one(ctx, 0, a2a_k_out, wt_k, cc_k_in, pfx + "ckk")
                    nc.gpsimd.collective_compute(
                        "AllReduce", OP.add, replica_groups=rg,
                        ins=[cc_k_in[:]], outs=[cc_k_out[:]])
                    _conv_kv_one(ctx, 1, a2a_v_out, wt_v, cc_v_in, pfx + "ckv")
                    nc.gpsimd.collective_compute(
                        "AllReduce", OP.add, replica_groups=rg,
                        ins=[cc_v_in[:]], outs=[cc_v_out[:]])

            with (
                tc.tile_pool(name=pfx + "kvp", bufs=1) as kvp,
                tc.tile_pool(name=pfx + "atp", bufs=1) as atp,
            ):
                kv_k = kvp.tile([P, 4096], BF16, tag="kvk", name="kv_k")
                nc.sync.dma_start(kv_k[:], cc_k_out[:])
                at_all = atp.tile([P, B * H * KT * 512], BF16, name="at_all")
                inv_all = atp.tile([1, B * H * 512], F32R, name="inv_all")
                _attention_scores(ctx, qo_sb, kv_k, at_all, inv_all, pfx + "atS")

                kv_v = kvp.tile([P, 4096], BF16, tag="kvv", name="kv_v")
                nc.sync.dma_start(kv_v[:], cc_v_out[:])
                _attention_av(ctx, qo_sb, kv_v, at_all, inv_all, pfx + "atO")

            def wo_cb(ot, c, pp):
                sl = slice(ot * T + c * 512, ot * T + c * 512 + 512)
                nc.vector.scalar_tensor_tensor(x[:, sl], pp[:],
                                               bo_col[:, ot:ot + 1], x[:, sl].bitcast(F32),
                                               OP.add, OP.add)

            _proj_T(ctx, io[pre + "wo"], qo_sb, wo_cb, pfx + "wo")

        g2_col = _load_col(ctx, io[pre + "ln2_g"], DT, clp, "g2c")
        b2c_col = _load_col(ctx, io[pre + "ln2_b"], DT, clp, "b2cc")
        xn2 = wap.tile([P, DT * T], BF16, tag="workA", name="xn2")
        _layernorm(ctx, x, g2_col, b2c_col, xn2, pfx + "ln2")
        _ffn(ctx, pre, x, xn2, clp, pfx + "ffn")


def build_program():
    nc = bacc.Bacc("TRN2", target_bir_lowering=False, debug=False, num_devices=NC)
    io = _declare_io(nc)
    with tile.TileContext(nc) as tc:
        with (
            tc.tile_pool(name="cst", bufs=1) as cst,
            tc.tile_pool(name="xp", bufs=1) as xp,
        ):
            ctx = Ctx(nc, tc, io)
            ident_f = cst.tile([P, P], F32, name="ident_f")
            make_identity(nc, ident_f[:])
            ctx.ident_r = cst.tile([P, P], F32R, name="ident_r")
            nc.vector.tensor_copy(ctx.ident_r[:], ident_f[:])
            ctx.ident_b = cst.tile([P, P], BF16, name="ident_b")
            nc.vector.tensor_copy(ctx.ident_b[:], ident_f[:])
            oc_f = cst.tile([P, 1], F32, name="oc_f")
            nc.vector.memset(oc_f[:], 1.0)
            ctx.ones_col = cst.tile([P, 1], F32R, name="ones_col")
            nc.vector.tensor_copy(ctx.ones_col[:], oc_f[:])
            ctx.ones_colb = cst.tile([P, 1], BF16, name="ones_colb")
            nc.vector.tensor_copy(ctx.ones_colb[:], oc_f[:])
            or_f = cst.tile([1, P], F32, name="or_f")
            nc.vector.memset(or_f[:], 1.0)
            ctx.ones_row = cst.tile([1, P], F32R, name="ones_row")
            nc.vector.tensor_copy(ctx.ones_row[:], or_f[:])
            ctx.eps_b = cst.tile([1, 1], F32, name="eps_b")
            nc.vector.memset(ctx.eps_b[:], 1e-5)

            # load x -> feature-major x^T
            x = xp.tile([P, DT * T], F32R, name="x")
            with (
                tc.tile_pool(name="iop", bufs=3) as iop,
                tc.tile_pool(name="iops", bufs=2, space="PSUM") as iops,
            ):
                for tt in range(8):  # tt = b*4 + nt
                    b, nt = divmod(tt, 4)
                    xtok = iop.tile([P, D], F32R, tag="xtok")
                    nc.sync.dma_start(xtok[:], io["x_local"][b, nt * P:(nt + 1) * P, :].bitcast(F32R))
                    for dg in range(2):
                        tps = iops.tile([P, 512], F32R, tag="xt")
                        for i in range(4):
                            dt = dg * 4 + i
                            nc.tensor.transpose(tps[:, i * P:(i + 1) * P],
                                                xtok[:, dt * P:(dt + 1) * P], ctx.ident_r[:])
                        nc.vector.tensor_copy(
                            x[:].rearrange("p (dt t) -> p dt t", dt=DT)[:, dg * 4:(dg + 1) * 4,
                                                                        b * 512 + nt * P:][:, :, :P],
                            tps[:].rearrange("p (i t) -> p i t", i=4).bitcast(F32))

            for li in range(L):
                _build_layer(ctx, li, "lin", x)
            for li in range(L):
                _build_layer(ctx, li, "conv", x)

            # write out: transpose back to token-major
            with (
                tc.tile_pool(name="oop", bufs=3) as oop,
                tc.tile_pool(name="oops", bufs=2, space="PSUM") as oops,
            ):
                for tt in range(8):
                    b, nt = divmod(tt, 4)
                    ytok = oop.tile([P, D], F32, tag="ytok")
                    for dg in range(2):
                        tps = oops.tile([P, 512], F32R, tag="yt")
                        for i in range(4):
                            dt = dg * 4 + i
                            nc.tensor.transpose(tps[:, i * P:(i + 1) * P],
                                                x[:, dt * T + b * 512 + nt * P:][:, :P],
                                                ctx.ident_r[:])
                        nc.vector.tensor_copy(ytok[:, dg * 512:(dg + 1) * 512], tps[:].bitcast(F32))
                    nc.sync.dma_start(io["y_local"][b, nt * P:(nt + 1) * P, :], ytok[:])
    nc.compile()
    return nc


_PROGRAM = None


def _get_program():
    global _PROGRAM
    if _PROGRAM is None:
        _PROGRAM = build_program()
    return _PROGRAM


def _make_in_maps(inputs):
    bf, fl, nb, nf = _blob_layout()
    in_maps = []
    for c in range(NC):
        wb = np.empty(nb, dtype=ml_dtypes.bfloat16)
        cb = np.empty(nf, dtype=np.float32)
        for entries, blob in ((bf, wb), (fl, cb)):
            for key, kind, nm, off, shape in entries:
                li = int(key[len(kind):key.index("_")])
                v = np.asarray(inputs[f"{kind}_{nm}"][li], dtype=np.float32)
                if nm in ("pk", "pv"):
                    if kind == "lin":
                        v = v[c * NL:(c + 1) * NL, :]
                    else:
                        v = v[:, c * P:(c + 1) * P, :]
                blob[off:off + int(np.prod(shape))] = v.ravel().astype(blob.dtype)
        m = {
            "x_local": np.ascontiguousarray(inputs["x"][:, c * NL:(c + 1) * NL, :], dtype=np.float32),
            "wb": wb,
            "cb": cb,
        }
        in_maps.append(m)
    return in_maps


def kernel(**inputs):
    nc = _get_program()
    in_maps = _make_in_maps(inputs)
    res = run_bass_kernel_spmd(nc, in_maps, core_ids=list(range(NC)))
    out = np.concatenate([res.results[c]["y_local"] for c in range(NC)], axis=1)
    return out.astype(np.float32)


# revision 30
# speedup vs baseline: 27.5645x; 1.0078x over previous
"""ConvLinformer forward on 8 Trainium2 NeuronCores (Bass/Tile).

Sharding: 8-way over sequence (512 tokens/core/batch). Weights replicated
(bf16), except the conv kernels [O,C,S] which are channel(C)-sliced per core;
the conv contraction is channel-sharded via AllToAll of ke/ve activations
followed by AllReduce of the k_/v_ partials (Linformer layers use the same
AllReduce for their sequence-projection partials).

Overlap strategy: collectives are split per-tensor (k vs v) and launched as
soon as their producer finishes; the q projection and the attention score
phase run while the AllReduces are in flight. Attention is two-phase: scores/
softmax/transpose for all (b,h) first (needs k only), then all AV matmuls
(needs v).

Layout: residual kept feature-major f32: x^T = [128, (dt:8, b:2, tl:512)].
All other activations and all weights bf16; PSUM accumulation f32.

Self-contained: shapes hardcoded; host shards inputs / gathers outputs.
"""

import ml_dtypes
import numpy as np

import concourse.bacc as bacc
import concourse.mybir as mybir
import concourse.tile as tile
from concourse.bass_utils import run_bass_kernel_spmd
from concourse.masks import make_identity

P = 128
B, N, D, H, DH, K, S, DFF, L = 2, 4096, 1024, 8, 128, 256, 16, 4096, 2
NC = 8
NL = N // NC          # 512 local tokens per batch
T = B * NL            # 1024 local tokens, free layout (b, tl)
DT = D // P           # 8 feature tiles
DFT = DFF // P        # 32 dff tiles
KT = K // P           # 2 kv-position tiles
SCALE = float(DH) ** -0.5

F32 = mybir.dt.float32
F32R = mybir.dt.float32r
BF16 = mybir.dt.bfloat16
AX = mybir.AxisListType
OP = mybir.AluOpType
AF = mybir.ActivationFunctionType

PARAM_NAMES = [
    "ln1_g", "ln1_b", "wq", "wk", "wv", "pk", "pv", "wo", "bo",
    "ln2_g", "ln2_b", "w1", "b1", "w2", "b2",
]
BF16_PARAMS = {"wq", "wk", "wv", "wo", "w1", "w2", "pk", "pv"}


def _param_shape(kind, nm):
    if nm in ("ln1_g", "ln1_b", "bo", "ln2_g", "ln2_b", "b2"):
        return (D,)
    if nm == "b1":
        return (DFF,)
    if nm in ("wq", "wk", "wv", "wo"):
        return (D, D)
    if nm == "w1":
        return (D, DFF)
    if nm == "w2":
        return (DFF, D)
    # pk / pv: per-core slice
    return (NL, K) if kind == "lin" else (D, P, S)


def _blob_layout():
    """Fixed packing order -> [(key, kind, nm, offset, shape)] per dtype blob."""
    bf, fl = [], []
    off_b = off_f = 0
    for kind in ("lin", "conv"):
        for li in range(L):
            for nm in PARAM_NAMES:
                shape = _param_shape(kind, nm)
                n = int(np.prod(shape))
                key = f"{kind}{li}_{nm}"
                if nm in BF16_PARAMS:
                    bf.append((key, kind, nm, off_b, shape))
                    off_b += n
                else:
                    fl.append((key, kind, nm, off_f, shape))
                    off_f += n
    return bf, fl, off_b, off_f


def _declare_io(nc):
    bf, fl, nb, nf = _blob_layout()
    d = {}
    d["x_local"] = nc.dram_tensor("x_local", [B, NL, D], F32, kind="ExternalInput").ap()
    wb = nc.dram_tensor("wb", [nb], BF16, kind="ExternalInput").ap()
    cb = nc.dram_tensor("cb", [nf], F32, kind="ExternalInput").ap()
    for key, kind, nm, off, shape in bf:
        ap = wb[off:off + int(np.prod(shape))]
        if len(shape) == 2:
            ap = ap.rearrange("(r c) -> r c", c=shape[1])
        elif len(shape) == 3:
            ap = ap.rearrange("(o c s) -> o c s", c=shape[1], s=shape[2])
        d[key] = ap
    for key, kind, nm, off, shape in fl:
        d[key] = cb[off:off + int(np.prod(shape))]
    d["y_local"] = nc.dram_tensor("y_local", [B, NL, D], F32, kind="ExternalOutput").ap()
    return d


class Ctx:
    def __init__(self, nc, tc, io):
        self.nc, self.tc, self.io = nc, tc, io


def _load_col(ctx, dram_vec, width, pool, name):
    """Load a [width*128] dram vector as a [128, width] column tile (f32)."""
    nc = ctx.nc
    t = pool.tile([P, width], F32, name=name)
    nc.sync.dma_start(t[:], dram_vec.rearrange("(w p) -> p w", p=P))
    return t


def _layernorm(ctx, x, g_col, b_col, xn, pfx):
    """xn = LN(x) * g + b -> bf16 feature-major [128, DT*T]."""
    nc, tc = ctx.nc, ctx.tc
    with (
        tc.tile_pool(name=pfx + "sb", bufs=2) as sb,
        tc.tile_pool(name=pfx + "xq", bufs=1) as xqp,
        tc.tile_pool(name=pfx + "ps", bufs=2, space="PSUM") as ps,
        tc.tile_pool(name=pfx + "bps", bufs=2, space="PSUM") as bps,
    ):
        xsq = xqp.tile([P, DT * T], BF16, name="xsq")
        xsq_v = xsq[:].rearrange("p (dt c t) -> p dt c t", dt=DT, c=2)
        x_v = x[:].bitcast(F32).rearrange("p (dt c t) -> p dt c t", dt=DT, c=2)
        for c in range(2):
            nc.gpsimd.tensor_mul(xsq_v[:, :, c], x_v[:, :, c], x_v[:, :, c])
        s0_row = sb.tile([1, T], F32R, name="s0row", bufs=1)
        s1_row = sb.tile([1, T], F32R, name="s1row", bufs=1)
        for c in range(2):  # token chunks of 512 (c == batch)
            st1 = ps.tile([1, 512], F32, tag="st")
            st2 = ps.tile([1, 512], F32, tag="st")
            for dt in range(DT):
                nc.tensor.matmul(st1[:], ctx.ones_col[:], x[:, dt * T + c * 512:][:, :512],
                                 start=(dt == 0), stop=(dt == DT - 1))
            for dt in range(DT):
                nc.tensor.matmul(st2[:], ctx.ones_colb[:], xsq[:, dt * T + c * 512:][:, :512],
                                 start=(dt == 0), stop=(dt == DT - 1))
            m_row = sb.tile([1, 512], F32, tag="m")
            nc.vector.tensor_scalar_mul(m_row[:], st1[:], 1.0 / D)
            msq = sb.tile([1, 512], F32, tag="msq")
            nc.vector.tensor_mul(msq[:], m_row[:], m_row[:])
            var = sb.tile([1, 512], F32, tag="var")
            nc.vector.scalar_tensor_tensor(var[:], st2[:], 1.0 / D, msq[:], OP.mult, OP.subtract)
            sd = sb.tile([1, 512], F32, tag="sd")
            nc.scalar.activation(sd[:], var[:], AF.Sqrt, bias=ctx.eps_b[:], scale=1.0)
            r_row = sb.tile([1, 512], F32, tag="r")
            nc.vector.reciprocal(r_row[:], sd[:])
            nc.vector.tensor_copy(s1_row[:, c * 512:][:, :512], r_row[:])
            nc.vector.scalar_tensor_tensor(
                s0_row[:, c * 512:][:, :512], m_row[:], -1.0,
                r_row[:], OP.mult, OP.mult)
        for c in range(2):
            s0bc = bps.tile([P, 512], F32, tag="bc0")
            s1bc = bps.tile([P, 512], F32, tag="bc1")
            nc.tensor.matmul(s0bc[:], ctx.ones_row[:], s0_row[:, c * 512:][:, :512],
                             start=True, stop=True)
            nc.tensor.matmul(s1bc[:], ctx.ones_row[:], s1_row[:, c * 512:][:, :512],
                             start=True, stop=True)
            # stage broadcasts in SBUF so GpSimd can help with the apply
            s0sb = sb.tile([P, 512], F32, tag="s0sb")
            nc.scalar.activation(s0sb[:], s0bc[:], AF.Copy)
            s1sb = sb.tile([P, 512], F32, tag="s1sb")
            nc.scalar.activation(s1sb[:], s1bc[:], AF.Copy)
            for dt in range(DT):
                sl = slice(dt * T + c * 512, dt * T + c * 512 + 512)
                p1 = sb.tile([P, 512], BF16, tag="p1")
                nc.vector.scalar_tensor_tensor(p1[:], x[:, sl].bitcast(F32),
                                               g_col[:, dt:dt + 1], s1sb[:], OP.mult, OP.mult)
                p2 = sb.tile([P, 512], BF16, tag="p2")
                nc.scalar.activation(p2[:], s0sb[:], AF.Identity,
                                     bias=b_col[:, dt:dt + 1], scale=g_col[:, dt:dt + 1])
                nc.gpsimd.tensor_add(xn[:, sl], p1[:], p2[:])


def _proj_T(ctx, w_dram, src, out_cb, pfx):
    """Feature-major projection: psum[ot, c] = sum_dt W[dt,ot].T @ src[dt,c]."""
    nc, tc = ctx.nc, ctx.tc
    with (
        tc.tile_pool(name=pfx + "w", bufs=DT + 1) as wp,
        tc.tile_pool(name=pfx + "ps", bufs=4, space="PSUM") as ps,
    ):
        w_sb = []
        for dt in range(DT):
            wt = wp.tile([P, D], BF16, tag="w", name=f"w{dt}")
            nc.sync.dma_start(wt[:], w_dram[dt * P:(dt + 1) * P, :])
            w_sb.append(wt)
        for c in range(2):
            for ot in range(DT):
                pp = ps.tile([P, 512], F32, tag="pj")
                for dt in range(DT):
                    nc.tensor.matmul(pp[:], w_sb[dt][:, ot * P:(ot + 1) * P],
                                     src[:, dt * T + c * 512:][:, :512],
                                     start=(dt == 0), stop=(dt == DT - 1))
                out_cb(ot, c, pp)


def _lin_kv_one(ctx, li, xn, ten, cc_in, pfx):
    """Linformer k (ten=0) or v (ten=1): full token-major, project, stage."""
    nc, tc = ctx.nc, ctx.tc
    io = ctx.io
    pre = f"lin{li}_"
    with (
        tc.tile_pool(name=pfx + "w", bufs=DT) as wp,
        tc.tile_pool(name=pfx + "kv", bufs=2) as kvp,
        tc.tile_pool(name=pfx + "p", bufs=1) as pp_,
        tc.tile_pool(name=pfx + "ar", bufs=1) as arp,
        tc.tile_pool(name=pfx + "ps", bufs=3, space="PSUM") as ps,
        tc.tile_pool(name=pfx + "ps2", bufs=3, space="PSUM") as ps2,
    ):
        pname = pre + ("pk" if ten == 0 else "pv")
        p_sb = pp_.tile([P, 4 * K], BF16, name="p_sb")
        nc.sync.dma_start(p_sb[:].rearrange("p (nt k) -> p nt k", nt=4),
                          io[pname].rearrange("(nt p) k -> p nt k", p=P))
        wname = pre + ("wk" if ten == 0 else "wv")
        w_sb = []
        for dt in range(DT):
            wt = wp.tile([P, D], BF16, tag="w", name=f"w{dt}")
            nc.sync.dma_start(wt[:], io[wname][dt * P:(dt + 1) * P, :])
            w_sb.append(wt)
        arh = arp.tile([P, 4096], BF16, tag="arh", name="arh")
        for b in range(B):
            full = kvp.tile([P, 4 * D], BF16, tag="full", name="full")
            for nt in range(4):
                for c2 in range(2):
                    fp = ps.tile([P, 512], F32, tag="pf")
                    for dt in range(DT):
                        lhs = xn[:, dt * T + b * 512 + nt * P:][:, :P]
                        nc.tensor.matmul(fp[:], lhs, w_sb[dt][:, c2 * 512:][:, :512],
                                         start=(dt == 0), stop=(dt == DT - 1))
                    nc.vector.tensor_copy(full[:, nt * D + c2 * 512:][:, :512], fp[:])
            if ten == 0:
                # k_^T partials: [dt][128, K] per b (feature-major)
                for dt in range(DT):
                    kp = ps2.tile([P, K], F32, tag="kp")
                    for nt in range(4):
                        nc.tensor.matmul(kp[:], full[:, nt * D + dt * P:][:, :P],
                                         p_sb[:, nt * K:][:, :K],
                                         start=(nt == 0), stop=(nt == 3))
                    nc.vector.tensor_copy(arh[:, b * (DT * K) + dt * K:][:, :K], kp[:])
            else:
                # v_ token-major partials: [kt][128, D] per b
                for kt in range(KT):
                    for c2 in range(2):
                        vp = ps2.tile([P, 512], F32, tag="vp")
                        for nt in range(4):
                            nc.tensor.matmul(vp[:], p_sb[:, nt * K + kt * P:][:, :P],
                                             full[:, nt * D + c2 * 512:][:, :512],
                                             start=(nt == 0), stop=(nt == 3))
                        nc.vector.tensor_copy(
                            arh[:, (b * KT + kt) * D + c2 * 512:][:, :512], vp[:])
        nc.sync.dma_start(cc_in[:], arh[:])


def _conv_wt_prep(ctx, li, ten, wtp, pfx):
    """Transpose conv kernel slice [O,128c,S] -> wt [128c, (s, o)] bf16."""
    nc, tc = ctx.nc, ctx.tc
    io = ctx.io
    pre = f"conv{li}_"
    wname = pre + ("pk" if ten == 0 else "pv")
    wt_sb = wtp.tile([P, S * D], BF16, tag=f"wt{ten}", name=f"wt{ten}")
    with (
        tc.tile_pool(name=pfx + "wn", bufs=2) as wnp,
        tc.tile_pool(name=pfx + "ps", bufs=2, space="PSUM") as ps,
    ):
        for ot in range(DT):
            wn = wnp.tile([P, P * S], BF16, tag="wn")
            nc.sync.dma_start(
                wn[:], io[wname][ot * P:(ot + 1) * P].rearrange("o c s -> o (c s)"))
            for s4 in range(4):
                tp_ps = ps.tile([P, 512], BF16, tag="wtp")
                for si in range(4):
                    s = s4 * 4 + si
                    nc.tensor.transpose(
                        tp_ps[:, si * P:(si + 1) * P],
                        wn[:].rearrange("o (c s) -> o s c", s=S)[:, s],
                        ctx.ident_b[:])
                nc.vector.tensor_copy(
                    wt_sb[:].rearrange("c (s o) -> c s o", s=S)[:, s4 * 4:(s4 + 1) * 4, ot * P:(ot + 1) * P],
                    tp_ps[:].rearrange("c (si o) -> c si o", si=4))
    return wt_sb


def _conv_kv_one(ctx, ten, a2a_out, wt_sb, cc_in, pfx):
    """Contract local c-slice of the A2A'd ke/ve with the sliced kernel."""
    nc, tc = ctx.nc, ctx.tc
    with (
        tc.tile_pool(name=pfx + "cs", bufs=1) as csp,
        tc.tile_pool(name=pfx + "ar", bufs=1) as arp,
        tc.tile_pool(name=pfx + "cps", bufs=2, space="PSUM") as cps,
    ):
        # readback: [128c, (b, peer, tl)]
        ecs = csp.tile([P, B * N], BF16, tag="ecs", name="ecs")
        nc.sync.dma_start(
            ecs[:].rearrange("c (b j t) -> c b j t", b=B, j=NC),
            a2a_out[:].rearrange("j c (b t) -> c b j t", b=B))
        arh = arp.tile([P, 4096], BF16, tag="arh", name="arh")
        if ten == 0:
            # k_^T feature-major: [ot][128, (b, K)]
            for ot in range(DT):
                kp = cps.tile([P, 512], F32, tag="ck")
                for s in range(S):
                    rhs = ecs[:].rearrange("c (b j t) -> c b j t", b=B, j=NC)[:, :, :, s::S]
                    nc.tensor.matmul(kp[:].rearrange("o (b k) -> o b k", b=B),
                                     wt_sb[:, s * D + ot * P:][:, :P], rhs,
                                     start=(s == 0), stop=(s == S - 1))
                nc.vector.tensor_copy(
                    arh[:].rearrange("p (b dt k) -> p b dt k", b=B, dt=DT)[:, :, ot, :],
                    kp[:].rearrange("o (b k) -> o b k", b=B))
        else:
            # v_ token-major: [b, kt][128, D]
            for b in range(B):
                for kt in range(KT):
                    for c2 in range(2):
                        vp = cps.tile([P, 512], F32, tag="cv")
                        for s in range(S):
                            lhs = ecs[:].rearrange(
                                "c (b j t) -> c b j t", b=B, j=NC)[:, b, kt * 4:(kt + 1) * 4, s::S]
                            nc.tensor.matmul(vp[:], lhs,
                                             wt_sb[:, s * D + c2 * 512:][:, :512],
                                             start=(s == 0), stop=(s == S - 1))
                        nc.vector.tensor_copy(
                            arh[:, (b * KT + kt) * D + c2 * 512:][:, :512],
                            vp[:])
        nc.sync.dma_start(cc_in[:], arh[:])


def _attention_scores(ctx, qo_sb, kv_k, at_all, inv_all, pfx):
    """Phase S: per (b,h): transposed scores A^T = exp(K^T q / sqrt(d)) and
    softmax denominators (needs k only). A^T lands in at_all kv-major."""
    nc, tc = ctx.nc, ctx.tc
    with (
        tc.tile_pool(name=pfx + "ps", bufs=3, space="PSUM") as ps_s,
        tc.tile_pool(name=pfx + "pd", bufs=2, space="PSUM") as ps_d,
    ):
        for b in range(B):
            for h in range(H):
                bh = b * H + h
                for kt in range(KT):
                    st_ps = ps_s.tile([P, 512], F32, tag="st")
                    nc.tensor.matmul(st_ps[:],
                                     kv_k[:, b * (DT * K) + h * K + kt * P:][:, :P],
                                     qo_sb[:, h * T + b * 512:][:, :512],
                                     start=True, stop=True)
                    nc.scalar.activation(at_all[:, (bh * KT + kt) * 512:][:, :512],
                                         st_ps[:], AF.Exp, scale=SCALE)
                den_ps = ps_d.tile([1, 512], F32, tag="den")
                for kt in range(KT):
                    nc.tensor.matmul(den_ps[:], ctx.ones_colb[:],
                                     at_all[:, (bh * KT + kt) * 512:][:, :512],
                                     start=(kt == 0), stop=(kt == KT - 1))
                with nc.allow_low_precision(reason="softmax denom broadcast via f32r matmul"):
                    nc.vector.reciprocal(inv_all[:, bh * 512:][:, :512], den_ps[:])


def _attention_av(ctx, qo_sb, kv_v, at_all, inv_all, pfx):
    """Phase O: o = A^T.T @ v per (b,h), normalized by 1/den; overwrites q."""
    nc, tc = ctx.nc, ctx.tc
    with (
        tc.tile_pool(name=pfx + "po", bufs=2, space="PSUM") as ps_o,
        tc.tile_pool(name=pfx + "pb", bufs=2, space="PSUM") as ps_b,
        tc.tile_pool(name=pfx + "ib", bufs=2) as ibp,
    ):
        for b in range(B):
            for h in range(H):
                bh = b * H + h
                oo = ps_o.tile([P, 512], F32, tag="oo")
                for kt in range(KT):
                    nc.tensor.matmul(oo[:],
                                     kv_v[:, (b * KT + kt) * D + h * P:][:, :P],
                                     at_all[:, (bh * KT + kt) * 512:][:, :512],
                                     start=(kt == 0), stop=(kt == KT - 1))
                invbc = ps_b.tile([P, 512], F32, tag="invbc")
                nc.tensor.matmul(invbc[:], ctx.ones_row[:],
                                 inv_all[:, bh * 512:][:, :512], start=True, stop=True)
                invsb = ibp.tile([P, 512], BF16, tag="invsb")
                nc.scalar.activation(invsb[:], invbc[:], AF.Copy)
                nc.vector.tensor_mul(qo_sb[:, h * T + b * 512:][:, :512], oo[:], invsb[:])


def _ffn(ctx, pre, x, xn2, clp, pfx):
    nc, tc = ctx.nc, ctx.tc
    io = ctx.io
    with (
        tc.tile_pool(name=pfx + "h", bufs=1) as hp,
        tc.tile_pool(name=pfx + "w1", bufs=DT + 2) as w1p,
        tc.tile_pool(name=pfx + "w2", bufs=2) as w2p,
        tc.tile_pool(name=pfx + "ph", bufs=4, space="PSUM") as ps_h,
        tc.tile_pool(name=pfx + "pf", bufs=3, space="PSUM") as ps_f,
    ):
        b1_col = _load_col(ctx, io[pre + "b1"], DFT, clp, "b1c")
        b2_col = _load_col(ctx, io[pre + "b2"], DT, clp, "b2c")
        h_sb = hp.tile([P, DFT * T], BF16, name="h_sb")
        for fc in range(8):
            w1_t = []
            for dt in range(DT):
                wt = w1p.tile([P, 512], BF16, tag="w1")
                nc.scalar.dma_start(
                    wt[:], io[pre + "w1"][dt * P:(dt + 1) * P, fc * 512:(fc + 1) * 512])
                w1_t.append(wt)
            for fi in range(4):
                ft = fc * 4 + fi
                for c in range(2):
                    hh = ps_h.tile([P, 512], F32, tag="hh")
                    for dt in range(DT):
                        nc.tensor.matmul(hh[:], w1_t[dt][:, fi * P:(fi + 1) * P],
                                         xn2[:, dt * T + c * 512:][:, :512],
                                         start=(dt == 0), stop=(dt == DT - 1))
                    nc.scalar.activation(h_sb[:, ft * T + c * 512:][:, :512], hh[:],
                                         AF.Gelu, bias=b1_col[:, ft:ft + 1], scale=1.0)
        for c in range(2):  # c-outer: chunk 0 of x finishes early for next LN
            for ot in range(DT):
                w2t = w2p.tile([P, DFT * P], BF16, tag="w2")
                nc.scalar.dma_start(
                    w2t[:].rearrange("p (ft o) -> p ft o", ft=DFT),
                    io[pre + "w2"][:, ot * P:(ot + 1) * P].rearrange("(ft p) o -> p ft o", p=P))
                ff = ps_f.tile([P, 512], F32, tag="ff")
                for ft in range(DFT):
                    nc.tensor.matmul(ff[:], w2t[:, ft * P:(ft + 1) * P],
                                     h_sb[:, ft * T + c * 512:][:, :512],
                                     start=(ft == 0), stop=(ft == DFT - 1))
                sl = slice(ot * T + c * 512, ot * T + c * 512 + 512)
                nc.vector.scalar_tensor_tensor(x[:, sl], ff[:],
                                               b2_col[:, ot:ot + 1], x[:, sl].bitcast(F32),
                                               OP.add, OP.add)


def _build_layer(ctx, li, kind, x):
    nc, tc = ctx.nc, ctx.tc
    io = ctx.io
    pre = f"{kind}{li}_"
    pfx = pre
    rg = [list(range(NC))]
    with (
        tc.tile_pool(name=pfx + "cl", bufs=1) as clp,
        tc.tile_pool(name=pfx + "wa", bufs=1) as wap,
        tc.tile_pool(name=pfx + "dram", bufs=1, space="DRAM") as dp,
    ):
        g1_col = _load_col(ctx, io[pre + "ln1_g"], DT, clp, "g1c")
        b1c_col = _load_col(ctx, io[pre + "ln1_b"], DT, clp, "b1cc")
        bo_col = _load_col(ctx, io[pre + "bo"], DT, clp, "boc")

        xn = wap.tile([P, DT * T], BF16, tag="workA", name="xn")
        _layernorm(ctx, x, g1_col, b1c_col, xn, pfx + "ln1")

        cc_k_in = dp.tile([P, 4096], BF16, tag="cck_in", name="cck_in")
        cc_k_out = dp.tile([P, 4096], BF16, tag="cck_out", name="cck_out", addr_space="Shared")
        cc_v_in = dp.tile([P, 4096], BF16, tag="ccv_in", name="ccv_in")
        cc_v_out = dp.tile([P, 4096], BF16, tag="ccv_out", name="ccv_out", addr_space="Shared")

        with tc.tile_pool(name=pfx + "qo", bufs=1) as qop:
            qo_sb = qop.tile([P, DT * T], BF16, tag="qo", name="qo_sb")

            def q_cb(ot, c, pp):
                nc.vector.tensor_copy(qo_sb[:, ot * T + c * 512:][:, :512], pp[:])

            if kind == "lin":
                # k partials -> AR_k; v partials -> AR_v; then q (overlaps ARs)
                _lin_kv_one(ctx, li, xn, 0, cc_k_in, pfx + "kvk")
                nc.gpsimd.collective_compute(
                    "AllReduce", OP.add, replica_groups=rg,
                    ins=[cc_k_in[:]], outs=[cc_k_out[:]])
                _lin_kv_one(ctx, li, xn, 1, cc_v_in, pfx + "kvv")
                nc.gpsimd.collective_compute(
                    "AllReduce", OP.add, replica_groups=rg,
                    ins=[cc_v_in[:]], outs=[cc_v_out[:]])
                _proj_T(ctx, io[pre + "wq"], xn, q_cb, pfx + "q")
            else:
                a2a_k_in = dp.tile([NC, P, T], BF16, tag="a2ak_in", name="a2ak_in")
                a2a_k_out = dp.tile([NC, P, T], BF16, tag="a2ak_out", name="a2ak_out")
                a2a_v_in = dp.tile([NC, P, T], BF16, tag="a2av_in", name="a2av_in")
                a2a_v_out = dp.tile([NC, P, T], BF16, tag="a2av_out", name="a2av_out")
                with (
                    tc.tile_pool(name=pfx + "kest", bufs=3) as ksp,
                    tc.tile_pool(name=pfx + "wt", bufs=1) as wtp,
                ):
                    def mk_cb(dst):
                        def cb(ot, c, pp):
                            st = ksp.tile([P, 512], BF16, tag="kest", name="kest")
                            nc.vector.tensor_copy(st[:], pp[:])
                            nc.sync.dma_start(dst[ot, :, c * 512:(c + 1) * 512], st[:])
                        return cb
                    _proj_T(ctx, io[pre + "wk"], xn, mk_cb(a2a_k_in), pfx + "ke")
                    nc.gpsimd.collective_compute(
                        "AllToAll", OP.bypass, replica_groups=rg,
                        ins=[a2a_k_in[:]], outs=[a2a_k_out[:]])
                    _proj_T(ctx, io[pre + "wv"], xn, mk_cb(a2a_v_in), pfx + "ve")
                    nc.gpsimd.collective_compute(
                        "AllToAll", OP.bypass, replica_groups=rg,
                        ins=[a2a_v_in[:]], outs=[a2a_v_out[:]])
                    _proj_T(ctx, io[pre + "wq"], xn, q_cb, pfx + "q")
                    wt_k = _conv_wt_prep(ctx, li, 0, wtp, pfx + "wpk")
                    wt_v = _conv_wt_prep(ctx, li, 1, wtp, pfx + "wpv")
                    _conv_kv_one(ctx, 0, a2a_k_out, wt_k, cc_k_in, pfx + "ckk")
                    nc.gpsimd.collective_compute(
                        "AllReduce", OP.add, replica_groups=rg,
                        ins=[cc_k_in[:]], outs=[cc_k_out[:]])
                    _conv_kv_one(ctx, 1, a2a_v_out, wt_v, cc_v_in, pfx + "ckv")
                    nc.gpsimd.collective_compute(
                        "AllReduce", OP.add, replica_groups=rg,
                        ins=[cc_v_in[:]], outs=[cc_v_out[:]])

            with (
                tc.tile_pool(name=pfx + "kvp", bufs=1) as kvp,
                tc.tile_pool(name=pfx + "atp", bufs=1) as atp,
            ):
                kv_k = kvp.tile([P, 4096], BF16, tag="kvk", name="kv_k")
                nc.sync.dma_start(kv_k[:], cc_k_out[:])
                at_all = atp.tile([P, B * H * KT * 512], BF16, name="at_all")
                inv_all = atp.tile([1, B * H * 512], F32R, name="inv_all")
                _attention_scores(ctx, qo_sb, kv_k, at_all, inv_all, pfx + "atS")

                kv_v = kvp.tile([P, 4096], BF16, tag="kvv", name="kv_v")
                nc.sync.dma_start(kv_v[:], cc_v_out[:])
                _attention_av(ctx, qo_sb, kv_v, at_all, inv_all, pfx + "atO")

            def wo_cb(ot, c, pp):
                sl = slice(ot * T + c * 512, ot * T + c * 512 + 512)
                nc.vector.scalar_tensor_tensor(x[:, sl], pp[:],
                                               bo_col[:, ot:ot + 1], x[:, sl].bitcast(F32),
                                               OP.add, OP.add)

            _proj_T(ctx, io[pre + "wo"], qo_sb, wo_cb, pfx + "wo")

        g2_col = _load_col(ctx, io[pre + "ln2_g"], DT, clp, "g2c")
        b2c_col = _load_col(ctx, io[pre + "ln2_b"], DT, clp, "b2cc")
        xn2 = wap.tile([P, DT * T], BF16, tag="workA", name="xn2")
        _layernorm(ctx, x, g2_col, b2c_col, xn2, pfx + "ln2")
        _ffn(ctx, pre, x, xn2, clp, pfx + "ffn")


def build_program():
    nc = bacc.Bacc("TRN2", target_bir_lowering=False, debug=False, num_devices=NC)
    io = _declare_io(nc)
    with tile.TileContext(nc) as tc:
        with (
            tc.tile_pool(name="cst", bufs=1) as cst,
            tc.tile_pool(name="xp", bufs=1) as xp,
        ):
            ctx = Ctx(nc, tc, io)
            ident_f = cst.tile([P, P], F32, name="ident_f")
            make_identity(nc, ident_f[:])
            ctx.ident_r = cst.tile([P, P], F32R, name="ident_r")
            nc.vector.tensor_copy(ctx.ident_r[:], ident_f[:])
            ctx.ident_b = cst.tile([P, P], BF16, name="ident_b")
            nc.vector.tensor_copy(ctx.ident_b[:], ident_f[:])
            oc_f = cst.tile([P, 1], F32, name="oc_f")
            nc.vector.memset(oc_f[:], 1.0)
            ctx.ones_col = cst.tile([P, 1], F32R, name="ones_col")
            nc.vector.tensor_copy(ctx.ones_col[:], oc_f[:])
            ctx.ones_colb = cst.tile([P, 1], BF16, name="ones_colb")
            nc.vector.tensor_copy(ctx.ones_colb[:], oc_f[:])
            or_f = cst.tile([1, P], F32, name="or_f")
            nc.vector.memset(or_f[:], 1.0)
            ctx.ones_row = cst.tile([1, P], F32R, name="ones_row")
            nc.vector.tensor_copy(ctx.ones_row[:], or_f[:])
            ctx.eps_b = cst.tile([1, 1], F32, name="eps_b")
            nc.vector.memset(ctx.eps_b[:], 1e-5)

            # load x -> feature-major x^T
            x = xp.tile([P, DT * T], F32R, name="x")
            with (
                tc.tile_pool(name="iop", bufs=3) as iop,
                tc.tile_pool(name="iops", bufs=2, space="PSUM") as iops,
            ):
                for tt in range(8):  # tt = b*4 + nt
                    b, nt = divmod(tt, 4)
                    xtok = iop.tile([P, D], F32R, tag="xtok")
                    nc.sync.dma_start(xtok[:], io["x_local"][b, nt * P:(nt + 1) * P, :].bitcast(F32R))
                    for dg in range(2):
                        tps = iops.tile([P, 512], F32R, tag="xt")
                        for i in range(4):
                            dt = dg * 4 + i
                            nc.tensor.transpose(tps[:, i * P:(i + 1) * P],
                                                xtok[:, dt * P:(dt + 1) * P], ctx.ident_r[:])
                        nc.vector.tensor_copy(
                            x[:].rearrange("p (dt t) -> p dt t", dt=DT)[:, dg * 4:(dg + 1) * 4,
                                                                        b * 512 + nt * P:][:, :, :P],
                            tps[:].rearrange("p (i t) -> p i t", i=4).bitcast(F32))

            for li in range(L):
                _build_layer(ctx, li, "lin", x)
            for li in range(L):
                _build_layer(ctx, li, "conv", x)

            # write out: transpose back to token-major
            with (
                tc.tile_pool(name="oop", bufs=3) as oop,
                tc.tile_pool(name="oops", bufs=2, space="PSUM") as oops,
            ):
                for tt in range(8):
                    b, nt = divmod(tt, 4)
                    ytok = oop.tile([P, D], F32, tag="ytok")
                    for dg in range(2):
                        tps = oops.tile([P, 512], F32R, tag="yt")
                        for i in range(4):
                            dt = dg * 4 + i
                            nc.tensor.transpose(tps[:, i * P:(i + 1) * P],
                                                x[:, dt * T + b * 512 + nt * P:][:, :P],
                                                ctx.ident_r[:])
                        nc.vector.tensor_copy(ytok[:, dg * 512:(dg + 1) * 512], tps[:].bitcast(F32))
                    nc.sync.dma_start(io["y_local"][b, nt * P:(nt + 1) * P, :], ytok[:])
    nc.compile()
    return nc


_PROGRAM = None


def _get_program():
    global _PROGRAM
    if _PROGRAM is None:
        _PROGRAM = build_program()
    return _PROGRAM


def _make_in_maps(inputs):
    bf, fl, nb, nf = _blob_layout()
    in_maps = []
    for c in range(NC):
        wb = np.empty(nb, dtype=ml_dtypes.bfloat16)
        cb = np.empty(nf, dtype=np.float32)
        for entries, blob in ((bf, wb), (fl, cb)):
            for key, kind, nm, off, shape in entries:
                li = int(key[len(kind):key.index("_")])
                v = np.asarray(inputs[f"{kind}_{nm}"][li], dtype=np.float32)
                if nm in ("pk", "pv"):
                    if kind == "lin":
                        v = v[c * NL:(c + 1) * NL, :]
                    else:
                        v = v[:, c * P:(c + 1) * P, :]
                blob[off:off + int(np.prod(shape))] = v.ravel().astype(blob.dtype)
        m = {
            "x_local": np.ascontiguousarray(inputs["x"][:, c * NL:(c + 1) * NL, :], dtype=np.float32),
            "wb": wb,
            "cb": cb,
        }
        in_maps.append(m)
    return in_maps


def kernel(**inputs):
    nc = _get_program()
    in_maps = _make_in_maps(inputs)
    res = run_bass_kernel_spmd(nc, in_maps, core_ids=list(range(NC)))
    out = np.concatenate([res.results[c]["y_local"] for c in range(NC)], axis=1)
    return out.astype(np.float32)


# revision 32
# speedup vs baseline: 27.6731x; 1.0039x over previous
"""ConvLinformer forward on 8 Trainium2 NeuronCores (Bass/Tile).

Sharding: 8-way over sequence (512 tokens/core/batch). Weights replicated
(bf16), except the conv kernels [O,C,S] which are channel(C)-sliced per core;
the conv contraction is channel-sharded via AllToAll of ke/ve activations
followed by AllReduce of the k_/v_ partials (Linformer layers use the same
AllReduce for their sequence-projection partials).

Overlap strategy: collectives are split per-tensor (k vs v) and launched as
soon as their producer finishes; the q projection and the attention score
phase run while the AllReduces are in flight. Attention is two-phase: scores/
softmax/transpose for all (b,h) first (needs k only), then all AV matmuls
(needs v).

Layout: residual kept feature-major f32: x^T = [128, (dt:8, b:2, tl:512)].
All other activations and all weights bf16; PSUM accumulation f32.

Self-contained: shapes hardcoded; host shards inputs / gathers outputs.
"""

import ml_dtypes
import numpy as np

import concourse.bacc as bacc
import concourse.mybir as mybir
import concourse.tile as tile
from concourse.bass_utils import run_bass_kernel_spmd
from concourse.masks import make_identity

P = 128
B, N, D, H, DH, K, S, DFF, L = 2, 4096, 1024, 8, 128, 256, 16, 4096, 2
NC = 8
NL = N // NC          # 512 local tokens per batch
T = B * NL            # 1024 local tokens, free layout (b, tl)
DT = D // P           # 8 feature tiles
DFT = DFF // P        # 32 dff tiles
KT = K // P           # 2 kv-position tiles
SCALE = float(DH) ** -0.5

F32 = mybir.dt.float32
F32R = mybir.dt.float32r
BF16 = mybir.dt.bfloat16
AX = mybir.AxisListType
OP = mybir.AluOpType
AF = mybir.ActivationFunctionType

PARAM_NAMES = [
    "ln1_g", "ln1_b", "wq", "wk", "wv", "pk", "pv", "wo", "bo",
    "ln2_g", "ln2_b", "w1", "b1", "w2", "b2",
]
BF16_PARAMS = {"wq", "wk", "wv", "wo", "w1", "w2", "pk", "pv"}


def _param_shape(kind, nm):
    if nm in ("ln1_g", "ln1_b", "bo", "ln2_g", "ln2_b", "b2"):
        return (D,)
    if nm == "b1":
        return (DFF,)
    if nm in ("wq", "wk", "wv", "wo"):
        return (D, D)
    if nm == "w1":
        return (D, DFF)
    if nm == "w2":
        return (DFF, D)
    # pk / pv: per-core slice
    return (NL, K) if kind == "lin" else (D, P, S)


def _blob_layout():
    """Fixed packing order -> [(key, kind, nm, offset, shape)] per dtype blob."""
    bf, fl = [], []
    off_b = off_f = 0
    for kind in ("lin", "conv"):
        for li in range(L):
            for nm in PARAM_NAMES:
                shape = _param_shape(kind, nm)
                n = int(np.prod(shape))
                key = f"{kind}{li}_{nm}"
                if nm in BF16_PARAMS:
                    bf.append((key, kind, nm, off_b, shape))
                    off_b += n
                else:
                    fl.append((key, kind, nm, off_f, shape))
                    off_f += n
    return bf, fl, off_b, off_f


def _declare_io(nc):
    bf, fl, nb, nf = _blob_layout()
    d = {}
    d["x_local"] = nc.dram_tensor("x_local", [B, NL, D], F32, kind="ExternalInput").ap()
    wb = nc.dram_tensor("wb", [nb], BF16, kind="ExternalInput").ap()
    cb = nc.dram_tensor("cb", [nf], F32, kind="ExternalInput").ap()
    for key, kind, nm, off, shape in bf:
        ap = wb[off:off + int(np.prod(shape))]
        if len(shape) == 2:
            ap = ap.rearrange("(r c) -> r c", c=shape[1])
        elif len(shape) == 3:
            ap = ap.rearrange("(o c s) -> o c s", c=shape[1], s=shape[2])
        d[key] = ap
    for key, kind, nm, off, shape in fl:
        d[key] = cb[off:off + int(np.prod(shape))]
    d["y_local"] = nc.dram_tensor("y_local", [B, NL, D], F32, kind="ExternalOutput").ap()
    return d


class Ctx:
    def __init__(self, nc, tc, io):
        self.nc, self.tc, self.io = nc, tc, io


def _load_col(ctx, dram_vec, width, pool, name):
    """Load a [width*128] dram vector as a [128, width] column tile (f32)."""
    nc = ctx.nc
    t = pool.tile([P, width], F32, name=name)
    nc.sync.dma_start(t[:], dram_vec.rearrange("(w p) -> p w", p=P))
    return t


def _layernorm(ctx, x, g_col, b_col, xn, pfx):
    """xn = LN(x) * g + b -> bf16 feature-major [128, DT*T]."""
    nc, tc = ctx.nc, ctx.tc
    with (
        tc.tile_pool(name=pfx + "sb", bufs=2) as sb,
        tc.tile_pool(name=pfx + "xq", bufs=1) as xqp,
        tc.tile_pool(name=pfx + "ps", bufs=2, space="PSUM") as ps,
        tc.tile_pool(name=pfx + "bps", bufs=2, space="PSUM") as bps,
    ):
        xsq = xqp.tile([P, DT * T], BF16, name="xsq")
        xsq_v = xsq[:].rearrange("p (dt c t) -> p dt c t", dt=DT, c=2)
        x_v = x[:].bitcast(F32).rearrange("p (dt c t) -> p dt c t", dt=DT, c=2)
        for c in range(2):
            nc.gpsimd.tensor_mul(xsq_v[:, :, c], x_v[:, :, c], x_v[:, :, c])
        s0_row = sb.tile([1, T], F32R, name="s0row", bufs=1)
        s1_row = sb.tile([1, T], F32R, name="s1row", bufs=1)
        for c in range(2):  # token chunks of 512 (c == batch)
            st1 = ps.tile([1, 512], F32, tag="st")
            st2 = ps.tile([1, 512], F32, tag="st")
            for dt in range(DT):
                nc.tensor.matmul(st1[:], ctx.ones_col[:], x[:, dt * T + c * 512:][:, :512],
                                 start=(dt == 0), stop=(dt == DT - 1))
            for dt in range(DT):
                nc.tensor.matmul(st2[:], ctx.ones_colb[:], xsq[:, dt * T + c * 512:][:, :512],
                                 start=(dt == 0), stop=(dt == DT - 1))
            m_row = sb.tile([1, 512], F32, tag="m")
            nc.vector.tensor_scalar_mul(m_row[:], st1[:], 1.0 / D)
            msq = sb.tile([1, 512], F32, tag="msq")
            nc.vector.tensor_mul(msq[:], m_row[:], m_row[:])
            var = sb.tile([1, 512], F32, tag="var")
            nc.vector.scalar_tensor_tensor(var[:], st2[:], 1.0 / D, msq[:], OP.mult, OP.subtract)
            sd = sb.tile([1, 512], F32, tag="sd")
            nc.scalar.activation(sd[:], var[:], AF.Sqrt, bias=ctx.eps_b[:], scale=1.0)
            r_row = sb.tile([1, 512], F32, tag="r")
            nc.vector.reciprocal(r_row[:], sd[:])
            nc.vector.tensor_copy(s1_row[:, c * 512:][:, :512], r_row[:])
            nc.vector.scalar_tensor_tensor(
                s0_row[:, c * 512:][:, :512], m_row[:], -1.0,
                r_row[:], OP.mult, OP.mult)
        for c in range(2):
            s0bc = bps.tile([P, 512], F32, tag="bc0")
            s1bc = bps.tile([P, 512], F32, tag="bc1")
            nc.tensor.matmul(s0bc[:], ctx.ones_row[:], s0_row[:, c * 512:][:, :512],
                             start=True, stop=True)
            nc.tensor.matmul(s1bc[:], ctx.ones_row[:], s1_row[:, c * 512:][:, :512],
                             start=True, stop=True)
            # stage broadcasts in SBUF so GpSimd can help with the apply
            s0sb = sb.tile([P, 512], F32, tag="s0sb")
            nc.scalar.activation(s0sb[:], s0bc[:], AF.Copy)
            s1sb = sb.tile([P, 512], F32, tag="s1sb")
            nc.scalar.activation(s1sb[:], s1bc[:], AF.Copy)
            for dt in range(DT):
                sl = slice(dt * T + c * 512, dt * T + c * 512 + 512)
                p1 = sb.tile([P, 512], BF16, tag="p1")
                nc.vector.scalar_tensor_tensor(p1[:], x[:, sl].bitcast(F32),
                                               g_col[:, dt:dt + 1], s1sb[:], OP.mult, OP.mult)
                p2 = sb.tile([P, 512], BF16, tag="p2")
                nc.scalar.activation(p2[:], s0sb[:], AF.Identity,
                                     bias=b_col[:, dt:dt + 1], scale=g_col[:, dt:dt + 1])
                nc.gpsimd.tensor_add(xn[:, sl], p1[:], p2[:])


def _proj_T(ctx, w_dram, src, out_cb, pfx):
    """Feature-major projection: psum[ot, c] = sum_dt W[dt,ot].T @ src[dt,c]."""
    nc, tc = ctx.nc, ctx.tc
    with (
        tc.tile_pool(name=pfx + "w", bufs=DT + 1) as wp,
        tc.tile_pool(name=pfx + "ps", bufs=4, space="PSUM") as ps,
    ):
        w_sb = []
        for dt in range(DT):
            wt = wp.tile([P, D], BF16, tag="w", name=f"w{dt}")
            nc.sync.dma_start(wt[:], w_dram[dt * P:(dt + 1) * P, :])
            w_sb.append(wt)
        for c in range(2):
            for ot in range(DT):
                pp = ps.tile([P, 512], F32, tag="pj")
                for dt in range(DT):
                    nc.tensor.matmul(pp[:], w_sb[dt][:, ot * P:(ot + 1) * P],
                                     src[:, dt * T + c * 512:][:, :512],
                                     start=(dt == 0), stop=(dt == DT - 1))
                out_cb(ot, c, pp)


def _lin_kv_one(ctx, li, xn, ten, cc_in, pfx):
    """Linformer k (ten=0) or v (ten=1): full token-major, project, stage."""
    nc, tc = ctx.nc, ctx.tc
    io = ctx.io
    pre = f"lin{li}_"
    with (
        tc.tile_pool(name=pfx + "w", bufs=DT) as wp,
        tc.tile_pool(name=pfx + "kv", bufs=2) as kvp,
        tc.tile_pool(name=pfx + "p", bufs=1) as pp_,
        tc.tile_pool(name=pfx + "ar", bufs=1) as arp,
        tc.tile_pool(name=pfx + "ps", bufs=3, space="PSUM") as ps,
        tc.tile_pool(name=pfx + "ps2", bufs=3, space="PSUM") as ps2,
    ):
        pname = pre + ("pk" if ten == 0 else "pv")
        p_sb = pp_.tile([P, 4 * K], BF16, name="p_sb")
        nc.sync.dma_start(p_sb[:].rearrange("p (nt k) -> p nt k", nt=4),
                          io[pname].rearrange("(nt p) k -> p nt k", p=P))
        wname = pre + ("wk" if ten == 0 else "wv")
        w_sb = []
        for dt in range(DT):
            wt = wp.tile([P, D], BF16, tag="w", name=f"w{dt}")
            nc.sync.dma_start(wt[:], io[wname][dt * P:(dt + 1) * P, :])
            w_sb.append(wt)
        arh = arp.tile([P, 4096], BF16, tag="arh", name="arh")
        for b in range(B):
            full = kvp.tile([P, 4 * D], BF16, tag="full", name="full")
            for nt in range(4):
                for c2 in range(2):
                    fp = ps.tile([P, 512], F32, tag="pf")
                    for dt in range(DT):
                        lhs = xn[:, dt * T + b * 512 + nt * P:][:, :P]
                        nc.tensor.matmul(fp[:], lhs, w_sb[dt][:, c2 * 512:][:, :512],
                                         start=(dt == 0), stop=(dt == DT - 1))
                    nc.vector.tensor_copy(full[:, nt * D + c2 * 512:][:, :512], fp[:])
            if ten == 0:
                # k_^T partials: [dt][128, K] per b (feature-major)
                for dt in range(DT):
                    kp = ps2.tile([P, K], F32, tag="kp")
                    for nt in range(4):
                        nc.tensor.matmul(kp[:], full[:, nt * D + dt * P:][:, :P],
                                         p_sb[:, nt * K:][:, :K],
                                         start=(nt == 0), stop=(nt == 3))
                    nc.vector.tensor_copy(arh[:, b * (DT * K) + dt * K:][:, :K], kp[:])
            else:
                # v_ token-major partials: [kt][128, D] per b
                for kt in range(KT):
                    for c2 in range(2):
                        vp = ps2.tile([P, 512], F32, tag="vp")
                        for nt in range(4):
                            nc.tensor.matmul(vp[:], p_sb[:, nt * K + kt * P:][:, :P],
                                             full[:, nt * D + c2 * 512:][:, :512],
                                             start=(nt == 0), stop=(nt == 3))
                        nc.vector.tensor_copy(
                            arh[:, (b * KT + kt) * D + c2 * 512:][:, :512], vp[:])
        nc.sync.dma_start(cc_in[:], arh[:])


def _conv_wt_prep(ctx, li, ten, wtp, pfx):
    """Transpose conv kernel slice [O,128c,S] -> wt [128c, (s, o)] bf16."""
    nc, tc = ctx.nc, ctx.tc
    io = ctx.io
    pre = f"conv{li}_"
    wname = pre + ("pk" if ten == 0 else "pv")
    wt_sb = wtp.tile([P, S * D], BF16, tag=f"wt{ten}", name=f"wt{ten}")
    with (
        tc.tile_pool(name=pfx + "wn", bufs=2) as wnp,
        tc.tile_pool(name=pfx + "ps", bufs=2, space="PSUM") as ps,
    ):
        for ot in range(DT):
            wn = wnp.tile([P, P * S], BF16, tag="wn")
            nc.sync.dma_start(
                wn[:], io[wname][ot * P:(ot + 1) * P].rearrange("o c s -> o (c s)"))
            for s4 in range(4):
                tp_ps = ps.tile([P, 512], BF16, tag="wtp")
                for si in range(4):
                    s = s4 * 4 + si
                    nc.tensor.transpose(
                        tp_ps[:, si * P:(si + 1) * P],
                        wn[:].rearrange("o (c s) -> o s c", s=S)[:, s],
                        ctx.ident_b[:])
                nc.vector.tensor_copy(
                    wt_sb[:].rearrange("c (s o) -> c s o", s=S)[:, s4 * 4:(s4 + 1) * 4, ot * P:(ot + 1) * P],
                    tp_ps[:].rearrange("c (si o) -> c si o", si=4))
    return wt_sb


def _conv_kv_one(ctx, ten, a2a_out, wt_sb, cc_in, pfx):
    """Contract local c-slice of the A2A'd ke/ve with the sliced kernel."""
    nc, tc = ctx.nc, ctx.tc
    with (
        tc.tile_pool(name=pfx + "cs", bufs=1) as csp,
        tc.tile_pool(name=pfx + "ar", bufs=1) as arp,
        tc.tile_pool(name=pfx + "cps", bufs=2, space="PSUM") as cps,
    ):
        # readback: [128c, (b, peer, tl)]
        ecs = csp.tile([P, B * N], BF16, tag="ecs", name="ecs")
        nc.sync.dma_start(
            ecs[:].rearrange("c (b j t) -> c b j t", b=B, j=NC),
            a2a_out[:].rearrange("j c (b t) -> c b j t", b=B))
        arh = arp.tile([P, 4096], BF16, tag="arh", name="arh")
        if ten == 0:
            # k_^T feature-major: [ot][128, (b, K)]
            for ot in range(DT):
                kp = cps.tile([P, 512], F32, tag="ck")
                for s in range(S):
                    rhs = ecs[:].rearrange("c (b j t) -> c b j t", b=B, j=NC)[:, :, :, s::S]
                    nc.tensor.matmul(kp[:].rearrange("o (b k) -> o b k", b=B),
                                     wt_sb[:, s * D + ot * P:][:, :P], rhs,
                                     start=(s == 0), stop=(s == S - 1))
                nc.vector.tensor_copy(
                    arh[:].rearrange("p (b dt k) -> p b dt k", b=B, dt=DT)[:, :, ot, :],
                    kp[:].rearrange("o (b k) -> o b k", b=B))
        else:
            # v_ token-major: [b, kt][128, D]
            for b in range(B):
                for kt in range(KT):
                    for c2 in range(2):
                        vp = cps.tile([P, 512], F32, tag="cv")
                        for s in range(S):
                            lhs = ecs[:].rearrange(
                                "c (b j t) -> c b j t", b=B, j=NC)[:, b, kt * 4:(kt + 1) * 4, s::S]
                            nc.tensor.matmul(vp[:], lhs,
                                             wt_sb[:, s * D + c2 * 512:][:, :512],
                                             start=(s == 0), stop=(s == S - 1))
                        nc.vector.tensor_copy(
                            arh[:, (b * KT + kt) * D + c2 * 512:][:, :512],
                            vp[:])
        nc.sync.dma_start(cc_in[:], arh[:])


def _attention_scores(ctx, qo_sb, kv_k, at_all, inv_all, pfx):
    """Phase S: per (b,h): transposed scores A^T = exp(K^T q / sqrt(d)) and
    softmax denominators (needs k only). A^T lands in at_all kv-major."""
    nc, tc = ctx.nc, ctx.tc
    with (
        tc.tile_pool(name=pfx + "ps", bufs=3, space="PSUM") as ps_s,
        tc.tile_pool(name=pfx + "pd", bufs=2, space="PSUM") as ps_d,
    ):
        for b in range(B):
            for h in range(H):
                bh = b * H + h
                for kt in range(KT):
                    st_ps = ps_s.tile([P, 512], F32, tag="st")
                    nc.tensor.matmul(st_ps[:],
                                     kv_k[:, b * (DT * K) + h * K + kt * P:][:, :P],
                                     qo_sb[:, h * T + b * 512:][:, :512],
                                     start=True, stop=True)
                    nc.scalar.activation(at_all[:, (bh * KT + kt) * 512:][:, :512],
                                         st_ps[:], AF.Exp, scale=SCALE)
                den_ps = ps_d.tile([1, 512], F32, tag="den")
                for kt in range(KT):
                    nc.tensor.matmul(den_ps[:], ctx.ones_colb[:],
                                     at_all[:, (bh * KT + kt) * 512:][:, :512],
                                     start=(kt == 0), stop=(kt == KT - 1))
                with nc.allow_low_precision(reason="softmax denom broadcast via f32r matmul"):
                    nc.vector.reciprocal(inv_all[:, bh * 512:][:, :512], den_ps[:])


def _attention_av(ctx, qo_sb, kv_v, at_all, inv_all, pfx):
    """Phase O: o = A^T.T @ v per (b,h), normalized by 1/den; overwrites q."""
    nc, tc = ctx.nc, ctx.tc
    with (
        tc.tile_pool(name=pfx + "po", bufs=2, space="PSUM") as ps_o,
        tc.tile_pool(name=pfx + "pb", bufs=2, space="PSUM") as ps_b,
        tc.tile_pool(name=pfx + "ib", bufs=2) as ibp,
    ):
        for b in range(B):
            for h in range(H):
                bh = b * H + h
                oo = ps_o.tile([P, 512], F32, tag="oo")
                for kt in range(KT):
                    nc.tensor.matmul(oo[:],
                                     kv_v[:, (b * KT + kt) * D + h * P:][:, :P],
                                     at_all[:, (bh * KT + kt) * 512:][:, :512],
                                     start=(kt == 0), stop=(kt == KT - 1))
                invbc = ps_b.tile([P, 512], F32, tag="invbc")
                nc.tensor.matmul(invbc[:], ctx.ones_row[:],
                                 inv_all[:, bh * 512:][:, :512], start=True, stop=True)
                invsb = ibp.tile([P, 512], BF16, tag="invsb")
                nc.scalar.activation(invsb[:], invbc[:], AF.Copy)
                nc.vector.tensor_mul(qo_sb[:, h * T + b * 512:][:, :512], oo[:], invsb[:])


def _ffn(ctx, pre, x, xn2, clp, pfx):
    nc, tc = ctx.nc, ctx.tc
    io = ctx.io
    with (
        tc.tile_pool(name=pfx + "h", bufs=1) as hp,
        tc.tile_pool(name=pfx + "w1", bufs=DT + 2) as w1p,
        tc.tile_pool(name=pfx + "w2", bufs=2) as w2p,
        tc.tile_pool(name=pfx + "ph", bufs=4, space="PSUM") as ps_h,
        tc.tile_pool(name=pfx + "pf", bufs=3, space="PSUM") as ps_f,
    ):
        b1_col = _load_col(ctx, io[pre + "b1"], DFT, clp, "b1c")
        b2_col = _load_col(ctx, io[pre + "b2"], DT, clp, "b2c")
        h_sb = hp.tile([P, DFT * T], BF16, name="h_sb")
        for fc in range(8):
            w1_t = []
            for dt in range(DT):
                wt = w1p.tile([P, 512], BF16, tag="w1")
                nc.scalar.dma_start(
                    wt[:], io[pre + "w1"][dt * P:(dt + 1) * P, fc * 512:(fc + 1) * 512])
                w1_t.append(wt)
            for fi in range(4):
                ft = fc * 4 + fi
                for c in range(2):
                    hh = ps_h.tile([P, 512], F32, tag="hh")
                    for dt in range(DT):
                        nc.tensor.matmul(hh[:], w1_t[dt][:, fi * P:(fi + 1) * P],
                                         xn2[:, dt * T + c * 512:][:, :512],
                                         start=(dt == 0), stop=(dt == DT - 1))
                    nc.scalar.activation(h_sb[:, ft * T + c * 512:][:, :512], hh[:],
                                         AF.Gelu, bias=b1_col[:, ft:ft + 1], scale=1.0)
        for c in range(2):  # c-outer: chunk 0 of x finishes early for next LN
            for ot in range(DT):
                w2t = w2p.tile([P, DFT * P], BF16, tag="w2")
                nc.scalar.dma_start(
                    w2t[:].rearrange("p (ft o) -> p ft o", ft=DFT),
                    io[pre + "w2"][:, ot * P:(ot + 1) * P].rearrange("(ft p) o -> p ft o", p=P))
                ff = ps_f.tile([P, 512], F32, tag="ff")
                for ft in range(DFT):
                    nc.tensor.matmul(ff[:], w2t[:, ft * P:(ft + 1) * P],
                                     h_sb[:, ft * T + c * 512:][:, :512],
                                     start=(ft == 0), stop=(ft == DFT - 1))
                sl = slice(ot * T + c * 512, ot * T + c * 512 + 512)
                nc.vector.scalar_tensor_tensor(x[:, sl], ff[:],
                                               b2_col[:, ot:ot + 1], x[:, sl].bitcast(F32),
                                               OP.add, OP.add)


def _build_layer(ctx, li, kind, x):
    nc, tc = ctx.nc, ctx.tc
    io = ctx.io
    pre = f"{kind}{li}_"
    pfx = pre
    rg = [list(range(NC))]
    with (
        tc.tile_pool(name=pfx + "cl", bufs=1) as clp,
        tc.tile_pool(name=pfx + "wa", bufs=1) as wap,
        tc.tile_pool(name=pfx + "dram", bufs=1, space="DRAM") as dp,
    ):
        g1_col = _load_col(ctx, io[pre + "ln1_g"], DT, clp, "g1c")
        b1c_col = _load_col(ctx, io[pre + "ln1_b"], DT, clp, "b1cc")
        bo_col = _load_col(ctx, io[pre + "bo"], DT, clp, "boc")

        xn = wap.tile([P, DT * T], BF16, tag="workA", name="xn")
        _layernorm(ctx, x, g1_col, b1c_col, xn, pfx + "ln1")

        cc_k_in = dp.tile([P, 4096], BF16, tag="cck_in", name="cck_in")
        cc_k_out = dp.tile([P, 4096], BF16, tag="cck_out", name="cck_out", addr_space="Shared")
        cc_v_in = dp.tile([P, 4096], BF16, tag="ccv_in", name="ccv_in")
        cc_v_out = dp.tile([P, 4096], BF16, tag="ccv_out", name="ccv_out", addr_space="Shared")

        with tc.tile_pool(name=pfx + "qo", bufs=1) as qop:
            qo_sb = qop.tile([P, DT * T], BF16, tag="qo", name="qo_sb")

            def q_cb(ot, c, pp):
                nc.vector.tensor_copy(qo_sb[:, ot * T + c * 512:][:, :512], pp[:])

            if kind == "lin":
                # k partials -> AR_k; v partials -> AR_v; then q (overlaps ARs)
                _lin_kv_one(ctx, li, xn, 0, cc_k_in, pfx + "kvk")
                nc.gpsimd.collective_compute(
                    "AllReduce", OP.add, replica_groups=rg,
                    ins=[cc_k_in[:]], outs=[cc_k_out[:]])
                _lin_kv_one(ctx, li, xn, 1, cc_v_in, pfx + "kvv")
                nc.gpsimd.collective_compute(
                    "AllReduce", OP.add, replica_groups=rg,
                    ins=[cc_v_in[:]], outs=[cc_v_out[:]])
                _proj_T(ctx, io[pre + "wq"], xn, q_cb, pfx + "q")
            else:
                a2a_k_in = dp.tile([NC, P, T], BF16, tag="a2ak_in", name="a2ak_in")
                a2a_k_out = dp.tile([NC, P, T], BF16, tag="a2ak_out", name="a2ak_out")
                a2a_v_in = dp.tile([NC, P, T], BF16, tag="a2av_in", name="a2av_in")
                a2a_v_out = dp.tile([NC, P, T], BF16, tag="a2av_out", name="a2av_out")
                with (
                    tc.tile_pool(name=pfx + "kest", bufs=3) as ksp,
                    tc.tile_pool(name=pfx + "wt", bufs=1) as wtp,
                ):
                    def mk_cb(dst):
                        def cb(ot, c, pp):
                            st = ksp.tile([P, 512], BF16, tag="kest", name="kest")
                            nc.vector.tensor_copy(st[:], pp[:])
                            nc.sync.dma_start(dst[ot, :, c * 512:(c + 1) * 512], st[:])
                        return cb
                    _proj_T(ctx, io[pre + "wk"], xn, mk_cb(a2a_k_in), pfx + "ke")
                    nc.gpsimd.collective_compute(
                        "AllToAll", OP.bypass, replica_groups=rg,
                        ins=[a2a_k_in[:]], outs=[a2a_k_out[:]])
                    _proj_T(ctx, io[pre + "wv"], xn, mk_cb(a2a_v_in), pfx + "ve")
                    nc.gpsimd.collective_compute(
                        "AllToAll", OP.bypass, replica_groups=rg,
                        ins=[a2a_v_in[:]], outs=[a2a_v_out[:]])
                    _proj_T(ctx, io[pre + "wq"], xn, q_cb, pfx + "q")
                    wt_k = _conv_wt_prep(ctx, li, 0, wtp, pfx + "wpk")
                    wt_v = _conv_wt_prep(ctx, li, 1, wtp, pfx + "wpv")
                    _conv_kv_one(ctx, 0, a2a_k_out, wt_k, cc_k_in, pfx + "ckk")
                    nc.gpsimd.collective_compute(
                        "AllReduce", OP.add, replica_groups=rg,
                        ins=[cc_k_in[:]], outs=[cc_k_out[:]])
                    _conv_kv_one(ctx, 1, a2a_v_out, wt_v, cc_v_in, pfx + "ckv")
                    nc.gpsimd.collective_compute(
                        "AllReduce", OP.add, replica_groups=rg,
                        ins=[cc_v_in[:]], outs=[cc_v_out[:]])

            with (
                tc.tile_pool(name=pfx + "kvp", bufs=1) as kvp,
                tc.tile_pool(name=pfx + "atp", bufs=1) as atp,
            ):
                kv_k = kvp.tile([P, 4096], BF16, tag="kvk", name="kv_k")
                nc.sync.dma_start(kv_k[:], cc_k_out[:])
                at_all = atp.tile([P, B * H * KT * 512], BF16, name="at_all")
                inv_all = atp.tile([1, B * H * 512], F32R, name="inv_all")
                _attention_scores(ctx, qo_sb, kv_k, at_all, inv_all, pfx + "atS")

                kv_v = kvp.tile([P, 4096], BF16, tag="kvv", name="kv_v")
                nc.sync.dma_start(kv_v[:], cc_v_out[:])
                _attention_av(ctx, qo_sb, kv_v, at_all, inv_all, pfx + "atO")

            def wo_cb(ot, c, pp):
                sl = slice(ot * T + c * 512, ot * T + c * 512 + 512)
                nc.vector.scalar_tensor_tensor(x[:, sl], pp[:],
                                               bo_col[:, ot:ot + 1], x[:, sl].bitcast(F32),
                                               OP.add, OP.add)

            _proj_T(ctx, io[pre + "wo"], qo_sb, wo_cb, pfx + "wo")

        g2_col = _load_col(ctx, io[pre + "ln2_g"], DT, clp, "g2c")
        b2c_col = _load_col(ctx, io[pre + "ln2_b"], DT, clp, "b2cc")
        xn2 = wap.tile([P, DT * T], BF16, tag="workA", name="xn2")
        _layernorm(ctx, x, g2_col, b2c_col, xn2, pfx + "ln2")
        _ffn(ctx, pre, x, xn2, clp, pfx + "ffn")


def build_program():
    nc = bacc.Bacc("TRN2", target_bir_lowering=False, debug=False, num_devices=NC)
    io = _declare_io(nc)
    with tile.TileContext(nc) as tc:
        with (
            tc.tile_pool(name="cst", bufs=1) as cst,
            tc.tile_pool(name="xp", bufs=1) as xp,
        ):
            ctx = Ctx(nc, tc, io)
            ident_f = cst.tile([P, P], F32, name="ident_f")
            make_identity(nc, ident_f[:])
            ctx.ident_r = cst.tile([P, P], F32R, name="ident_r")
            nc.vector.tensor_copy(ctx.ident_r[:], ident_f[:])
            ctx.ident_b = cst.tile([P, P], BF16, name="ident_b")
            nc.vector.tensor_copy(ctx.ident_b[:], ident_f[:])
            oc_f = cst.tile([P, 1], F32, name="oc_f")
            nc.vector.memset(oc_f[:], 1.0)
            ctx.ones_col = cst.tile([P, 1], F32R, name="ones_col")
            nc.vector.tensor_copy(ctx.ones_col[:], oc_f[:])
            ctx.ones_colb = cst.tile([P, 1], BF16, name="ones_colb")
            nc.vector.tensor_copy(ctx.ones_colb[:], oc_f[:])
            or_f = cst.tile([1, P], F32, name="or_f")
            nc.vector.memset(or_f[:], 1.0)
            ctx.ones_row = cst.tile([1, P], F32R, name="ones_row")
            nc.vector.tensor_copy(ctx.ones_row[:], or_f[:])
            ctx.eps_b = cst.tile([1, 1], F32, name="eps_b")
            nc.vector.memset(ctx.eps_b[:], 1e-5)

            # load x -> feature-major x^T
            x = xp.tile([P, DT * T], F32R, name="x")
            with (
                tc.tile_pool(name="iop", bufs=3) as iop,
                tc.tile_pool(name="iops", bufs=2, space="PSUM") as iops,
            ):
                for tt in range(8):  # tt = b*4 + nt
                    b, nt = divmod(tt, 4)
                    xtok = iop.tile([P, D], F32R, tag="xtok")
                    nc.sync.dma_start(xtok[:], io["x_local"][b, nt * P:(nt + 1) * P, :].bitcast(F32R))
                    for dg in range(2):
                        tps = iops.tile([P, 512], F32R, tag="xt")
                        for i in range(4):
                            dt = dg * 4 + i
                            nc.tensor.transpose(tps[:, i * P:(i + 1) * P],
                                                xtok[:, dt * P:(dt + 1) * P], ctx.ident_r[:])
                        nc.vector.tensor_copy(
                            x[:].rearrange("p (dt t) -> p dt t", dt=DT)[:, dg * 4:(dg + 1) * 4,
                                                                        b * 512 + nt * P:][:, :, :P],
                            tps[:].rearrange("p (i t) -> p i t", i=4).bitcast(F32))

            for li in range(L):
                _build_layer(ctx, li, "lin", x)
            for li in range(L):
                _build_layer(ctx, li, "conv", x)

            # write out: transpose back to token-major
            with (
                tc.tile_pool(name="oop", bufs=3) as oop,
                tc.tile_pool(name="oops", bufs=2, space="PSUM") as oops,
            ):
                for tt in range(8):
                    b, nt = divmod(tt, 4)
                    ytok = oop.tile([P, D], F32, tag="ytok")
                    for dg in range(2):
                        tps = oops.tile([P, 512], F32R, tag="yt")
                        for i in range(4):
                            dt = dg * 4 + i
                            nc.tensor.transpose(tps[:, i * P:(i + 1) * P],
                                                x[:, dt * T + b * 512 + nt * P:][:, :P],
                                                ctx.ident_r[:])
                        nc.vector.tensor_copy(ytok[:, dg * 512:(dg + 1) * 512], tps[:].bitcast(F32))
                    nc.sync.dma_start(io["y_local"][b, nt * P:(nt + 1) * P, :], ytok[:])
    nc.compile()
    return nc


_PROGRAM = None


def _get_program():
    global _PROGRAM
    if _PROGRAM is None:
        _PROGRAM = build_program()
    return _PROGRAM


def _make_in_maps(inputs):
    bf, fl, nb, nf = _blob_layout()
    in_maps = []
    for c in range(NC):
        wb = np.empty(nb, dtype=ml_dtypes.bfloat16)
        cb = np.empty(nf, dtype=np.float32)
        for entries, blob in ((bf, wb), (fl, cb)):
            for key, kind, nm, off, shape in entries:
                li = int(key[len(kind):key.index("_")])
                v = np.asarray(inputs[f"{kind}_{nm}"][li], dtype=np.float32)
                if nm in ("pk", "pv"):
                    if kind == "lin":
                        v = v[c * NL:(c + 1) * NL, :]
                    else:
                        v = v[:, c * P:(c + 1) * P, :]
                blob[off:off + int(np.prod(shape))] = v.ravel().astype(blob.dtype)
        m = {
            "x_local": np.ascontiguousarray(inputs["x"][:, c * NL:(c + 1) * NL, :], dtype=np.float32),
            "wb": wb,
            "cb": cb,
        }
        in_maps.append(m)
    return in_maps


def kernel(**inputs):
    nc = _get_program()
    in_maps = _make_in_maps(inputs)
    res = run_bass_kernel_spmd(nc, in_maps, core_ids=list(range(NC)))
    out = np.concatenate([res.results[c]["y_local"] for c in range(NC)], axis=1)
    return out.astype(np.float32)
